# revision 24
# baseline (speedup 1.0000x reference)
"""Self-contained Trainium2 Bass kernel for a 2-layer GAT + BatchNorm + graph pooling.

Contract: kernel(**inputs) takes the FULL (unsharded) inputs and returns the
FULL [G, 1024] float32 output.

v3 design: replicated dense layers, fp16 gather tables, tiny overlapped
collectives, channel-interleaved layout for fast DVE broadcasts.

  - Channels are stored (c, h)-interleaved (dev channel c*4+h = torch channel
    h*64+c, permuted host-side in the weights and un-permuted in postprocess)
    so every per-head broadcast multiply has a packed innermost dim of 4 —
    DVE runs these at 16-bit double rate instead of broadcast-stride-0 rate.
  - dense L1 is REPLICATED: every core computes the full table
    T1b[n] = [h(256 fp16) | al_src(4 f32 riding as 8 fp16 slots) | pad]
    (768 B rows) from x; writes are batched 8 blocks per DMA (the HWDGE
    ~600 ns fixed cost per dma_start dominated v2's dense phases).
  - "al_dst" logits live in a plain [N, 4] f32 table whose gather view is
    [N/16, 64] (256 B rows = the dma_gather minimum); the per-edge value is
    extracted with a one-hot-over-16 dot on DVE.  This keeps gather indices
    (node//16) inside int16 and lets the replicated dense write it cheaply.
  - edge phase (per 128-dst-node block): one combined index/metadata DMA,
    dma_gather rows by src (lo/hi split for int16; <=1024 idxs per
    instruction — 2048 hangs the HW, verified), -1-padded index streams skip
    pad transfers (per-core valid counts are reg_load-ed from SBUF),
    softmax-weighted segment sum via 0/1 fp16 selector-matrix matmuls
    accumulating [out | denom] in PSUM.  Logits stay f32 (exp via ACT,
    clamped at +8 so pad garbage cannot overflow fp16).  The per-node
    epilogue folds bias+relu into the post-transpose ACT copy.
  - between layers only x1's pre-BN value g1 (256ch fp16, transposed) is
    AllGathered — 3.2 MB per rank in 4 column-chunks issued as edge L1
    drains, overlapping wire time with edge compute.  BN stats go through a
    [128,4] AllReduce; the BN affine + relu is fused into dense L2's
    activation load.  dense L2 is replicated from the gathered chunks.
  - pooling: per-channel-tile segmented running sum & max along the node
    axis (tensor_tensor_scan); host reads each graph's last column and
    combines the <=2 per-graph partials from adjacent cores.
"""

import numpy as np

import concourse.bass as bass
import concourse.bacc as bacc
import concourse.tile as tile
from concourse import mybir
from concourse import bass_utils
from concourse.masks import make_identity

F32 = mybir.dt.float32
F16 = mybir.dt.float16
I16 = mybir.dt.int16
I32 = mybir.dt.int32
ALU = mybir.AluOpType
ACTF = mybir.ActivationFunctionType

# problem constants (hardcoded per the harness contract)
N, F_IN, C0, C1, H, E, G = 50000, 128, 64, 64, 4, 800000, 256
HC = H * C0            # 256
NEG_SLOPE = 0.2
BN_EPS = 1e-5
NCORES = 8
NPC = N // NCORES      # nodes per core (6250)
SPLIT = 32768          # dma_gather int16 index limit -> split gather table
RWH = 384              # fp16 table row width (768 B): h(256) + al_src(8) + pad
PART = 128
NPAD = 50048           # N rounded to 128 blocks (391 blocks)
NBLK = NPAD // PART    # 391 dense blocks
CHUNK = 8              # 128-idx groups per dma_gather (1024 idx HW limit)
ZCLAMP = 8.0           # logit clamp (real logits ~ +-6); keeps exp fp16-finite
NAGC = 4               # AllGather column-chunks for the inter-layer feature
GRP = 8                # dense blocks batched per table-row DMA

# dev channel k = c*4+h  <->  torch channel h*64+c
PERM = np.array([(k % H) * C0 + k // H for k in range(HC)], np.int64)

PHASES = 6             # build phases 1..6 (bisection aid)


# --------------------------------------------------------------------------
# host-side preprocessing
# --------------------------------------------------------------------------

def _pack16(stream_i16, ncols):
    """dma_gather index layout: position i -> [i%16, i//16], replicated to
    partition groups 16k+p for the 8 Q7 cores."""
    base = stream_i16.reshape(ncols, 16).T          # [16, ncols]
    return np.tile(base, (8, 1)).astype(np.int16)   # [128, ncols]


def _pad_stream(vals, nslots):
    """Pad an index stream to nslots with -1 (skipped by dma_gather) and
    return (idx_i16, per-1024-chunk valid counts).  A chunk with zero valid
    indices gets one dummy index 0 (count 1): the HW needs at least one
    non-negative index per instruction."""
    n = len(vals)
    out = np.full(nslots, -1, np.int16)
    out[:n] = vals
    counts = []
    for c0 in range(0, nslots, CHUNK * PART):
        span = min(CHUNK * PART, nslots - c0)
        cnt = min(max(n - c0, 0), span)
        if cnt == 0:
            out[c0] = 0
            cnt = 1
        counts.append(cnt)
    return out, counts


def preprocess(x, edge_index, batch,
               W1, att_src1, att_dst1, b1, gamma, beta,
               W2, att_src2, att_dst2, b2):
    x = np.asarray(x, np.float32)
    edge_index = np.asarray(edge_index)
    batch = np.asarray(batch).astype(np.int64)
    W1 = np.asarray(W1, np.float32); W2 = np.asarray(W2, np.float32)

    src = np.concatenate([edge_index[0], np.arange(N, dtype=np.int64)])
    dst = np.concatenate([edge_index[1], np.arange(N, dtype=np.int64)])

    NB = (NPC + PART - 1) // PART                      # dst blocks per core

    # ---- per-core edge streams ----
    blocks = []
    nlo_max = nhi_max = 0
    for r in range(NCORES):
        m = (dst >= r * NPC) & (dst < (r + 1) * NPC)
        s_r = src[m]; d_r = dst[m]
        dloc = d_r - r * NPC
        order = np.argsort(dloc, kind="stable")
        s_r = s_r[order]; d_r = d_r[order]; dloc = dloc[order]
        blk = dloc // PART
        core_blocks = []
        for b in range(NB):
            bm = blk == b
            sb_ = s_r[bm]; db_ = d_r[bm]
            lo_m = sb_ < SPLIT
            core_blocks.append((sb_[lo_m], sb_[~lo_m] - SPLIT,
                                db_[lo_m], db_[~lo_m]))
            nlo_max = max(nlo_max, int(lo_m.sum()))
            nhi_max = max(nhi_max, int((~lo_m).sum()))
        blocks.append(core_blocks)

    KLO = max(1, (nlo_max + PART - 1) // PART)
    KHI = max(1, (nhi_max + PART - 1) // PART)
    KT = KLO + KHI
    CL = (KLO + CHUNK - 1) // CHUNK
    CH = (KHI + CHUNK - 1) // CHUNK
    IDXW = KT * 18       # [il|ih|ial|iah] (KT*16) + dl (KT) + dm (KT), i16

    ib_t = np.zeros((NCORES, NB, PART, IDXW), np.int16)
    cnt_t = np.zeros((NCORES, NB, 8), np.int32)
    for r in range(NCORES):
        for b in range(NB):
            lo_src, hi_src, abs_lo, abs_hi = blocks[r][b]
            ls, c_lo = _pad_stream(lo_src, KLO * PART)
            hs, c_hi = _pad_stream(hi_src, KHI * PART)
            al_lo, _ = _pad_stream(abs_lo // 16, KLO * PART)
            al_hi, _ = _pad_stream(abs_hi // 16, KHI * PART)
            ib_t[r, b, :, 0:KLO * 8] = _pack16(ls, KLO * 8)
            ib_t[r, b, :, KLO * 8:KT * 8] = _pack16(hs, KHI * 8)
            ib_t[r, b, :, KT * 8:KT * 8 + KLO * 8] = _pack16(al_lo, KLO * 8)
            ib_t[r, b, :, KT * 8 + KLO * 8:KT * 16] = _pack16(al_hi, KHI * 8)
            cnt_t[r, b, :CL] = c_lo
            cnt_t[r, b, CL:CL + CH] = c_hi
            dl = np.full(KT * PART, 999.0, np.float32)
            dm = np.zeros(KT * PART, np.float32)
            dl[:len(abs_lo)] = (abs_lo - r * NPC) % PART
            dm[:len(abs_lo)] = abs_lo % 16
            dl[KLO * PART:KLO * PART + len(abs_hi)] = (abs_hi - r * NPC) % PART
            dm[KLO * PART:KLO * PART + len(abs_hi)] = abs_hi % 16
            ib_t[r, b, :, KT * 16:KT * 17] = \
                dl.reshape(KT, PART).T.astype(np.float16).view(np.int16)
            ib_t[r, b, :, KT * 17:KT * 18] = \
                dm.reshape(KT, PART).T.astype(np.float16).view(np.int16)

    # ---- batch-derived pooling metadata ----
    counts = np.bincount(batch, minlength=G).astype(np.float64)
    maskrow = np.zeros((NCORES, 1, NPC), np.float32)
    cinvrow = np.zeros((NCORES, 1, NPC), np.float32)
    lastcol = [dict() for _ in range(NCORES)]
    for r in range(NCORES):
        bseg = batch[r * NPC:(r + 1) * NPC]
        same = np.ones(NPC, np.float32)
        same[0] = 0.0
        same[1:] = (bseg[1:] == bseg[:-1]).astype(np.float32)
        maskrow[r, 0] = same
        cinvrow[r, 0] = (1.0 / np.maximum(counts[bseg], 1.0)).astype(np.float32)
        gids, last_idx = np.unique(bseg[::-1], return_index=True)
        for g_, li in zip(gids, last_idx):
            lastcol[r][int(g_)] = NPC - 1 - int(li)

    # ---- weights (replicated; channel-permuted to dev order) ----
    def bmat(W, a_s, a_d, fin):
        Wr = W.reshape(fin, H, C0)
        bs = np.einsum("khc,hc->kh", Wr, np.asarray(a_s, np.float32))
        bd = np.einsum("khc,hc->kh", Wr, np.asarray(a_d, np.float32))
        return np.concatenate([bs, bd], axis=1).astype(np.float16)  # [fin, 8]

    xh = np.zeros((F_IN, NPAD), np.float16)
    xh[:, :N] = x.T.astype(np.float16)

    W1p = W1[:, PERM]
    W2p = W2[PERM][:, PERM]
    b1p = np.asarray(b1, np.float32)[PERM]
    b2p = np.asarray(b2, np.float32)[PERM]

    shared = dict(
        xh16T=xh,
        W1h=W1p.astype(np.float16), B1h=bmat(W1, att_src1, att_dst1, F_IN),
        W2h=W2p.astype(np.float16),
        B2h=bmat(W2, att_src2, att_dst2, HC)[PERM, :],
        b1colT=b1p.reshape(2, PART).T.copy(),
        b2colT=b2p.reshape(2, PART).T.copy(),
        gcol=np.asarray(gamma, np.float32)[PERM].reshape(2, PART).T.copy(),
        bcol=np.asarray(beta, np.float32)[PERM].reshape(2, PART).T.copy(),
    )
    in_maps = []
    for r in range(NCORES):
        in_maps.append(dict(
            shared,
            ib=ib_t[r],
            cnts=cnt_t[r].reshape(1, NB * 8),
            maskrow=maskrow[r],
            cinvrow=cinvrow[r],
        ))
    meta = dict(NB=NB, KLO=KLO, KHI=KHI, KT=KT, CL=CL, CH=CH,
                lastcol=lastcol, counts=counts)
    return in_maps, meta


# --------------------------------------------------------------------------
# device program
# --------------------------------------------------------------------------

def build_program(meta, sim_local=False):
    NB, KLO, KHI, KT = meta["NB"], meta["KLO"], meta["KHI"], meta["KT"]
    CL, CH = meta["CL"], meta["CH"]
    IDXW = KT * 18
    nc = bacc.Bacc("TRN2", target_bir_lowering=False, debug=False,
                   num_devices=1 if sim_local else NCORES)

    def ein(name, shape, dt=F32):
        return nc.dram_tensor(name, list(shape), dt, kind="ExternalInput").ap()

    xh_d = ein("xh16T", [F_IN, NPAD], F16)
    W1_d = ein("W1h", [F_IN, HC], F16); B1_d = ein("B1h", [F_IN, 8], F16)
    W2_d = ein("W2h", [HC, HC], F16);   B2_d = ein("B2h", [HC, 8], F16)
    b1c_d = ein("b1colT", [PART, 2]); b2c_d = ein("b2colT", [PART, 2])
    gcol_d = ein("gcol", [PART, 2]); bcol_d = ein("bcol", [PART, 2])
    ib_d = ein("ib", [NB, PART, IDXW], I16)
    cnt_d = ein("cnts", [1, NB * 8], I32)
    mask_d = ein("maskrow", [1, NPC])
    cinv_d = ein("cinvrow", [1, NPC])

    omax_d = nc.dram_tensor("out_max", [4 * PART, NPC], F32, kind="ExternalOutput").ap()
    omean_d = nc.dram_tensor("out_mean", [4 * PART, NPC], F32, kind="ExternalOutput").ap()

    # internal DRAM
    T1b = nc.dram_tensor("T1b", [NPAD, RWH], F16).ap()
    T2b = nc.dram_tensor("T2b", [NPAD, RWH], F16).ap()
    al1pk = nc.dram_tensor("al1pk", [NPAD, 4], F32).ap()
    al2pk = nc.dram_tensor("al2pk", [NPAD, 4], F32).ap()
    x2T = nc.dram_tensor("x2T", [HC, NPC], F16).ap()
    ar_in = nc.dram_tensor("ar_in", [PART, 4], F32).ap()
    ar_out = nc.dram_tensor("ar_out", [PART, 4], F32, addr_space="Shared").ap()

    # AllGather chunks of the inter-layer feature (transposed, fp16)
    bpc = (NB + NAGC - 1) // NAGC
    blk_of_chunk = [list(range(c * bpc, min(NB, (c + 1) * bpc)))
                    for c in range(NAGC)]
    chunk_cols = []
    g1h_c, Tag_c = [], []
    for c in range(NAGC):
        c0 = blk_of_chunk[c][0] * PART
        c1 = min(NPC, (blk_of_chunk[c][-1] + 1) * PART)
        chunk_cols.append((c0, c1))
        g1h_c.append(nc.dram_tensor(f"g1h_{c}", [HC, c1 - c0], F16).ap())
        Tag_c.append(nc.dram_tensor(f"Tag_{c}", [NCORES * HC, c1 - c0], F16,
                                    addr_space="Shared").ap())

    rgroups = [list(range(NCORES))]

    class _PhaseStopE(Exception):
        pass

    with tile.TileContext(nc) as tc:
      try:
        # ---------- shared constant tiles ----------
        with tc.tile_pool(name="const", bufs=1) as cpool:
            ident = cpool.tile([PART, PART], F32)
            make_identity(nc, ident[:])
            iota_i = cpool.tile([PART, PART], mybir.dt.int32)
            nc.gpsimd.iota(iota_i[:], pattern=[[1, PART]], base=0,
                           channel_multiplier=0)
            iota_h = cpool.tile([PART, PART], F16)
            nc.vector.tensor_copy(out=iota_h[:], in_=iota_i[:])
            iota16 = cpool.tile([PART, 16], F16)
            nc.vector.tensor_copy(out=iota16[:], in_=iota_i[:, 0:16])

            cnt_sb = cpool.tile([1, NB * 8], I32)
            nc.sync.dma_start(out=cnt_sb[:], in_=cnt_d[:, :])
            b1cv = cpool.tile([PART, 2], F32)
            nc.sync.dma_start(out=b1cv[:], in_=b1c_d[:, :])
            b2cv = cpool.tile([PART, 2], F32)
            nc.sync.dma_start(out=b2cv[:], in_=b2c_d[:, :])

            # ---------- dense L1 (replicated: full table on every core) ----
            _sc = nc.enter_named_scope("dense1", False)[0]
            with tc.tile_pool(name="d1", bufs=2) as dp, \
                 tc.tile_pool(name="d1w", bufs=1) as wp, \
                 tc.tile_pool(name="d1x", bufs=2) as xp, \
                 tc.tile_pool(name="d1ps", bufs=2, space="PSUM") as pp:
                W1_sb = wp.tile([F_IN, HC], F16)
                nc.sync.dma_start(out=W1_sb[:], in_=W1_d[:, :])
                B1_sb = wp.tile([F_IN, 8], F16)
                nc.sync.dma_start(out=B1_sb[:], in_=B1_d[:, :])
                XCH = 6272                      # x column chunk (49 blocks)
                x_sb = None
                row8 = al8 = None
                for b in range(NBLK):
                    if b % 49 == 0:
                        x_sb = xp.tile([F_IN, XCH], F16, tag="xsb")
                        x0 = b * PART
                        nc.sync.dma_start(out=x_sb[:, 0:min(XCH, NPAD - x0)],
                                          in_=xh_d[:, x0:min(x0 + XCH, NPAD)])
                    k = b % GRP
                    if k == 0:
                        row8 = dp.tile([PART, GRP, RWH], F16, tag="row8")
                        al8 = dp.tile([PART, GRP, 4], F32, tag="al8")
                    col = (b % 49) * PART
                    ps = pp.tile([PART, 264], F32, tag="dps")
                    nc.tensor.matmul(ps[:, 0:HC], lhsT=x_sb[:, col:col + PART],
                                     rhs=W1_sb[:], start=True, stop=True)
                    nc.tensor.matmul(ps[:, HC:HC + 8], lhsT=x_sb[:, col:col + PART],
                                     rhs=B1_sb[:], start=True, stop=True)
                    nc.scalar.activation(out=row8[:, k, 0:HC], in_=ps[:, 0:HC],
                                         func=ACTF.Copy)
                    nc.vector.tensor_copy(out=row8[:, k, HC:HC + 8].bitcast(F32),
                                          in_=ps[:, HC:HC + 4])
                    nc.vector.tensor_copy(out=al8[:, k, :], in_=ps[:, HC + 4:HC + 8])
                    if k == GRP - 1 or b == NBLK - 1:
                        ng = k + 1
                        n0 = (b - k) * PART
                        nc.sync.dma_start(
                            out=T1b[n0:n0 + ng * PART, :].rearrange(
                                "(k p) w -> p k w", p=PART),
                            in_=row8[:, 0:ng, :])
                        nc.sync.dma_start(
                            out=al1pk[n0:n0 + ng * PART, :].rearrange(
                                "(k p) c -> p k c", p=PART),
                            in_=al8[:, 0:ng, :])
            nc.leave_named_scope("dense1", _sc, False)

            # ---------- edge phase (shared for both layers) ----------
            def edge_phase(Tbl, alpk, bias_cv, outT, relu, scope, ag=False):
                """outT: None for L1 (writes g1h chunks), else x2T."""
                _es = nc.enter_named_scope(scope, False)[0]
                alview = alpk[:, :].rearrange("(r j) c -> r (j c)", j=16)
                with tc.tile_pool(name="eidx", bufs=2) as ip, \
                     tc.tile_pool(name="eg", bufs=2) as gp, \
                     tc.tile_pool(name="ew", bufs=2) as wp2, \
                     tc.tile_pool(name="eps", bufs=2, space="PSUM") as ep, \
                     tc.tile_pool(name="etps", bufs=2, space="PSUM") as tps:
                    # pre-zero both gather buffers: -1-skipped slots must hold
                    # finite floats (uninitialized SBUF could be NaN -> NaN*0
                    # = NaN in PSUM)
                    for _z in range(2):
                        for tg, shp, dt_ in (("glo", [PART, KLO, RWH], F16),
                                             ("ghi", [PART, KHI, RWH], F16),
                                             ("ga", [PART, KT, 64], F32)):
                            zt = gp.tile(shp, dt_, tag=tg)
                            nc.vector.memset(zt[:], 0.0)

                    cnt_regs = [nc.gpsimd.alloc_register(f"cnt_{scope}_{i}")
                                for i in range(4)]
                    reg_rr = [0]

                    def gather(gtile, src_ap, ixtile, ktot, elem, cnt_base):
                        for ci, c0 in enumerate(range(0, ktot, CHUNK)):
                            cw = min(CHUNK, ktot - c0)
                            reg = cnt_regs[reg_rr[0] % 4]
                            reg_rr[0] += 1
                            nc.gpsimd.reg_load(
                                reg, cnt_sb[0:1, cnt_base + ci:cnt_base + ci + 1])
                            nc.gpsimd.dma_gather(
                                out_ap=gtile[:, c0:c0 + cw, :],
                                in_ap=src_ap, idxs_ap=ixtile[:, c0 * 8:(c0 + cw) * 8],
                                num_idxs=cw * PART, num_idxs_reg=reg,
                                elem_size=elem)

                    for b in range(NB):
                        mb = min(PART, NPC - b * PART)
                        ib = ip.tile([PART, IDXW], I16, tag="ib")
                        nc.sync.dma_start(out=ib[:], in_=ib_d[b, :, :])
                        il = ib[:, 0:KLO * 8]
                        ih = ib[:, KLO * 8:KT * 8]
                        ial = ib[:, KT * 8:KT * 8 + KLO * 8]
                        iah = ib[:, KT * 8 + KLO * 8:KT * 16]
                        dl = ib[:, KT * 16:KT * 17].bitcast(F16)
                        dm = ib[:, KT * 17:KT * 18].bitcast(F16)

                        glo = gp.tile([PART, KLO, RWH], F16, tag="glo")
                        gather(glo, Tbl[0:SPLIT, :], il, KLO, RWH, b * 8)
                        ghi = gp.tile([PART, KHI, RWH], F16, tag="ghi")
                        gather(ghi, Tbl[SPLIT:NPAD, :], ih, KHI, RWH, b * 8 + CL)
                        ga = gp.tile([PART, KT, 64], F32, tag="ga")
                        gather(ga[:, 0:KLO, :], alview, ial, KLO, 64, b * 8)
                        gather(ga[:, KLO:KT, :], alview, iah, KHI, 64, b * 8 + CL)

                        # selector matrix S01[e, kt, d] = (dl == d), fp16
                        S01 = wp2.tile([PART, KT, PART], F16, tag="S01")
                        nc.vector.tensor_tensor(
                            out=S01[:],
                            in0=dl[:].unsqueeze(-1).to_broadcast([PART, KT, PART]),
                            in1=iota_h[:].unsqueeze(1).to_broadcast([PART, KT, PART]),
                            op=ALU.is_equal)

                        # al_dst extraction: one-hot over the 16-node pack
                        oh = wp2.tile([PART, KT, 16], F32, tag="oh")
                        nc.vector.tensor_tensor(
                            out=oh[:],
                            in0=dm[:].unsqueeze(-1).to_broadcast([PART, KT, 16]),
                            in1=iota16[:].unsqueeze(1).to_broadcast([PART, KT, 16]),
                            op=ALU.is_equal)
                        adp = wp2.tile([PART, KT, 4, 16], F32, tag="adp")
                        nc.vector.tensor_tensor(
                            out=adp[:],
                            in0=ga[:].rearrange("p k (j h) -> p k h j", j=16),
                            in1=oh[:].unsqueeze(2).to_broadcast([PART, KT, 4, 16]),
                            op=ALU.mult)
                        Z = wp2.tile([PART, KT, 4], F32, tag="Z")
                        nc.vector.tensor_reduce(
                            out=Z[:].unsqueeze(-1), in_=adp[:],
                            axis=mybir.AxisListType.X, op=ALU.add)
                        nc.vector.tensor_tensor(
                            out=Z[:, 0:KLO, :], in0=Z[:, 0:KLO, :],
                            in1=glo[:, :, HC:HC + 8].bitcast(F32), op=ALU.add)
                        nc.vector.tensor_tensor(
                            out=Z[:, KLO:KT, :], in0=Z[:, KLO:KT, :],
                            in1=ghi[:, :, HC:HC + 8].bitcast(F32), op=ALU.add)
                        # leaky-relu (one fused op), clamp, exp -> fp16
                        nc.vector.scalar_tensor_tensor(
                            out=Z[:], in0=Z[:], scalar=NEG_SLOPE, in1=Z[:],
                            op0=ALU.mult, op1=ALU.max)
                        nc.vector.tensor_scalar_min(out=Z[:], in0=Z[:], scalar1=ZCLAMP)
                        EXh = wp2.tile([PART, KT, 4], F16, tag="EXh")
                        nc.scalar.activation(out=EXh[:], in_=Z[:], func=ACTF.Exp)

                        # Hp = [ex-weighted h | ex] (fp16, (c,h)-interleaved)
                        Hp = wp2.tile([PART, KT, 260], F16, tag="Hp")
                        nc.vector.tensor_tensor(
                            out=Hp[:, 0:KLO, 0:HC].rearrange("p k (c h) -> p k c h", h=H),
                            in0=glo[:, :, 0:HC].rearrange("p k (c h) -> p k c h", h=H),
                            in1=EXh[:, 0:KLO, :].unsqueeze(2).to_broadcast([PART, KLO, C0, H]),
                            op=ALU.mult)
                        nc.vector.tensor_tensor(
                            out=Hp[:, KLO:KT, 0:HC].rearrange("p k (c h) -> p k c h", h=H),
                            in0=ghi[:, :, 0:HC].rearrange("p k (c h) -> p k c h", h=H),
                            in1=EXh[:, KLO:KT, :].unsqueeze(2).to_broadcast([PART, KHI, C0, H]),
                            op=ALU.mult)
                        nc.vector.tensor_copy(out=Hp[:, :, HC:HC + 4], in_=EXh[:])

                        acc = ep.tile([PART, 260], F32, tag="acc")
                        for e in range(KT):
                            nc.tensor.matmul(acc[:], lhsT=S01[:, e, :], rhs=Hp[:, e, :],
                                             start=(e == 0), stop=(e == KT - 1))

                        dn = wp2.tile([PART, 4], F32, tag="dn")
                        nc.vector.tensor_scalar_add(out=dn[:], in0=acc[:, HC:HC + 4],
                                                    scalar1=1e-16)
                        rec = wp2.tile([PART, 4], F32, tag="rec")
                        nc.vector.reciprocal(out=rec[:], in_=dn[:])
                        ob = wp2.tile([PART, HC], F32, tag="ob")
                        nc.vector.tensor_tensor(
                            out=ob[:].rearrange("p (c h) -> p c h", h=H),
                            in0=acc[:, 0:HC].rearrange("p (c h) -> p c h", h=H),
                            in1=rec[:].unsqueeze(1).to_broadcast([PART, C0, H]),
                            op=ALU.mult)
                        for ct in range(2):
                            tp = tps.tile([PART, PART], F32, tag="ttp")
                            nc.tensor.transpose(out=tp[:], in_=ob[:, ct * PART:(ct + 1) * PART],
                                                identity=ident[:])
                            tsh = wp2.tile([PART, PART], F16, tag="tsh")
                            nc.scalar.activation(out=tsh[:], in_=tp[:],
                                                 func=ACTF.Relu if relu else ACTF.Identity,
                                                 bias=bias_cv[:, ct:ct + 1])
                            if outT is None:
                                ci = min(b // bpc, NAGC - 1)
                                cc0 = chunk_cols[ci][0]
                                nc.sync.dma_start(
                                    out=g1h_c[ci][ct * PART:(ct + 1) * PART,
                                                  b * PART - cc0:b * PART - cc0 + mb],
                                    in_=tsh[:, 0:mb])
                            else:
                                nc.sync.dma_start(
                                    out=outT[ct * PART:(ct + 1) * PART,
                                             b * PART:b * PART + mb],
                                    in_=tsh[:, 0:mb])
                        if ag:
                            ci = min(b // bpc, NAGC - 1)
                            if b == blk_of_chunk[ci][-1]:
                                if sim_local:
                                    for r_ in range(NCORES):
                                        nc.sync.dma_start(
                                            out=Tag_c[ci][r_ * HC:(r_ + 1) * HC, :],
                                            in_=g1h_c[ci][:, :])
                                else:
                                    nc.gpsimd.collective_compute(
                                        "AllGather", ALU.bypass,
                                        replica_groups=rgroups,
                                        ins=[g1h_c[ci][:, :]],
                                        outs=[Tag_c[ci][:, :]])
                nc.leave_named_scope(scope, _es, False)

            if PHASES >= 2:
                edge_phase(T1b, al1pk, b1cv, None, relu=False, scope="edge1",
                           ag=True)

            # ---------- BN stats + AllReduce ----------
            if PHASES < 3:
                raise _PhaseStopE
            _sc = nc.enter_named_scope("bnstat", False)[0]
            with tc.tile_pool(name="st", bufs=1) as sp, \
                 tc.tile_pool(name="stw", bufs=1) as sw:
                stats = sw.tile([PART, 4], F32)
                for ct in range(2):
                    gt = sp.tile([PART, NPC], F16, tag="gt")
                    for ci in range(NAGC):
                        cc0, cc1 = chunk_cols[ci]
                        nc.sync.dma_start(
                            out=gt[:, cc0:cc1],
                            in_=g1h_c[ci][ct * PART:(ct + 1) * PART, :])
                    nc.vector.tensor_reduce(out=stats[:, ct:ct + 1], in_=gt[:],
                                            axis=mybir.AxisListType.X, op=ALU.add)
                    sq = sp.tile([PART, NPC], F32, tag="sq")
                    nc.scalar.activation(out=sq[:], in_=gt[:], func=ACTF.Square)
                    nc.vector.tensor_reduce(out=stats[:, 2 + ct:3 + ct], in_=sq[:],
                                            axis=mybir.AxisListType.X, op=ALU.add)
                nc.sync.dma_start(out=ar_in[:, :], in_=stats[:])
            nc.leave_named_scope("bnstat", _sc, False)

            _sc = nc.enter_named_scope("ar", False)[0]
            if sim_local:
                nc.sync.dma_start(out=ar_out[:, :], in_=ar_in[:, :])
            else:
                nc.gpsimd.collective_compute(
                    "AllReduce", ALU.add, replica_groups=rgroups,
                    ins=[ar_in[:, :]], outs=[ar_out[:, :]])
            nc.leave_named_scope("ar", _sc, False)

            with tc.tile_pool(name="bnw", bufs=1) as bw:
                ar_sb = bw.tile([PART, 4], F32)
                nc.sync.dma_start(out=ar_sb[:], in_=ar_out[:, :])
                mean = bw.tile([PART, 2], F32)
                nc.vector.tensor_scalar_mul(out=mean[:], in0=ar_sb[:, 0:2], scalar1=1.0 / N)
                msq = bw.tile([PART, 2], F32)
                nc.vector.tensor_scalar_mul(out=msq[:], in0=ar_sb[:, 2:4], scalar1=1.0 / N)
                var = bw.tile([PART, 2], F32)
                nc.vector.tensor_tensor(out=var[:], in0=mean[:], in1=mean[:], op=ALU.mult)
                nc.vector.tensor_tensor(out=var[:], in0=msq[:], in1=var[:], op=ALU.subtract)
                nc.vector.tensor_scalar_add(out=var[:], in0=var[:], scalar1=BN_EPS)
                sd = bw.tile([PART, 2], F32)
                nc.scalar.activation(out=sd[:], in_=var[:], func=ACTF.Sqrt)
                rinv = bw.tile([PART, 2], F32)
                nc.vector.reciprocal(out=rinv[:], in_=sd[:])
                gc = bw.tile([PART, 2], F32)
                nc.sync.dma_start(out=gc[:], in_=gcol_d[:, :])
                bc = bw.tile([PART, 2], F32)
                nc.sync.dma_start(out=bc[:], in_=bcol_d[:, :])
                scale_c = bw.tile([PART, 2], F32)
                nc.vector.tensor_tensor(out=scale_c[:], in0=gc[:], in1=rinv[:], op=ALU.mult)
                shift_c = bw.tile([PART, 2], F32)
                nc.vector.tensor_tensor(out=shift_c[:], in0=mean[:], in1=scale_c[:], op=ALU.mult)
                nc.vector.tensor_tensor(out=shift_c[:], in0=bc[:], in1=shift_c[:], op=ALU.subtract)

                # ---------- dense L2 (replicated, from AllGathered x1) -----
                if PHASES < 4:
                    raise _PhaseStopE
                _sc = nc.enter_named_scope("dense2", False)[0]
                with tc.tile_pool(name="d2", bufs=2) as dp2, \
                     tc.tile_pool(name="d2w", bufs=1) as wp3, \
                     tc.tile_pool(name="d2x", bufs=2) as xp2, \
                     tc.tile_pool(name="d2ps", bufs=2, space="PSUM") as pp2:
                    W2_sb = [wp3.tile([PART, HC], F16, tag=f"w2_{kt}", name=f"w2_{kt}")
                             for kt in range(2)]
                    B2_sb = [wp3.tile([PART, 8], F16, tag=f"b2_{kt}", name=f"b2_{kt}")
                             for kt in range(2)]
                    for kt in range(2):
                        nc.sync.dma_start(out=W2_sb[kt][:],
                                          in_=W2_d[kt * PART:(kt + 1) * PART, :])
                        nc.sync.dma_start(out=B2_sb[kt][:],
                                          in_=B2_d[kt * PART:(kt + 1) * PART, :])
                    for r_ in range(NCORES):
                        for ci in range(NAGC):
                            cc0, cc1 = chunk_cols[ci]
                            w = cc1 - cc0
                            xs = []
                            for kt in range(2):
                                gl = xp2.tile([PART, bpc * PART], F16, tag=f"gl{kt}",
                                              name=f"gl{kt}")
                                nc.sync.dma_start(
                                    out=gl[:, 0:w],
                                    in_=Tag_c[ci][r_ * HC + kt * PART:r_ * HC + (kt + 1) * PART, :])
                                x1s = xp2.tile([PART, bpc * PART], F16, tag=f"x1s{kt}",
                                               name=f"x1s{kt}")
                                nc.scalar.activation(out=x1s[:, 0:w], in_=gl[:, 0:w],
                                                     func=ACTF.Relu,
                                                     bias=shift_c[:, kt:kt + 1],
                                                     scale=scale_c[:, kt:kt + 1])
                                xs.append(x1s)
                            nblk2 = (w + PART - 1) // PART
                            row8 = al8 = None
                            for bl in range(nblk2):
                                lb = bl * PART
                                mb2 = min(PART, w - lb)
                                k = bl % GRP
                                if k == 0:
                                    row8 = dp2.tile([PART, GRP, RWH], F16, tag="d2row8")
                                    al8 = dp2.tile([PART, GRP, 4], F32, tag="d2al8")
                                ps = pp2.tile([PART, 264], F32, tag="d2ps")
                                for kt in range(2):
                                    nc.tensor.matmul(ps[0:mb2, 0:HC],
                                                     lhsT=xs[kt][:, lb:lb + mb2],
                                                     rhs=W2_sb[kt][:],
                                                     start=(kt == 0), stop=(kt == 1))
                                for kt in range(2):
                                    nc.tensor.matmul(ps[0:mb2, HC:HC + 8],
                                                     lhsT=xs[kt][:, lb:lb + mb2],
                                                     rhs=B2_sb[kt][:],
                                                     start=(kt == 0), stop=(kt == 1))
                                nc.scalar.activation(out=row8[:, k, 0:HC], in_=ps[:, 0:HC],
                                                     func=ACTF.Copy)
                                nc.vector.tensor_copy(out=row8[:, k, HC:HC + 8].bitcast(F32),
                                                      in_=ps[:, HC:HC + 4])
                                nc.vector.tensor_copy(out=al8[:, k, :],
                                                      in_=ps[:, HC + 4:HC + 8])
                                if k == GRP - 1 or bl == nblk2 - 1:
                                    # batch-flush the full blocks; a ragged
                                    # tail block (mb2 < PART) is written solo
                                    ng = k + (1 if mb2 == PART else 0)
                                    n0 = r_ * NPC + cc0 + (bl - k) * PART
                                    if ng > 0:
                                        nc.sync.dma_start(
                                            out=T2b[n0:n0 + ng * PART, :].rearrange(
                                                "(k p) w -> p k w", p=PART),
                                            in_=row8[:, 0:ng, :])
                                        nc.sync.dma_start(
                                            out=al2pk[n0:n0 + ng * PART, :].rearrange(
                                                "(k p) c -> p k c", p=PART),
                                            in_=al8[:, 0:ng, :])
                                    if mb2 < PART:
                                        nr = n0 + k * PART
                                        nc.sync.dma_start(
                                            out=T2b[nr:nr + mb2, :],
                                            in_=row8[0:mb2, k, :])
                                        nc.sync.dma_start(
                                            out=al2pk[nr:nr + mb2, :],
                                            in_=al8[0:mb2, k, :])
                nc.leave_named_scope("dense2", _sc, False)

                if PHASES < 5:
                    raise _PhaseStopE
                edge_phase(T2b, al2pk, b2cv, x2T, relu=True, scope="edge2")

                # ---------- pooling ----------
                if PHASES < 6:
                    raise _PhaseStopE
                _sc = nc.enter_named_scope("pool", False)[0]
                with tc.tile_pool(name="pl", bufs=1) as pl:
                    mk = pl.tile([PART, NPC], F32, tag="mk")
                    nc.sync.dma_start(out=mk[:], in_=mask_d[0:1, :].to_broadcast([PART, NPC]))
                    cv = pl.tile([PART, NPC], F32, tag="cv")
                    nc.sync.dma_start(out=cv[:], in_=cinv_d[0:1, :].to_broadcast([PART, NPC]))
                    for ct in range(4):
                        xt = pl.tile([PART, NPC], F32, tag="xt")
                        if ct < 2:
                            gld = pl.tile([PART, NPC], F16, tag="gld")
                            for ci in range(NAGC):
                                cc0, cc1 = chunk_cols[ci]
                                nc.sync.dma_start(
                                    out=gld[:, cc0:cc1],
                                    in_=g1h_c[ci][ct * PART:(ct + 1) * PART, :])
                            nc.scalar.activation(out=xt[:], in_=gld[:], func=ACTF.Relu,
                                                 bias=shift_c[:, ct:ct + 1],
                                                 scale=scale_c[:, ct:ct + 1])
                        else:
                            x2l = pl.tile([PART, NPC], F16, tag="x2l")
                            nc.sync.dma_start(out=x2l[:],
                                              in_=x2T[(ct - 2) * PART:(ct - 1) * PART, :])
                            nc.scalar.activation(out=xt[:], in_=x2l[:], func=ACTF.Copy)
                        sm = pl.tile([PART, NPC], F32, tag="sm")
                        nc.vector.tensor_tensor_scan(out=sm[:], data0=mk[:], data1=xt[:],
                                                     initial=0.0, op0=ALU.mult, op1=ALU.max)
                        nc.sync.dma_start(out=omax_d[ct * PART:(ct + 1) * PART, :], in_=sm[:])
                        ss = pl.tile([PART, NPC], F32, tag="ss")
                        nc.vector.tensor_tensor_scan(out=ss[:], data0=mk[:], data1=xt[:],
                                                     initial=0.0, op0=ALU.mult, op1=ALU.add)
                        nc.vector.tensor_tensor(out=ss[:], in0=ss[:], in1=cv[:], op=ALU.mult)
                        nc.sync.dma_start(out=omean_d[ct * PART:(ct + 1) * PART, :], in_=ss[:])
                nc.leave_named_scope("pool", _sc, False)

      except _PhaseStopE:
        pass

    nc.compile()
    return nc


# --------------------------------------------------------------------------
# host-side combine
# --------------------------------------------------------------------------

def postprocess(results, meta):
    lastcol = meta["lastcol"]
    mean = np.zeros((G, 2 * HC), np.float32)
    mx = np.zeros((G, 2 * HC), np.float32)
    for r in range(NCORES):
        om = results[r]["out_mean"]   # [512, NPC], dev channel order
        ox = results[r]["out_max"]
        for g_, col in lastcol[r].items():
            mean[g_] += om[:, col]
            mx[g_] = np.maximum(mx[g_], ox[:, col])
    # un-permute dev channel order back to torch order
    dev2orig = np.concatenate([PERM, HC + PERM])
    mean_o = np.empty_like(mean); mx_o = np.empty_like(mx)
    mean_o[:, dev2orig] = mean
    mx_o[:, dev2orig] = mx
    return np.concatenate([mean_o, mx_o], axis=1).astype(np.float32)


_CACHE = {}


def kernel(**inputs):
    in_maps, meta = preprocess(**inputs)
    key = (meta["NB"], meta["KLO"], meta["KHI"])
    if key not in _CACHE:
        _CACHE[key] = build_program(meta)
    nc = _CACHE[key]
    res = bass_utils.run_bass_kernel_spmd(nc, in_maps, core_ids=list(range(NCORES)))
    return postprocess(res.results, meta)


# revision 33
# speedup vs baseline: 7.2562x; 7.2562x over previous
"""Self-contained Trainium2 Bass kernel for a 2-layer GAT + BatchNorm + graph pooling.

Contract: kernel(**inputs) takes the FULL (unsharded) inputs and returns the
FULL [G, 1024] float32 output.

v3 design: replicated dense layers, fp16 gather tables, tiny overlapped
collectives, channel-interleaved layout for fast DVE broadcasts.

  - Channels are stored (c, h)-interleaved (dev channel c*4+h = torch channel
    h*64+c, permuted host-side in the weights and un-permuted in postprocess)
    so every per-head broadcast multiply has a packed innermost dim of 4 —
    DVE runs these at 16-bit double rate instead of broadcast-stride-0 rate.
  - dense L1 is REPLICATED: every core computes the full table
    T1b[n] = [h(256 fp16) | al_src(4 f32 riding as 8 fp16 slots) | pad]
    (768 B rows) from x; writes are batched 8 blocks per DMA (the HWDGE
    ~600 ns fixed cost per dma_start dominated v2's dense phases).
  - "al_dst" logits live in a plain [N, 4] f32 table whose gather view is
    [N/16, 64] (256 B rows = the dma_gather minimum); the per-edge value is
    extracted with a one-hot-over-16 dot on DVE.  This keeps gather indices
    (node//16) inside int16 and lets the replicated dense write it cheaply.
  - edge phase (per 128-dst-node block): one combined index/metadata DMA,
    dma_gather rows by src (lo/hi split for int16; <=1024 idxs per
    instruction — 2048 hangs the HW, verified), -1-padded index streams skip
    pad transfers (per-core valid counts are reg_load-ed from SBUF),
    softmax-weighted segment sum via 0/1 fp16 selector-matrix matmuls
    accumulating [out | denom] in PSUM.  Logits stay f32 (exp via ACT,
    clamped at +8 so pad garbage cannot overflow fp16).  The per-node
    epilogue folds bias+relu into the post-transpose ACT copy.
  - between layers only x1's pre-BN value g1 (256ch fp16, transposed) is
    AllGathered — 3.2 MB per rank in 4 column-chunks issued as edge L1
    drains, overlapping wire time with edge compute.  BN stats go through a
    [128,4] AllReduce; the BN affine + relu is fused into dense L2's
    activation load.  dense L2 is replicated from the gathered chunks.
  - pooling: per-channel-tile segmented running sum & max along the node
    axis (tensor_tensor_scan); host reads each graph's last column and
    combines the <=2 per-graph partials from adjacent cores.
"""

import numpy as np

import concourse.bass as bass
import concourse.bacc as bacc
import concourse.tile as tile
from concourse import mybir
from concourse import bass_utils
from concourse.masks import make_identity

F32 = mybir.dt.float32
F16 = mybir.dt.float16
I16 = mybir.dt.int16
I32 = mybir.dt.int32
ALU = mybir.AluOpType
ACTF = mybir.ActivationFunctionType

# problem constants (hardcoded per the harness contract)
N, F_IN, C0, C1, H, E, G = 50000, 128, 64, 64, 4, 800000, 256
HC = H * C0            # 256
NEG_SLOPE = 0.2
BN_EPS = 1e-5
NCORES = 8
NPC = N // NCORES      # nodes per core (6250)
SPLIT = 32768          # dma_gather int16 index limit -> split gather table
RWH = 384              # fp16 table row width (768 B): h(256) + al_src(8) + pad
PART = 128
NPAD = 50048           # N rounded to 128 blocks (391 blocks)
NBLK = NPAD // PART    # 391 dense blocks
CHUNK = 8              # 128-idx groups per dma_gather (1024 idx HW limit)
ZCLAMP = 8.0           # logit clamp (real logits ~ +-6); keeps exp fp16-finite
NAGC = 4               # AllGather column-chunks for the inter-layer feature
GRP = 8                # dense blocks batched per table-row DMA

# dev channel k = c*4+h  <->  torch channel h*64+c
PERM = np.array([(k % H) * C0 + k // H for k in range(HC)], np.int64)

PHASES = 6             # build phases 1..6 (bisection aid)


# --------------------------------------------------------------------------
# host-side preprocessing
# --------------------------------------------------------------------------

def _pack16(stream_i16, ncols):
    """dma_gather index layout: position i -> [i%16, i//16], replicated to
    partition groups 16k+p for the 8 Q7 cores."""
    base = stream_i16.reshape(ncols, 16).T          # [16, ncols]
    return np.tile(base, (8, 1)).astype(np.int16)   # [128, ncols]


def _pad_stream(vals, nslots):
    """Pad an index stream to nslots with -1 (skipped by dma_gather) and
    return (idx_i16, per-1024-chunk valid counts).  A chunk with zero valid
    indices gets one dummy index 0 (count 1): the HW needs at least one
    non-negative index per instruction."""
    n = len(vals)
    out = np.full(nslots, -1, np.int16)
    out[:n] = vals
    counts = []
    for c0 in range(0, nslots, CHUNK * PART):
        span = min(CHUNK * PART, nslots - c0)
        cnt = min(max(n - c0, 0), span)
        if cnt == 0:
            out[c0] = 0
            cnt = 1
        counts.append(cnt)
    return out, counts


def preprocess(x, edge_index, batch,
               W1, att_src1, att_dst1, b1, gamma, beta,
               W2, att_src2, att_dst2, b2):
    x = np.asarray(x, np.float32)
    edge_index = np.asarray(edge_index)
    batch = np.asarray(batch).astype(np.int64)
    W1 = np.asarray(W1, np.float32); W2 = np.asarray(W2, np.float32)

    src = np.concatenate([edge_index[0], np.arange(N, dtype=np.int64)])
    dst = np.concatenate([edge_index[1], np.arange(N, dtype=np.int64)])

    NB = (NPC + PART - 1) // PART                      # dst blocks per core

    # ---- per-core edge streams ----
    blocks = []
    nlo_max = nhi_max = 0
    for r in range(NCORES):
        m = (dst >= r * NPC) & (dst < (r + 1) * NPC)
        s_r = src[m]; d_r = dst[m]
        dloc = d_r - r * NPC
        order = np.argsort(dloc, kind="stable")
        s_r = s_r[order]; d_r = d_r[order]; dloc = dloc[order]
        blk = dloc // PART
        core_blocks = []
        for b in range(NB):
            bm = blk == b
            sb_ = s_r[bm]; db_ = d_r[bm]
            lo_m = sb_ < SPLIT
            core_blocks.append((sb_[lo_m], sb_[~lo_m] - SPLIT,
                                db_[lo_m], db_[~lo_m]))
            nlo_max = max(nlo_max, int(lo_m.sum()))
            nhi_max = max(nhi_max, int((~lo_m).sum()))
        blocks.append(core_blocks)

    KLO = max(1, (nlo_max + PART - 1) // PART)
    KHI = max(1, (nhi_max + PART - 1) // PART)
    KT = KLO + KHI
    CL = (KLO + CHUNK - 1) // CHUNK
    CH = (KHI + CHUNK - 1) // CHUNK
    IDXW = KT * 18       # [il|ih|ial|iah] (KT*16) + dl (KT) + dm (KT), i16

    ib_t = np.zeros((NCORES, NB, PART, IDXW), np.int16)
    cnt_t = np.zeros((NCORES, NB, 8), np.int32)
    for r in range(NCORES):
        for b in range(NB):
            lo_src, hi_src, abs_lo, abs_hi = blocks[r][b]
            ls, c_lo = _pad_stream(lo_src, KLO * PART)
            hs, c_hi = _pad_stream(hi_src, KHI * PART)
            al_lo, _ = _pad_stream(abs_lo // 16, KLO * PART)
            al_hi, _ = _pad_stream(abs_hi // 16, KHI * PART)
            ib_t[r, b, :, 0:KLO * 8] = _pack16(ls, KLO * 8)
            ib_t[r, b, :, KLO * 8:KT * 8] = _pack16(hs, KHI * 8)
            ib_t[r, b, :, KT * 8:KT * 8 + KLO * 8] = _pack16(al_lo, KLO * 8)
            ib_t[r, b, :, KT * 8 + KLO * 8:KT * 16] = _pack16(al_hi, KHI * 8)
            cnt_t[r, b, :CL] = c_lo
            cnt_t[r, b, CL:CL + CH] = c_hi
            dl = np.full(KT * PART, 999.0, np.float32)
            dm = np.zeros(KT * PART, np.float32)
            dl[:len(abs_lo)] = (abs_lo - r * NPC) % PART
            dm[:len(abs_lo)] = abs_lo % 16
            dl[KLO * PART:KLO * PART + len(abs_hi)] = (abs_hi - r * NPC) % PART
            dm[KLO * PART:KLO * PART + len(abs_hi)] = abs_hi % 16
            ib_t[r, b, :, KT * 16:KT * 17] = \
                dl.reshape(KT, PART).T.astype(np.float16).view(np.int16)
            ib_t[r, b, :, KT * 17:KT * 18] = \
                dm.reshape(KT, PART).T.astype(np.float16).view(np.int16)

    # ---- batch-derived pooling metadata ----
    counts = np.bincount(batch, minlength=G).astype(np.float64)
    maskrow = np.zeros((NCORES, 1, NPC), np.float32)
    cinvrow = np.zeros((NCORES, 1, NPC), np.float32)
    lastcol = [dict() for _ in range(NCORES)]
    for r in range(NCORES):
        bseg = batch[r * NPC:(r + 1) * NPC]
        same = np.ones(NPC, np.float32)
        same[0] = 0.0
        same[1:] = (bseg[1:] == bseg[:-1]).astype(np.float32)
        maskrow[r, 0] = same
        cinvrow[r, 0] = (1.0 / np.maximum(counts[bseg], 1.0)).astype(np.float32)
        gids, last_idx = np.unique(bseg[::-1], return_index=True)
        for g_, li in zip(gids, last_idx):
            lastcol[r][int(g_)] = NPC - 1 - int(li)

    # ---- weights (replicated; channel-permuted to dev order) ----
    def bmat(W, a_s, a_d, fin):
        Wr = W.reshape(fin, H, C0)
        bs = np.einsum("khc,hc->kh", Wr, np.asarray(a_s, np.float32))
        bd = np.einsum("khc,hc->kh", Wr, np.asarray(a_d, np.float32))
        return np.concatenate([bs, bd], axis=1).astype(np.float16)  # [fin, 8]

    xh = np.zeros((F_IN, NPAD), np.float16)
    xh[:, :N] = x.T.astype(np.float16)

    W1p = W1[:, PERM]
    W2p = W2[PERM][:, PERM]
    b1p = np.asarray(b1, np.float32)[PERM]
    b2p = np.asarray(b2, np.float32)[PERM]

    shared = dict(
        xh16T=xh,
        W1h=W1p.astype(np.float16), B1h=bmat(W1, att_src1, att_dst1, F_IN),
        W2h=W2p.astype(np.float16),
        B2h=bmat(W2, att_src2, att_dst2, HC)[PERM, :],
        b1colT=b1p.reshape(2, PART).T.copy(),
        b2colT=b2p.reshape(2, PART).T.copy(),
        gcol=np.asarray(gamma, np.float32)[PERM].reshape(2, PART).T.copy(),
        bcol=np.asarray(beta, np.float32)[PERM].reshape(2, PART).T.copy(),
    )
    in_maps = []
    for r in range(NCORES):
        in_maps.append(dict(
            shared,
            ib=ib_t[r],
            cnts=cnt_t[r].reshape(1, NB * 8),
            maskrow=maskrow[r],
            cinvrow=cinvrow[r],
        ))
    meta = dict(NB=NB, KLO=KLO, KHI=KHI, KT=KT, CL=CL, CH=CH,
                lastcol=lastcol, counts=counts)
    return in_maps, meta


# --------------------------------------------------------------------------
# device program
# --------------------------------------------------------------------------

def build_program(meta, sim_local=False):
    NB, KLO, KHI, KT = meta["NB"], meta["KLO"], meta["KHI"], meta["KT"]
    CL, CH = meta["CL"], meta["CH"]
    IDXW = KT * 18
    nc = bacc.Bacc("TRN2", target_bir_lowering=False, debug=False,
                   num_devices=1 if sim_local else NCORES)

    def ein(name, shape, dt=F32):
        return nc.dram_tensor(name, list(shape), dt, kind="ExternalInput").ap()

    xh_d = ein("xh16T", [F_IN, NPAD], F16)
    W1_d = ein("W1h", [F_IN, HC], F16); B1_d = ein("B1h", [F_IN, 8], F16)
    W2_d = ein("W2h", [HC, HC], F16);   B2_d = ein("B2h", [HC, 8], F16)
    b1c_d = ein("b1colT", [PART, 2]); b2c_d = ein("b2colT", [PART, 2])
    gcol_d = ein("gcol", [PART, 2]); bcol_d = ein("bcol", [PART, 2])
    ib_d = ein("ib", [NB, PART, IDXW], I16)
    cnt_d = ein("cnts", [1, NB * 8], I32)
    mask_d = ein("maskrow", [1, NPC])
    cinv_d = ein("cinvrow", [1, NPC])

    omax_d = nc.dram_tensor("out_max", [4 * PART, NPC], F32, kind="ExternalOutput").ap()
    omean_d = nc.dram_tensor("out_mean", [4 * PART, NPC], F32, kind="ExternalOutput").ap()

    # internal DRAM
    T1b = nc.dram_tensor("T1b", [NPAD, RWH], F16).ap()
    T2b = nc.dram_tensor("T2b", [NPAD, RWH], F16).ap()
    al1pk = nc.dram_tensor("al1pk", [NPAD, 4], F32).ap()
    al2pk = nc.dram_tensor("al2pk", [NPAD, 4], F32).ap()
    x2T = nc.dram_tensor("x2T", [HC, NPC], F16).ap()
    ar_in = nc.dram_tensor("ar_in", [PART, 4], F32).ap()
    ar_out = nc.dram_tensor("ar_out", [PART, 4], F32, addr_space="Shared").ap()

    # AllGather chunks of the inter-layer feature (transposed, fp16)
    bpc = (NB + NAGC - 1) // NAGC
    blk_of_chunk = [list(range(c * bpc, min(NB, (c + 1) * bpc)))
                    for c in range(NAGC)]
    chunk_cols = []
    g1h_c, Tag_c = [], []
    for c in range(NAGC):
        c0 = blk_of_chunk[c][0] * PART
        c1 = min(NPC, (blk_of_chunk[c][-1] + 1) * PART)
        chunk_cols.append((c0, c1))
        g1h_c.append(nc.dram_tensor(f"g1h_{c}", [HC, c1 - c0], F16).ap())
        Tag_c.append(nc.dram_tensor(f"Tag_{c}", [NCORES * HC, c1 - c0], F16,
                                    addr_space="Shared").ap())

    rgroups = [list(range(NCORES))]

    class _PhaseStopE(Exception):
        pass

    with tile.TileContext(nc) as tc:
      try:
        # ---------- shared constant tiles ----------
        with tc.tile_pool(name="const", bufs=1) as cpool:
            ident = cpool.tile([PART, PART], F32)
            make_identity(nc, ident[:])
            iota_i = cpool.tile([PART, PART], mybir.dt.int32)
            nc.gpsimd.iota(iota_i[:], pattern=[[1, PART]], base=0,
                           channel_multiplier=0)
            iota_h = cpool.tile([PART, PART], F16)
            nc.vector.tensor_copy(out=iota_h[:], in_=iota_i[:])
            iota16 = cpool.tile([PART, 16], F16)
            nc.vector.tensor_copy(out=iota16[:], in_=iota_i[:, 0:16])

            cnt_sb = cpool.tile([1, NB * 8], I32)
            nc.sync.dma_start(out=cnt_sb[:], in_=cnt_d[:, :])
            b1cv = cpool.tile([PART, 2], F32)
            nc.sync.dma_start(out=b1cv[:], in_=b1c_d[:, :])
            b2cv = cpool.tile([PART, 2], F32)
            nc.sync.dma_start(out=b2cv[:], in_=b2c_d[:, :])

            # ---------- dense L1 (replicated: full table on every core) ----
            _sc = nc.enter_named_scope("dense1", False)[0]
            with tc.tile_pool(name="d1", bufs=3) as dp, \
                 tc.tile_pool(name="d1w", bufs=1) as wp, \
                 tc.tile_pool(name="d1x", bufs=2) as xp, \
                 tc.tile_pool(name="d1ps", bufs=4, space="PSUM") as pp:
                W1_sb = wp.tile([F_IN, HC], F16)
                nc.sync.dma_start(out=W1_sb[:], in_=W1_d[:, :])
                B1_sb = wp.tile([F_IN, 8], F16)
                nc.sync.dma_start(out=B1_sb[:], in_=B1_d[:, :])
                XCH = 6272                      # x column chunk (49 blocks)
                x_sb = None
                row8 = al8 = None
                for b in range(NBLK):
                    if b % 49 == 0:
                        x_sb = xp.tile([F_IN, XCH], F16, tag="xsb")
                        x0 = b * PART
                        nc.sync.dma_start(out=x_sb[:, 0:min(XCH, NPAD - x0)],
                                          in_=xh_d[:, x0:min(x0 + XCH, NPAD)])
                    k = b % GRP
                    if k == 0:
                        row8 = dp.tile([PART, GRP, HC + 8], F16, tag="row8")
                        al8 = dp.tile([PART, GRP, 4], F32, tag="al8")
                    col = (b % 49) * PART
                    ps = pp.tile([PART, 264], F32, tag="dps")
                    nc.tensor.matmul(ps[:, 0:HC], lhsT=x_sb[:, col:col + PART],
                                     rhs=W1_sb[:], start=True, stop=True)
                    nc.tensor.matmul(ps[:, HC:HC + 8], lhsT=x_sb[:, col:col + PART],
                                     rhs=B1_sb[:], start=True, stop=True)
                    nc.scalar.activation(out=row8[:, k, 0:HC], in_=ps[:, 0:HC],
                                         func=ACTF.Copy)
                    nc.vector.tensor_copy(out=row8[:, k, HC:HC + 8].bitcast(F32),
                                          in_=ps[:, HC:HC + 4])
                    nc.vector.tensor_copy(out=al8[:, k, :], in_=ps[:, HC + 4:HC + 8])
                    if k == GRP - 1 or b == NBLK - 1:
                        ng = k + 1
                        n0 = (b - k) * PART
                        nc.sync.dma_start(
                            out=T1b[n0:n0 + ng * PART, 0:HC + 8].rearrange(
                                "(k p) w -> p k w", p=PART),
                            in_=row8[:, 0:ng, :])
                        nc.sync.dma_start(
                            out=al1pk[n0:n0 + ng * PART, :].rearrange(
                                "(k p) c -> p k c", p=PART),
                            in_=al8[:, 0:ng, :])
            nc.leave_named_scope("dense1", _sc, False)

            # ---------- edge phase (shared for both layers) ----------
            def edge_phase(Tbl, alpk, bias_cv, outT, relu, scope, ag=False):
                """outT: None for L1 (writes g1h chunks), else x2T."""
                _es = nc.enter_named_scope(scope, False)[0]
                alview = alpk[:, :].rearrange("(r j) c -> r (j c)", j=16)
                with tc.tile_pool(name="eidx", bufs=2) as ip, \
                     tc.tile_pool(name="eg", bufs=2) as gp, \
                     tc.tile_pool(name="ew", bufs=2) as wp2, \
                     tc.tile_pool(name="eps", bufs=2, space="PSUM") as ep, \
                     tc.tile_pool(name="etps", bufs=2, space="PSUM") as tps:
                    # pre-zero both gather buffers: -1-skipped slots must hold
                    # finite floats (uninitialized SBUF could be NaN -> NaN*0
                    # = NaN in PSUM)
                    for _z in range(2):
                        for tg, shp, dt_ in (("gall", [PART, KT, RWH], F16),
                                             ("ga", [PART, KT, 64], F32)):
                            zt = gp.tile(shp, dt_, tag=tg)
                            nc.vector.memset(zt[:], 0.0)

                    cnt_regs = [nc.gpsimd.alloc_register(f"cnt_{scope}_{i}")
                                for i in range(4)]
                    reg_rr = [0]

                    def gather(gtile, src_ap, ixtile, ktot, elem, cnt_base):
                        for ci, c0 in enumerate(range(0, ktot, CHUNK)):
                            cw = min(CHUNK, ktot - c0)
                            reg = cnt_regs[reg_rr[0] % 4]
                            reg_rr[0] += 1
                            nc.gpsimd.reg_load(
                                reg, cnt_sb[0:1, cnt_base + ci:cnt_base + ci + 1])
                            nc.gpsimd.dma_gather(
                                out_ap=gtile[:, c0:c0 + cw, :],
                                in_ap=src_ap, idxs_ap=ixtile[:, c0 * 8:(c0 + cw) * 8],
                                num_idxs=cw * PART, num_idxs_reg=reg,
                                elem_size=elem)

                    for b in range(NB):
                        mb = min(PART, NPC - b * PART)
                        ib = ip.tile([PART, IDXW], I16, tag="ib")
                        nc.sync.dma_start(out=ib[:], in_=ib_d[b, :, :])
                        il = ib[:, 0:KLO * 8]
                        ih = ib[:, KLO * 8:KT * 8]
                        ial = ib[:, KT * 8:KT * 8 + KLO * 8]
                        iah = ib[:, KT * 8 + KLO * 8:KT * 16]
                        dl = ib[:, KT * 16:KT * 17].bitcast(F16)
                        dm = ib[:, KT * 17:KT * 18].bitcast(F16)

                        gall = gp.tile([PART, KT, RWH], F16, tag="gall")
                        gather(gall[:, 0:KLO, :], Tbl[0:SPLIT, :], il, KLO, RWH, b * 8)
                        gather(gall[:, KLO:KT, :], Tbl[SPLIT:NPAD, :], ih, KHI, RWH,
                               b * 8 + CL)
                        ga = gp.tile([PART, KT, 64], F32, tag="ga")
                        gather(ga[:, 0:KLO, :], alview, ial, KLO, 64, b * 8)
                        gather(ga[:, KLO:KT, :], alview, iah, KHI, 64, b * 8 + CL)

                        # selector matrix S01[e, kt, d] = (dl == d), fp16
                        S01 = wp2.tile([PART, KT, PART], F16, tag="S01")
                        nc.vector.tensor_tensor(
                            out=S01[:],
                            in0=dl[:].unsqueeze(-1).to_broadcast([PART, KT, PART]),
                            in1=iota_h[:].unsqueeze(1).to_broadcast([PART, KT, PART]),
                            op=ALU.is_equal)

                        # al_dst extraction: one-hot over the 16-node pack
                        oh = wp2.tile([PART, KT, 16], F32, tag="oh")
                        nc.vector.tensor_tensor(
                            out=oh[:],
                            in0=dm[:].unsqueeze(-1).to_broadcast([PART, KT, 16]),
                            in1=iota16[:].unsqueeze(1).to_broadcast([PART, KT, 16]),
                            op=ALU.is_equal)
                        adp = wp2.tile([PART, KT, 4, 16], F32, tag="adp")
                        nc.vector.tensor_tensor(
                            out=adp[:],
                            in0=ga[:].rearrange("p k (j h) -> p k h j", j=16),
                            in1=oh[:].unsqueeze(2).to_broadcast([PART, KT, 4, 16]),
                            op=ALU.mult)
                        Z = wp2.tile([PART, KT, 4], F32, tag="Z")
                        nc.vector.tensor_reduce(
                            out=Z[:].unsqueeze(-1), in_=adp[:],
                            axis=mybir.AxisListType.X, op=ALU.add)
                        nc.vector.tensor_tensor(
                            out=Z[:], in0=Z[:],
                            in1=gall[:, :, HC:HC + 8].bitcast(F32), op=ALU.add)
                        # leaky-relu (one fused op), clamp, exp -> fp16
                        nc.vector.scalar_tensor_tensor(
                            out=Z[:], in0=Z[:], scalar=NEG_SLOPE, in1=Z[:],
                            op0=ALU.mult, op1=ALU.max)
                        nc.vector.tensor_scalar_min(out=Z[:], in0=Z[:], scalar1=ZCLAMP)
                        EXh = wp2.tile([PART, KT, 4], F16, tag="EXh")
                        nc.scalar.activation(out=EXh[:], in_=Z[:], func=ACTF.Exp)

                        # Hp = [ex-weighted h | ex] (fp16, (c,h)-interleaved)
                        Hp = wp2.tile([PART, KT, 260], F16, tag="Hp")
                        nc.vector.tensor_tensor(
                            out=Hp[:, :, 0:HC].rearrange("p k (c h) -> p k c h", h=H),
                            in0=gall[:, :, 0:HC].rearrange("p k (c h) -> p k c h", h=H),
                            in1=EXh[:].unsqueeze(2).to_broadcast([PART, KT, C0, H]),
                            op=ALU.mult)
                        nc.vector.tensor_copy(out=Hp[:, :, HC:HC + 4], in_=EXh[:])

                        acc = ep.tile([PART, 260], F32, tag="acc")
                        for e in range(KT):
                            nc.tensor.matmul(acc[:], lhsT=S01[:, e, :], rhs=Hp[:, e, :],
                                             start=(e == 0), stop=(e == KT - 1))

                        dn = wp2.tile([PART, 4], F32, tag="dn")
                        nc.vector.tensor_scalar_add(out=dn[:], in0=acc[:, HC:HC + 4],
                                                    scalar1=1e-16)
                        rec = wp2.tile([PART, 4], F32, tag="rec")
                        nc.vector.reciprocal(out=rec[:], in_=dn[:])
                        ob = wp2.tile([PART, HC], F32, tag="ob")
                        nc.vector.tensor_tensor(
                            out=ob[:].rearrange("p (c h) -> p c h", h=H),
                            in0=acc[:, 0:HC].rearrange("p (c h) -> p c h", h=H),
                            in1=rec[:].unsqueeze(1).to_broadcast([PART, C0, H]),
                            op=ALU.mult)
                        for ct in range(2):
                            tp = tps.tile([PART, PART], F32, tag="ttp")
                            nc.tensor.transpose(out=tp[:], in_=ob[:, ct * PART:(ct + 1) * PART],
                                                identity=ident[:])
                            tsh = wp2.tile([PART, PART], F16, tag="tsh")
                            nc.scalar.activation(out=tsh[:], in_=tp[:],
                                                 func=ACTF.Relu if relu else ACTF.Identity,
                                                 bias=bias_cv[:, ct:ct + 1])
                            if outT is None:
                                ci = min(b // bpc, NAGC - 1)
                                cc0 = chunk_cols[ci][0]
                                nc.sync.dma_start(
                                    out=g1h_c[ci][ct * PART:(ct + 1) * PART,
                                                  b * PART - cc0:b * PART - cc0 + mb],
                                    in_=tsh[:, 0:mb])
                            else:
                                nc.sync.dma_start(
                                    out=outT[ct * PART:(ct + 1) * PART,
                                             b * PART:b * PART + mb],
                                    in_=tsh[:, 0:mb])
                        if ag:
                            ci = min(b // bpc, NAGC - 1)
                            if b == blk_of_chunk[ci][-1]:
                                if sim_local:
                                    for r_ in range(NCORES):
                                        nc.sync.dma_start(
                                            out=Tag_c[ci][r_ * HC:(r_ + 1) * HC, :],
                                            in_=g1h_c[ci][:, :])
                                else:
                                    nc.gpsimd.collective_compute(
                                        "AllGather", ALU.bypass,
                                        replica_groups=rgroups,
                                        ins=[g1h_c[ci][:, :]],
                                        outs=[Tag_c[ci][:, :]])
                nc.leave_named_scope(scope, _es, False)

            if PHASES >= 2:
                edge_phase(T1b, al1pk, b1cv, None, relu=False, scope="edge1",
                           ag=True)

            # ---------- BN stats + AllReduce ----------
            if PHASES < 3:
                raise _PhaseStopE
            _sc = nc.enter_named_scope("bnstat", False)[0]
            with tc.tile_pool(name="st", bufs=1) as sp, \
                 tc.tile_pool(name="stw", bufs=1) as sw:
                stats = sw.tile([PART, 4], F32)
                for ct in range(2):
                    gt = sp.tile([PART, NPC], F16, tag="gt")
                    for ci in range(NAGC):
                        cc0, cc1 = chunk_cols[ci]
                        nc.sync.dma_start(
                            out=gt[:, cc0:cc1],
                            in_=g1h_c[ci][ct * PART:(ct + 1) * PART, :])
                    nc.vector.tensor_reduce(out=stats[:, ct:ct + 1], in_=gt[:],
                                            axis=mybir.AxisListType.X, op=ALU.add)
                    sq = sp.tile([PART, NPC], F32, tag="sq")
                    nc.scalar.activation(out=sq[:], in_=gt[:], func=ACTF.Square)
                    nc.vector.tensor_reduce(out=stats[:, 2 + ct:3 + ct], in_=sq[:],
                                            axis=mybir.AxisListType.X, op=ALU.add)
                nc.sync.dma_start(out=ar_in[:, :], in_=stats[:])
            nc.leave_named_scope("bnstat", _sc, False)

            _sc = nc.enter_named_scope("ar", False)[0]
            if sim_local:
                nc.sync.dma_start(out=ar_out[:, :], in_=ar_in[:, :])
            else:
                nc.gpsimd.collective_compute(
                    "AllReduce", ALU.add, replica_groups=rgroups,
                    ins=[ar_in[:, :]], outs=[ar_out[:, :]])
            nc.leave_named_scope("ar", _sc, False)

            with tc.tile_pool(name="bnw", bufs=1) as bw:
                ar_sb = bw.tile([PART, 4], F32)
                nc.sync.dma_start(out=ar_sb[:], in_=ar_out[:, :])
                mean = bw.tile([PART, 2], F32)
                nc.vector.tensor_scalar_mul(out=mean[:], in0=ar_sb[:, 0:2], scalar1=1.0 / N)
                msq = bw.tile([PART, 2], F32)
                nc.vector.tensor_scalar_mul(out=msq[:], in0=ar_sb[:, 2:4], scalar1=1.0 / N)
                var = bw.tile([PART, 2], F32)
                nc.vector.tensor_tensor(out=var[:], in0=mean[:], in1=mean[:], op=ALU.mult)
                nc.vector.tensor_tensor(out=var[:], in0=msq[:], in1=var[:], op=ALU.subtract)
                nc.vector.tensor_scalar_add(out=var[:], in0=var[:], scalar1=BN_EPS)
                sd = bw.tile([PART, 2], F32)
                nc.scalar.activation(out=sd[:], in_=var[:], func=ACTF.Sqrt)
                rinv = bw.tile([PART, 2], F32)
                nc.vector.reciprocal(out=rinv[:], in_=sd[:])
                gc = bw.tile([PART, 2], F32)
                nc.sync.dma_start(out=gc[:], in_=gcol_d[:, :])
                bc = bw.tile([PART, 2], F32)
                nc.sync.dma_start(out=bc[:], in_=bcol_d[:, :])
                scale_c = bw.tile([PART, 2], F32)
                nc.vector.tensor_tensor(out=scale_c[:], in0=gc[:], in1=rinv[:], op=ALU.mult)
                shift_c = bw.tile([PART, 2], F32)
                nc.vector.tensor_tensor(out=shift_c[:], in0=mean[:], in1=scale_c[:], op=ALU.mult)
                nc.vector.tensor_tensor(out=shift_c[:], in0=bc[:], in1=shift_c[:], op=ALU.subtract)

                # ---------- dense L2 (replicated, from AllGathered x1) -----
                if PHASES < 4:
                    raise _PhaseStopE
                _sc = nc.enter_named_scope("dense2", False)[0]
                with tc.tile_pool(name="d2", bufs=3) as dp2, \
                     tc.tile_pool(name="d2w", bufs=1) as wp3, \
                     tc.tile_pool(name="d2x", bufs=2) as xp2, \
                     tc.tile_pool(name="d2ps", bufs=4, space="PSUM") as pp2:
                    W2_sb = [wp3.tile([PART, HC], F16, tag=f"w2_{kt}", name=f"w2_{kt}")
                             for kt in range(2)]
                    B2_sb = [wp3.tile([PART, 8], F16, tag=f"b2_{kt}", name=f"b2_{kt}")
                             for kt in range(2)]
                    for kt in range(2):
                        nc.sync.dma_start(out=W2_sb[kt][:],
                                          in_=W2_d[kt * PART:(kt + 1) * PART, :])
                        nc.sync.dma_start(out=B2_sb[kt][:],
                                          in_=B2_d[kt * PART:(kt + 1) * PART, :])
                    for r_ in range(NCORES):
                        for ci in range(NAGC):
                            cc0, cc1 = chunk_cols[ci]
                            w = cc1 - cc0
                            xs = []
                            for kt in range(2):
                                gl = xp2.tile([PART, bpc * PART], F16, tag=f"gl{kt}",
                                              name=f"gl{kt}")
                                nc.sync.dma_start(
                                    out=gl[:, 0:w],
                                    in_=Tag_c[ci][r_ * HC + kt * PART:r_ * HC + (kt + 1) * PART, :])
                                x1s = xp2.tile([PART, bpc * PART], F16, tag=f"x1s{kt}",
                                               name=f"x1s{kt}")
                                nc.scalar.activation(out=x1s[:, 0:w], in_=gl[:, 0:w],
                                                     func=ACTF.Relu,
                                                     bias=shift_c[:, kt:kt + 1],
                                                     scale=scale_c[:, kt:kt + 1])
                                xs.append(x1s)
                            nblk2 = (w + PART - 1) // PART
                            row8 = al8 = None
                            for bl in range(nblk2):
                                lb = bl * PART
                                mb2 = min(PART, w - lb)
                                k = bl % GRP
                                if k == 0:
                                    row8 = dp2.tile([PART, GRP, HC + 8], F16, tag="d2row8")
                                    al8 = dp2.tile([PART, GRP, 4], F32, tag="d2al8")
                                ps = pp2.tile([PART, 264], F32, tag="d2ps")
                                for kt in range(2):
                                    nc.tensor.matmul(ps[0:mb2, 0:HC],
                                                     lhsT=xs[kt][:, lb:lb + mb2],
                                                     rhs=W2_sb[kt][:],
                                                     start=(kt == 0), stop=(kt == 1))
                                for kt in range(2):
                                    nc.tensor.matmul(ps[0:mb2, HC:HC + 8],
                                                     lhsT=xs[kt][:, lb:lb + mb2],
                                                     rhs=B2_sb[kt][:],
                                                     start=(kt == 0), stop=(kt == 1))
                                nc.scalar.activation(out=row8[:, k, 0:HC],
                                                     in_=ps[:, 0:HC], func=ACTF.Copy)
                                nc.vector.tensor_copy(out=row8[:, k, HC:HC + 8].bitcast(F32),
                                                      in_=ps[:, HC:HC + 4])
                                nc.vector.tensor_copy(out=al8[:, k, :],
                                                      in_=ps[:, HC + 4:HC + 8])
                                if k == GRP - 1 or bl == nblk2 - 1:
                                    # batch-flush the full blocks; a ragged
                                    # tail block (mb2 < PART) is written solo
                                    ng = k + (1 if mb2 == PART else 0)
                                    n0 = r_ * NPC + cc0 + (bl - k) * PART
                                    if ng > 0:
                                        nc.sync.dma_start(
                                            out=T2b[n0:n0 + ng * PART, 0:HC + 8].rearrange(
                                                "(k p) w -> p k w", p=PART),
                                            in_=row8[:, 0:ng, :])
                                        nc.sync.dma_start(
                                            out=al2pk[n0:n0 + ng * PART, :].rearrange(
                                                "(k p) c -> p k c", p=PART),
                                            in_=al8[:, 0:ng, :])
                                    if mb2 < PART:
                                        nr = n0 + k * PART
                                        nc.sync.dma_start(
                                            out=T2b[nr:nr + mb2, 0:HC + 8],
                                            in_=row8[0:mb2, k, :])
                                        nc.sync.dma_start(
                                            out=al2pk[nr:nr + mb2, :],
                                            in_=al8[0:mb2, k, :])
                nc.leave_named_scope("dense2", _sc, False)

                if PHASES < 5:
                    raise _PhaseStopE
                edge_phase(T2b, al2pk, b2cv, x2T, relu=True, scope="edge2")

                # ---------- pooling ----------
                if PHASES < 6:
                    raise _PhaseStopE
                _sc = nc.enter_named_scope("pool", False)[0]
                with tc.tile_pool(name="pl", bufs=1) as pl:
                    mk = pl.tile([PART, NPC], F32, tag="mk")
                    nc.sync.dma_start(out=mk[:], in_=mask_d[0:1, :].to_broadcast([PART, NPC]))
                    cv = pl.tile([PART, NPC], F32, tag="cv")
                    nc.sync.dma_start(out=cv[:], in_=cinv_d[0:1, :].to_broadcast([PART, NPC]))
                    for ct in range(4):
                        xt = pl.tile([PART, NPC], F32, tag="xt")
                        if ct < 2:
                            gld = pl.tile([PART, NPC], F16, tag="gld")
                            for ci in range(NAGC):
                                cc0, cc1 = chunk_cols[ci]
                                nc.sync.dma_start(
                                    out=gld[:, cc0:cc1],
                                    in_=g1h_c[ci][ct * PART:(ct + 1) * PART, :])
                            nc.scalar.activation(out=xt[:], in_=gld[:], func=ACTF.Relu,
                                                 bias=shift_c[:, ct:ct + 1],
                                                 scale=scale_c[:, ct:ct + 1])
                        else:
                            x2l = pl.tile([PART, NPC], F16, tag="x2l")
                            nc.sync.dma_start(out=x2l[:],
                                              in_=x2T[(ct - 2) * PART:(ct - 1) * PART, :])
                            nc.scalar.activation(out=xt[:], in_=x2l[:], func=ACTF.Copy)
                        sm = pl.tile([PART, NPC], F32, tag="sm")
                        nc.vector.tensor_tensor_scan(out=sm[:], data0=mk[:], data1=xt[:],
                                                     initial=0.0, op0=ALU.mult, op1=ALU.max)
                        nc.sync.dma_start(out=omax_d[ct * PART:(ct + 1) * PART, :], in_=sm[:])
                        ss = pl.tile([PART, NPC], F32, tag="ss")
                        nc.vector.tensor_tensor_scan(out=ss[:], data0=mk[:], data1=xt[:],
                                                     initial=0.0, op0=ALU.mult, op1=ALU.add)
                        nc.vector.tensor_tensor(out=ss[:], in0=ss[:], in1=cv[:], op=ALU.mult)
                        nc.sync.dma_start(out=omean_d[ct * PART:(ct + 1) * PART, :], in_=ss[:])
                nc.leave_named_scope("pool", _sc, False)

      except _PhaseStopE:
        pass

    nc.compile()
    return nc


# --------------------------------------------------------------------------
# host-side combine
# --------------------------------------------------------------------------

def postprocess(results, meta):
    lastcol = meta["lastcol"]
    mean = np.zeros((G, 2 * HC), np.float32)
    mx = np.zeros((G, 2 * HC), np.float32)
    for r in range(NCORES):
        om = results[r]["out_mean"]   # [512, NPC], dev channel order
        ox = results[r]["out_max"]
        for g_, col in lastcol[r].items():
            mean[g_] += om[:, col]
            mx[g_] = np.maximum(mx[g_], ox[:, col])
    # un-permute dev channel order back to torch order
    dev2orig = np.concatenate([PERM, HC + PERM])
    mean_o = np.empty_like(mean); mx_o = np.empty_like(mx)
    mean_o[:, dev2orig] = mean
    mx_o[:, dev2orig] = mx
    return np.concatenate([mean_o, mx_o], axis=1).astype(np.float32)


_CACHE = {}


def kernel(**inputs):
    in_maps, meta = preprocess(**inputs)
    key = (meta["NB"], meta["KLO"], meta["KHI"])
    if key not in _CACHE:
        _CACHE[key] = build_program(meta)
    nc = _CACHE[key]
    res = bass_utils.run_bass_kernel_spmd(nc, in_maps, core_ids=list(range(NCORES)))
    return postprocess(res.results, meta)


# revision 37
# speedup vs baseline: 7.2738x; 1.0024x over previous
"""Self-contained Trainium2 Bass kernel for a 2-layer GAT + BatchNorm + graph pooling.

Contract: kernel(**inputs) takes the FULL (unsharded) inputs and returns the
FULL [G, 1024] float32 output.

v3 design: replicated dense layers, fp16 gather tables, tiny overlapped
collectives, channel-interleaved layout for fast DVE broadcasts.

  - Channels are stored (c, h)-interleaved (dev channel c*4+h = torch channel
    h*64+c, permuted host-side in the weights and un-permuted in postprocess)
    so every per-head broadcast multiply has a packed innermost dim of 4 —
    DVE runs these at 16-bit double rate instead of broadcast-stride-0 rate.
  - dense L1 is REPLICATED: every core computes the full table
    T1b[n] = [h(256 fp16) | al_src(4 f32 riding as 8 fp16 slots) | pad]
    (768 B rows) from x; writes are batched 8 blocks per DMA (the HWDGE
    ~600 ns fixed cost per dma_start dominated v2's dense phases).
  - "al_dst" logits live in a plain [N, 4] f32 table whose gather view is
    [N/16, 64] (256 B rows = the dma_gather minimum); the per-edge value is
    extracted with a one-hot-over-16 dot on DVE.  This keeps gather indices
    (node//16) inside int16 and lets the replicated dense write it cheaply.
  - edge phase (per 128-dst-node block): one combined index/metadata DMA,
    dma_gather rows by src (lo/hi split for int16; <=1024 idxs per
    instruction — 2048 hangs the HW, verified), -1-padded index streams skip
    pad transfers (per-core valid counts are reg_load-ed from SBUF),
    softmax-weighted segment sum via 0/1 fp16 selector-matrix matmuls
    accumulating [out | denom] in PSUM.  Logits stay f32 (exp via ACT,
    clamped at +8 so pad garbage cannot overflow fp16).  The per-node
    epilogue folds bias+relu into the post-transpose ACT copy.
  - between layers only x1's pre-BN value g1 (256ch fp16, transposed) is
    AllGathered — 3.2 MB per rank in 4 column-chunks issued as edge L1
    drains, overlapping wire time with edge compute.  BN stats go through a
    [128,4] AllReduce; the BN affine + relu is fused into dense L2's
    activation load.  dense L2 is replicated from the gathered chunks.
  - pooling: per-channel-tile segmented running sum & max along the node
    axis (tensor_tensor_scan); host reads each graph's last column and
    combines the <=2 per-graph partials from adjacent cores.
"""

import numpy as np

import concourse.bass as bass
import concourse.bacc as bacc
import concourse.tile as tile
from concourse import mybir
from concourse import bass_utils
from concourse.masks import make_identity

F32 = mybir.dt.float32
F16 = mybir.dt.float16
I16 = mybir.dt.int16
I32 = mybir.dt.int32
ALU = mybir.AluOpType
ACTF = mybir.ActivationFunctionType

# problem constants (hardcoded per the harness contract)
N, F_IN, C0, C1, H, E, G = 50000, 128, 64, 64, 4, 800000, 256
HC = H * C0            # 256
NEG_SLOPE = 0.2
BN_EPS = 1e-5
NCORES = 8
NPC = N // NCORES      # nodes per core (6250)
SPLIT = 32768          # dma_gather int16 index limit -> split gather table
RWH = 384              # fp16 table row width (768 B): h(256) + al_src(8) + pad
PART = 128
NPAD = 50048           # N rounded to 128 blocks (391 blocks)
NBLK = NPAD // PART    # 391 dense blocks
CHUNK = 8              # 128-idx groups per dma_gather (1024 idx HW limit)
ZCLAMP = 8.0           # logit clamp (real logits ~ +-6); keeps exp fp16-finite
NAGC = 4               # AllGather column-chunks for the inter-layer feature
GRP = 8                # dense blocks batched per table-row DMA

# dev channel k = c*4+h  <->  torch channel h*64+c
PERM = np.array([(k % H) * C0 + k // H for k in range(HC)], np.int64)

PHASES = 6             # build phases 1..6 (bisection aid)


# --------------------------------------------------------------------------
# host-side preprocessing
# --------------------------------------------------------------------------

def _pack16(stream_i16, ncols):
    """dma_gather index layout: position i -> [i%16, i//16], replicated to
    partition groups 16k+p for the 8 Q7 cores."""
    base = stream_i16.reshape(ncols, 16).T          # [16, ncols]
    return np.tile(base, (8, 1)).astype(np.int16)   # [128, ncols]


def _pad_stream(vals, nslots):
    """Pad an index stream to nslots with -1 (skipped by dma_gather) and
    return (idx_i16, per-1024-chunk valid counts).  A chunk with zero valid
    indices gets one dummy index 0 (count 1): the HW needs at least one
    non-negative index per instruction."""
    n = len(vals)
    out = np.full(nslots, -1, np.int16)
    out[:n] = vals
    counts = []
    for c0 in range(0, nslots, CHUNK * PART):
        span = min(CHUNK * PART, nslots - c0)
        cnt = min(max(n - c0, 0), span)
        if cnt == 0:
            out[c0] = 0
            cnt = 1
        counts.append(cnt)
    return out, counts


def preprocess(x, edge_index, batch,
               W1, att_src1, att_dst1, b1, gamma, beta,
               W2, att_src2, att_dst2, b2):
    x = np.asarray(x, np.float32)
    edge_index = np.asarray(edge_index)
    batch = np.asarray(batch).astype(np.int64)
    W1 = np.asarray(W1, np.float32); W2 = np.asarray(W2, np.float32)

    src = np.concatenate([edge_index[0], np.arange(N, dtype=np.int64)])
    dst = np.concatenate([edge_index[1], np.arange(N, dtype=np.int64)])

    NB = (NPC + PART - 1) // PART                      # dst blocks per core

    # ---- per-core edge streams ----
    blocks = []
    nlo_max = nhi_max = 0
    for r in range(NCORES):
        m = (dst >= r * NPC) & (dst < (r + 1) * NPC)
        s_r = src[m]; d_r = dst[m]
        dloc = d_r - r * NPC
        order = np.argsort(dloc, kind="stable")
        s_r = s_r[order]; d_r = d_r[order]; dloc = dloc[order]
        blk = dloc // PART
        core_blocks = []
        for b in range(NB):
            bm = blk == b
            sb_ = s_r[bm]; db_ = d_r[bm]
            lo_m = sb_ < SPLIT
            # sort each gather stream by src: ascending table addresses give
            # HBM row-buffer locality (edge order within a dst block is free
            # — the selector matrix handles any order)
            slo = sb_[lo_m]; dlo = db_[lo_m]
            shi = sb_[~lo_m]; dhi = db_[~lo_m]
            o = np.argsort(slo, kind="stable"); slo = slo[o]; dlo = dlo[o]
            o = np.argsort(shi, kind="stable"); shi = shi[o]; dhi = dhi[o]
            core_blocks.append((slo, shi - SPLIT, dlo, dhi))
            nlo_max = max(nlo_max, len(slo))
            nhi_max = max(nhi_max, len(shi))
        blocks.append(core_blocks)

    KLO = max(1, (nlo_max + PART - 1) // PART)
    KHI = max(1, (nhi_max + PART - 1) // PART)
    KT = KLO + KHI
    CL = (KLO + CHUNK - 1) // CHUNK
    CH = (KHI + CHUNK - 1) // CHUNK
    IDXW = KT * 18       # [il|ih|ial|iah] (KT*16) + dl (KT) + dm (KT), i16

    ib_t = np.zeros((NCORES, NB, PART, IDXW), np.int16)
    cnt_t = np.zeros((NCORES, NB, 8), np.int32)
    for r in range(NCORES):
        for b in range(NB):
            lo_src, hi_src, abs_lo, abs_hi = blocks[r][b]
            ls, c_lo = _pad_stream(lo_src, KLO * PART)
            hs, c_hi = _pad_stream(hi_src, KHI * PART)
            al_lo, _ = _pad_stream(abs_lo // 16, KLO * PART)
            al_hi, _ = _pad_stream(abs_hi // 16, KHI * PART)
            ib_t[r, b, :, 0:KLO * 8] = _pack16(ls, KLO * 8)
            ib_t[r, b, :, KLO * 8:KT * 8] = _pack16(hs, KHI * 8)
            ib_t[r, b, :, KT * 8:KT * 8 + KLO * 8] = _pack16(al_lo, KLO * 8)
            ib_t[r, b, :, KT * 8 + KLO * 8:KT * 16] = _pack16(al_hi, KHI * 8)
            cnt_t[r, b, :CL] = c_lo
            cnt_t[r, b, CL:CL + CH] = c_hi
            dl = np.full(KT * PART, 999.0, np.float32)
            dm = np.zeros(KT * PART, np.float32)
            dl[:len(abs_lo)] = (abs_lo - r * NPC) % PART
            dm[:len(abs_lo)] = abs_lo % 16
            dl[KLO * PART:KLO * PART + len(abs_hi)] = (abs_hi - r * NPC) % PART
            dm[KLO * PART:KLO * PART + len(abs_hi)] = abs_hi % 16
            ib_t[r, b, :, KT * 16:KT * 17] = \
                dl.reshape(KT, PART).T.astype(np.float16).view(np.int16)
            ib_t[r, b, :, KT * 17:KT * 18] = \
                dm.reshape(KT, PART).T.astype(np.float16).view(np.int16)

    # ---- batch-derived pooling metadata ----
    counts = np.bincount(batch, minlength=G).astype(np.float64)
    maskrow = np.zeros((NCORES, 1, NPC), np.float32)
    cinvrow = np.zeros((NCORES, 1, NPC), np.float32)
    lastcol = [dict() for _ in range(NCORES)]
    for r in range(NCORES):
        bseg = batch[r * NPC:(r + 1) * NPC]
        same = np.ones(NPC, np.float32)
        same[0] = 0.0
        same[1:] = (bseg[1:] == bseg[:-1]).astype(np.float32)
        maskrow[r, 0] = same
        cinvrow[r, 0] = (1.0 / np.maximum(counts[bseg], 1.0)).astype(np.float32)
        gids, last_idx = np.unique(bseg[::-1], return_index=True)
        for g_, li in zip(gids, last_idx):
            lastcol[r][int(g_)] = NPC - 1 - int(li)

    # ---- weights (replicated; channel-permuted to dev order) ----
    def bmat(W, a_s, a_d, fin):
        Wr = W.reshape(fin, H, C0)
        bs = np.einsum("khc,hc->kh", Wr, np.asarray(a_s, np.float32))
        bd = np.einsum("khc,hc->kh", Wr, np.asarray(a_d, np.float32))
        return np.concatenate([bs, bd], axis=1).astype(np.float16)  # [fin, 8]

    xh = np.zeros((F_IN, NPAD), np.float16)
    xh[:, :N] = x.T.astype(np.float16)

    W1p = W1[:, PERM]
    W2p = W2[PERM][:, PERM]
    b1p = np.asarray(b1, np.float32)[PERM]
    b2p = np.asarray(b2, np.float32)[PERM]

    shared = dict(
        xh16T=xh,
        W1h=W1p.astype(np.float16), B1h=bmat(W1, att_src1, att_dst1, F_IN),
        W2h=W2p.astype(np.float16),
        B2h=bmat(W2, att_src2, att_dst2, HC)[PERM, :],
        b1colT=b1p.reshape(2, PART).T.copy(),
        b2colT=b2p.reshape(2, PART).T.copy(),
        gcol=np.asarray(gamma, np.float32)[PERM].reshape(2, PART).T.copy(),
        bcol=np.asarray(beta, np.float32)[PERM].reshape(2, PART).T.copy(),
    )
    in_maps = []
    for r in range(NCORES):
        in_maps.append(dict(
            shared,
            ib=ib_t[r],
            cnts=cnt_t[r].reshape(1, NB * 8),
            maskrow=maskrow[r],
            cinvrow=cinvrow[r],
        ))
    meta = dict(NB=NB, KLO=KLO, KHI=KHI, KT=KT, CL=CL, CH=CH,
                lastcol=lastcol, counts=counts)
    return in_maps, meta


# --------------------------------------------------------------------------
# device program
# --------------------------------------------------------------------------

def build_program(meta, sim_local=False):
    NB, KLO, KHI, KT = meta["NB"], meta["KLO"], meta["KHI"], meta["KT"]
    CL, CH = meta["CL"], meta["CH"]
    IDXW = KT * 18
    nc = bacc.Bacc("TRN2", target_bir_lowering=False, debug=False,
                   num_devices=1 if sim_local else NCORES)

    def ein(name, shape, dt=F32):
        return nc.dram_tensor(name, list(shape), dt, kind="ExternalInput").ap()

    xh_d = ein("xh16T", [F_IN, NPAD], F16)
    W1_d = ein("W1h", [F_IN, HC], F16); B1_d = ein("B1h", [F_IN, 8], F16)
    W2_d = ein("W2h", [HC, HC], F16);   B2_d = ein("B2h", [HC, 8], F16)
    b1c_d = ein("b1colT", [PART, 2]); b2c_d = ein("b2colT", [PART, 2])
    gcol_d = ein("gcol", [PART, 2]); bcol_d = ein("bcol", [PART, 2])
    ib_d = ein("ib", [NB, PART, IDXW], I16)
    cnt_d = ein("cnts", [1, NB * 8], I32)
    mask_d = ein("maskrow", [1, NPC])
    cinv_d = ein("cinvrow", [1, NPC])

    omax_d = nc.dram_tensor("out_max", [4 * PART, NPC], F32, kind="ExternalOutput").ap()
    omean_d = nc.dram_tensor("out_mean", [4 * PART, NPC], F32, kind="ExternalOutput").ap()

    # internal DRAM
    T1b = nc.dram_tensor("T1b", [NPAD, RWH], F16).ap()
    T2b = nc.dram_tensor("T2b", [NPAD, RWH], F16).ap()
    al1pk = nc.dram_tensor("al1pk", [NPAD, 4], F32).ap()
    al2pk = nc.dram_tensor("al2pk", [NPAD, 4], F32).ap()
    x2T = nc.dram_tensor("x2T", [HC, NPC], F16).ap()
    ar_in = nc.dram_tensor("ar_in", [PART, 4], F32).ap()
    ar_out = nc.dram_tensor("ar_out", [PART, 4], F32, addr_space="Shared").ap()

    # AllGather chunks of the inter-layer feature (transposed, fp16)
    bpc = (NB + NAGC - 1) // NAGC
    blk_of_chunk = [list(range(c * bpc, min(NB, (c + 1) * bpc)))
                    for c in range(NAGC)]
    chunk_cols = []
    g1h_c, Tag_c = [], []
    for c in range(NAGC):
        c0 = blk_of_chunk[c][0] * PART
        c1 = min(NPC, (blk_of_chunk[c][-1] + 1) * PART)
        chunk_cols.append((c0, c1))
        g1h_c.append(nc.dram_tensor(f"g1h_{c}", [HC, c1 - c0], F16).ap())
        Tag_c.append(nc.dram_tensor(f"Tag_{c}", [NCORES * HC, c1 - c0], F16,
                                    addr_space="Shared").ap())

    rgroups = [list(range(NCORES))]

    class _PhaseStopE(Exception):
        pass

    with tile.TileContext(nc) as tc:
      try:
        # ---------- shared constant tiles ----------
        with tc.tile_pool(name="const", bufs=1) as cpool:
            ident = cpool.tile([PART, PART], F32)
            make_identity(nc, ident[:])
            iota_i = cpool.tile([PART, PART], mybir.dt.int32)
            nc.gpsimd.iota(iota_i[:], pattern=[[1, PART]], base=0,
                           channel_multiplier=0)
            iota_h = cpool.tile([PART, PART], F16)
            nc.vector.tensor_copy(out=iota_h[:], in_=iota_i[:])
            iota16 = cpool.tile([PART, 16], F16)
            nc.vector.tensor_copy(out=iota16[:], in_=iota_i[:, 0:16])

            cnt_sb = cpool.tile([1, NB * 8], I32)
            nc.sync.dma_start(out=cnt_sb[:], in_=cnt_d[:, :])
            b1cv = cpool.tile([PART, 2], F32)
            nc.sync.dma_start(out=b1cv[:], in_=b1c_d[:, :])
            b2cv = cpool.tile([PART, 2], F32)
            nc.sync.dma_start(out=b2cv[:], in_=b2c_d[:, :])

            # ---------- dense L1 (replicated: full table on every core) ----
            _sc = nc.enter_named_scope("dense1", False)[0]
            with tc.tile_pool(name="d1", bufs=3) as dp, \
                 tc.tile_pool(name="d1w", bufs=1) as wp, \
                 tc.tile_pool(name="d1x", bufs=2) as xp, \
                 tc.tile_pool(name="d1ps", bufs=3, space="PSUM") as pp:
                W1_sb = wp.tile([F_IN, HC], F16)
                nc.sync.dma_start(out=W1_sb[:], in_=W1_d[:, :])
                B1_sb = wp.tile([F_IN, 8], F16)
                nc.sync.dma_start(out=B1_sb[:], in_=B1_d[:, :])
                XCH = 6272                      # x column chunk (49 blocks)
                x_sb = None
                row8 = al8 = None
                for b in range(NBLK):
                    if b % 49 == 0:
                        x_sb = xp.tile([F_IN, XCH], F16, tag="xsb")
                        x0 = b * PART
                        nc.sync.dma_start(out=x_sb[:, 0:min(XCH, NPAD - x0)],
                                          in_=xh_d[:, x0:min(x0 + XCH, NPAD)])
                    k = b % GRP
                    if k == 0:
                        row8 = dp.tile([PART, GRP, HC + 8], F16, tag="row8")
                        al8 = dp.tile([PART, GRP, 4], F32, tag="al8")
                    col = (b % 49) * PART
                    j = b % 2
                    if j == 0:
                        ps_h = pp.tile([PART, 2, HC], F32, tag="dpsh")
                        ps_al = pp.tile([PART, 2, 8], F32, tag="dpsal")
                    nc.tensor.matmul(ps_h[:, j, :], lhsT=x_sb[:, col:col + PART],
                                     rhs=W1_sb[:], start=True, stop=True)
                    nc.tensor.matmul(ps_al[:, j, :], lhsT=x_sb[:, col:col + PART],
                                     rhs=B1_sb[:], start=True, stop=True)
                    if j == 1 or b == NBLK - 1:
                        nj = j + 1
                        k0 = k - j
                        nc.scalar.activation(out=row8[:, k0:k0 + nj, 0:HC],
                                             in_=ps_h[:, 0:nj, :], func=ACTF.Copy)
                        nc.vector.tensor_copy(
                            out=row8[:, k0:k0 + nj, HC:HC + 8].bitcast(F32),
                            in_=ps_al[:, 0:nj, 0:4])
                        nc.vector.tensor_copy(out=al8[:, k0:k0 + nj, :],
                                              in_=ps_al[:, 0:nj, 4:8])
                    if k == GRP - 1 or b == NBLK - 1:
                        ng = k + 1
                        n0 = (b - k) * PART
                        nc.sync.dma_start(
                            out=T1b[n0:n0 + ng * PART, 0:HC + 8].rearrange(
                                "(k p) w -> p k w", p=PART),
                            in_=row8[:, 0:ng, :])
                        nc.sync.dma_start(
                            out=al1pk[n0:n0 + ng * PART, :].rearrange(
                                "(k p) c -> p k c", p=PART),
                            in_=al8[:, 0:ng, :])
            nc.leave_named_scope("dense1", _sc, False)

            # ---------- edge phase (shared for both layers) ----------
            def edge_phase(Tbl, alpk, bias_cv, outT, relu, scope, ag=False):
                """outT: None for L1 (writes g1h chunks), else x2T."""
                _es = nc.enter_named_scope(scope, False)[0]
                alview = alpk[:, :].rearrange("(r j) c -> r (j c)", j=16)
                with tc.tile_pool(name="eidx", bufs=2) as ip, \
                     tc.tile_pool(name="eg", bufs=2) as gp, \
                     tc.tile_pool(name="ew", bufs=2) as wp2, \
                     tc.tile_pool(name="eps", bufs=2, space="PSUM") as ep, \
                     tc.tile_pool(name="etps", bufs=2, space="PSUM") as tps:
                    # pre-zero both gather buffers: -1-skipped slots must hold
                    # finite floats (uninitialized SBUF could be NaN -> NaN*0
                    # = NaN in PSUM)
                    for _z in range(2):
                        for tg, shp, dt_ in (("gall", [PART, KT, RWH], F16),
                                             ("ga", [PART, KT, 64], F32)):
                            zt = gp.tile(shp, dt_, tag=tg)
                            nc.vector.memset(zt[:], 0.0)

                    cnt_regs = [nc.gpsimd.alloc_register(f"cnt_{scope}_{i}")
                                for i in range(4)]
                    reg_rr = [0]

                    def gather(gtile, src_ap, ixtile, ktot, elem, cnt_base):
                        for ci, c0 in enumerate(range(0, ktot, CHUNK)):
                            cw = min(CHUNK, ktot - c0)
                            reg = cnt_regs[reg_rr[0] % 4]
                            reg_rr[0] += 1
                            nc.gpsimd.reg_load(
                                reg, cnt_sb[0:1, cnt_base + ci:cnt_base + ci + 1])
                            nc.gpsimd.dma_gather(
                                out_ap=gtile[:, c0:c0 + cw, :],
                                in_ap=src_ap, idxs_ap=ixtile[:, c0 * 8:(c0 + cw) * 8],
                                num_idxs=cw * PART, num_idxs_reg=reg,
                                elem_size=elem)

                    for b in range(NB):
                        mb = min(PART, NPC - b * PART)
                        ib = ip.tile([PART, IDXW], I16, tag="ib")
                        nc.sync.dma_start(out=ib[:], in_=ib_d[b, :, :])
                        il = ib[:, 0:KLO * 8]
                        ih = ib[:, KLO * 8:KT * 8]
                        ial = ib[:, KT * 8:KT * 8 + KLO * 8]
                        iah = ib[:, KT * 8 + KLO * 8:KT * 16]
                        dl = ib[:, KT * 16:KT * 17].bitcast(F16)
                        dm = ib[:, KT * 17:KT * 18].bitcast(F16)

                        gall = gp.tile([PART, KT, RWH], F16, tag="gall")
                        gather(gall[:, 0:KLO, :], Tbl[0:SPLIT, :], il, KLO, RWH, b * 8)
                        gather(gall[:, KLO:KT, :], Tbl[SPLIT:NPAD, :], ih, KHI, RWH,
                               b * 8 + CL)
                        ga = gp.tile([PART, KT, 64], F32, tag="ga")
                        gather(ga[:, 0:KLO, :], alview, ial, KLO, 64, b * 8)
                        gather(ga[:, KLO:KT, :], alview, iah, KHI, 64, b * 8 + CL)

                        # selector matrix S01[e, kt, d] = (dl == d), fp16
                        S01 = wp2.tile([PART, KT, PART], F16, tag="S01")
                        nc.vector.tensor_tensor(
                            out=S01[:],
                            in0=dl[:].unsqueeze(-1).to_broadcast([PART, KT, PART]),
                            in1=iota_h[:].unsqueeze(1).to_broadcast([PART, KT, PART]),
                            op=ALU.is_equal)

                        # al_dst extraction: one-hot over the 16-node pack
                        oh = wp2.tile([PART, KT, 16], F32, tag="oh")
                        nc.vector.tensor_tensor(
                            out=oh[:],
                            in0=dm[:].unsqueeze(-1).to_broadcast([PART, KT, 16]),
                            in1=iota16[:].unsqueeze(1).to_broadcast([PART, KT, 16]),
                            op=ALU.is_equal)
                        adp = wp2.tile([PART, KT, 4, 16], F32, tag="adp")
                        nc.vector.tensor_tensor(
                            out=adp[:],
                            in0=ga[:].rearrange("p k (j h) -> p k h j", j=16),
                            in1=oh[:].unsqueeze(2).to_broadcast([PART, KT, 4, 16]),
                            op=ALU.mult)
                        Z = wp2.tile([PART, KT, 4], F32, tag="Z")
                        nc.vector.tensor_reduce(
                            out=Z[:].unsqueeze(-1), in_=adp[:],
                            axis=mybir.AxisListType.X, op=ALU.add)
                        nc.vector.tensor_tensor(
                            out=Z[:], in0=Z[:],
                            in1=gall[:, :, HC:HC + 8].bitcast(F32), op=ALU.add)
                        # leaky-relu (one fused op), clamp, exp -> fp16
                        nc.vector.scalar_tensor_tensor(
                            out=Z[:], in0=Z[:], scalar=NEG_SLOPE, in1=Z[:],
                            op0=ALU.mult, op1=ALU.max)
                        nc.vector.tensor_scalar_min(out=Z[:], in0=Z[:], scalar1=ZCLAMP)
                        EXh = wp2.tile([PART, KT, 4], F16, tag="EXh")
                        nc.scalar.activation(out=EXh[:], in_=Z[:], func=ACTF.Exp)

                        # Hp = [ex-weighted h | ex] (fp16, (c,h)-interleaved)
                        Hp = wp2.tile([PART, KT, 260], F16, tag="Hp")
                        nc.vector.tensor_tensor(
                            out=Hp[:, :, 0:HC].rearrange("p k (c h) -> p k c h", h=H),
                            in0=gall[:, :, 0:HC].rearrange("p k (c h) -> p k c h", h=H),
                            in1=EXh[:].unsqueeze(2).to_broadcast([PART, KT, C0, H]),
                            op=ALU.mult)
                        nc.vector.tensor_copy(out=Hp[:, :, HC:HC + 4], in_=EXh[:])

                        acc = ep.tile([PART, 260], F32, tag="acc")
                        for e in range(KT):
                            nc.tensor.matmul(acc[:], lhsT=S01[:, e, :], rhs=Hp[:, e, :],
                                             start=(e == 0), stop=(e == KT - 1))

                        dn = wp2.tile([PART, 4], F32, tag="dn")
                        nc.vector.tensor_scalar_add(out=dn[:], in0=acc[:, HC:HC + 4],
                                                    scalar1=1e-16)
                        rec = wp2.tile([PART, 4], F32, tag="rec")
                        nc.vector.reciprocal(out=rec[:], in_=dn[:])
                        ob = wp2.tile([PART, HC], F32, tag="ob")
                        nc.vector.tensor_tensor(
                            out=ob[:].rearrange("p (c h) -> p c h", h=H),
                            in0=acc[:, 0:HC].rearrange("p (c h) -> p c h", h=H),
                            in1=rec[:].unsqueeze(1).to_broadcast([PART, C0, H]),
                            op=ALU.mult)
                        for ct in range(2):
                            tp = tps.tile([PART, PART], F32, tag="ttp")
                            nc.tensor.transpose(out=tp[:], in_=ob[:, ct * PART:(ct + 1) * PART],
                                                identity=ident[:])
                            tsh = wp2.tile([PART, PART], F16, tag="tsh")
                            nc.scalar.activation(out=tsh[:], in_=tp[:],
                                                 func=ACTF.Relu if relu else ACTF.Identity,
                                                 bias=bias_cv[:, ct:ct + 1])
                            if outT is None:
                                ci = min(b // bpc, NAGC - 1)
                                cc0 = chunk_cols[ci][0]
                                nc.sync.dma_start(
                                    out=g1h_c[ci][ct * PART:(ct + 1) * PART,
                                                  b * PART - cc0:b * PART - cc0 + mb],
                                    in_=tsh[:, 0:mb])
                            else:
                                nc.sync.dma_start(
                                    out=outT[ct * PART:(ct + 1) * PART,
                                             b * PART:b * PART + mb],
                                    in_=tsh[:, 0:mb])
                        if ag:
                            ci = min(b // bpc, NAGC - 1)
                            if b == blk_of_chunk[ci][-1]:
                                if sim_local:
                                    for r_ in range(NCORES):
                                        nc.sync.dma_start(
                                            out=Tag_c[ci][r_ * HC:(r_ + 1) * HC, :],
                                            in_=g1h_c[ci][:, :])
                                else:
                                    nc.gpsimd.collective_compute(
                                        "AllGather", ALU.bypass,
                                        replica_groups=rgroups,
                                        ins=[g1h_c[ci][:, :]],
                                        outs=[Tag_c[ci][:, :]])
                nc.leave_named_scope(scope, _es, False)

            if PHASES >= 2:
                edge_phase(T1b, al1pk, b1cv, None, relu=False, scope="edge1",
                           ag=True)

            # ---------- BN stats + AllReduce ----------
            if PHASES < 3:
                raise _PhaseStopE
            _sc = nc.enter_named_scope("bnstat", False)[0]
            with tc.tile_pool(name="st", bufs=1) as sp, \
                 tc.tile_pool(name="stw", bufs=1) as sw:
                stats = sw.tile([PART, 4], F32)
                for ct in range(2):
                    gt = sp.tile([PART, NPC], F16, tag="gt")
                    for ci in range(NAGC):
                        cc0, cc1 = chunk_cols[ci]
                        nc.sync.dma_start(
                            out=gt[:, cc0:cc1],
                            in_=g1h_c[ci][ct * PART:(ct + 1) * PART, :])
                    nc.vector.tensor_reduce(out=stats[:, ct:ct + 1], in_=gt[:],
                                            axis=mybir.AxisListType.X, op=ALU.add)
                    sq = sp.tile([PART, NPC], F32, tag="sq")
                    nc.scalar.activation(out=sq[:], in_=gt[:], func=ACTF.Square)
                    nc.vector.tensor_reduce(out=stats[:, 2 + ct:3 + ct], in_=sq[:],
                                            axis=mybir.AxisListType.X, op=ALU.add)
                nc.sync.dma_start(out=ar_in[:, :], in_=stats[:])
            nc.leave_named_scope("bnstat", _sc, False)

            _sc = nc.enter_named_scope("ar", False)[0]
            if sim_local:
                nc.sync.dma_start(out=ar_out[:, :], in_=ar_in[:, :])
            else:
                nc.gpsimd.collective_compute(
                    "AllReduce", ALU.add, replica_groups=rgroups,
                    ins=[ar_in[:, :]], outs=[ar_out[:, :]])
            nc.leave_named_scope("ar", _sc, False)

            with tc.tile_pool(name="bnw", bufs=1) as bw:
                ar_sb = bw.tile([PART, 4], F32)
                nc.sync.dma_start(out=ar_sb[:], in_=ar_out[:, :])
                mean = bw.tile([PART, 2], F32)
                nc.vector.tensor_scalar_mul(out=mean[:], in0=ar_sb[:, 0:2], scalar1=1.0 / N)
                msq = bw.tile([PART, 2], F32)
                nc.vector.tensor_scalar_mul(out=msq[:], in0=ar_sb[:, 2:4], scalar1=1.0 / N)
                var = bw.tile([PART, 2], F32)
                nc.vector.tensor_tensor(out=var[:], in0=mean[:], in1=mean[:], op=ALU.mult)
                nc.vector.tensor_tensor(out=var[:], in0=msq[:], in1=var[:], op=ALU.subtract)
                nc.vector.tensor_scalar_add(out=var[:], in0=var[:], scalar1=BN_EPS)
                sd = bw.tile([PART, 2], F32)
                nc.scalar.activation(out=sd[:], in_=var[:], func=ACTF.Sqrt)
                rinv = bw.tile([PART, 2], F32)
                nc.vector.reciprocal(out=rinv[:], in_=sd[:])
                gc = bw.tile([PART, 2], F32)
                nc.sync.dma_start(out=gc[:], in_=gcol_d[:, :])
                bc = bw.tile([PART, 2], F32)
                nc.sync.dma_start(out=bc[:], in_=bcol_d[:, :])
                scale_c = bw.tile([PART, 2], F32)
                nc.vector.tensor_tensor(out=scale_c[:], in0=gc[:], in1=rinv[:], op=ALU.mult)
                shift_c = bw.tile([PART, 2], F32)
                nc.vector.tensor_tensor(out=shift_c[:], in0=mean[:], in1=scale_c[:], op=ALU.mult)
                nc.vector.tensor_tensor(out=shift_c[:], in0=bc[:], in1=shift_c[:], op=ALU.subtract)

                # ---------- dense L2 (replicated, from AllGathered x1) -----
                if PHASES < 4:
                    raise _PhaseStopE
                _sc = nc.enter_named_scope("dense2", False)[0]
                with tc.tile_pool(name="d2", bufs=3) as dp2, \
                     tc.tile_pool(name="d2w", bufs=1) as wp3, \
                     tc.tile_pool(name="d2x", bufs=2) as xp2, \
                     tc.tile_pool(name="d2ps", bufs=3, space="PSUM") as pp2:
                    W2_sb = [wp3.tile([PART, HC], F16, tag=f"w2_{kt}", name=f"w2_{kt}")
                             for kt in range(2)]
                    B2_sb = [wp3.tile([PART, 8], F16, tag=f"b2_{kt}", name=f"b2_{kt}")
                             for kt in range(2)]
                    for kt in range(2):
                        nc.sync.dma_start(out=W2_sb[kt][:],
                                          in_=W2_d[kt * PART:(kt + 1) * PART, :])
                        nc.sync.dma_start(out=B2_sb[kt][:],
                                          in_=B2_d[kt * PART:(kt + 1) * PART, :])
                    for r_ in range(NCORES):
                        for ci in range(NAGC):
                            cc0, cc1 = chunk_cols[ci]
                            w = cc1 - cc0
                            xs = []
                            for kt in range(2):
                                gl = xp2.tile([PART, bpc * PART], F16, tag=f"gl{kt}",
                                              name=f"gl{kt}")
                                nc.sync.dma_start(
                                    out=gl[:, 0:w],
                                    in_=Tag_c[ci][r_ * HC + kt * PART:r_ * HC + (kt + 1) * PART, :])
                                x1s = xp2.tile([PART, bpc * PART], F16, tag=f"x1s{kt}",
                                               name=f"x1s{kt}")
                                nc.scalar.activation(out=x1s[:, 0:w], in_=gl[:, 0:w],
                                                     func=ACTF.Relu,
                                                     bias=shift_c[:, kt:kt + 1],
                                                     scale=scale_c[:, kt:kt + 1])
                                xs.append(x1s)
                            nblk2 = (w + PART - 1) // PART
                            row8 = al8 = None
                            for bl in range(nblk2):
                                lb = bl * PART
                                mb2 = min(PART, w - lb)
                                k = bl % GRP
                                if k == 0:
                                    row8 = dp2.tile([PART, GRP, HC + 8], F16, tag="d2row8")
                                    al8 = dp2.tile([PART, GRP, 4], F32, tag="d2al8")
                                j = bl % 2
                                if j == 0:
                                    ps_h = pp2.tile([PART, 2, HC], F32, tag="d2psh")
                                    ps_al = pp2.tile([PART, 2, 8], F32, tag="d2psal")
                                for kt in range(2):
                                    nc.tensor.matmul(ps_h[0:mb2, j, :],
                                                     lhsT=xs[kt][:, lb:lb + mb2],
                                                     rhs=W2_sb[kt][:],
                                                     start=(kt == 0), stop=(kt == 1))
                                for kt in range(2):
                                    nc.tensor.matmul(ps_al[0:mb2, j, :],
                                                     lhsT=xs[kt][:, lb:lb + mb2],
                                                     rhs=B2_sb[kt][:],
                                                     start=(kt == 0), stop=(kt == 1))
                                if j == 1 or bl == nblk2 - 1:
                                    nj = j + 1
                                    k0 = k - j
                                    nc.scalar.activation(out=row8[:, k0:k0 + nj, 0:HC],
                                                         in_=ps_h[:, 0:nj, :],
                                                         func=ACTF.Copy)
                                    nc.vector.tensor_copy(
                                        out=row8[:, k0:k0 + nj, HC:HC + 8].bitcast(F32),
                                        in_=ps_al[:, 0:nj, 0:4])
                                    nc.vector.tensor_copy(out=al8[:, k0:k0 + nj, :],
                                                          in_=ps_al[:, 0:nj, 4:8])
                                if k == GRP - 1 or bl == nblk2 - 1:
                                    # batch-flush the full blocks; a ragged
                                    # tail block (mb2 < PART) is written solo
                                    ng = k + (1 if mb2 == PART else 0)
                                    n0 = r_ * NPC + cc0 + (bl - k) * PART
                                    if ng > 0:
                                        nc.sync.dma_start(
                                            out=T2b[n0:n0 + ng * PART, 0:HC + 8].rearrange(
                                                "(k p) w -> p k w", p=PART),
                                            in_=row8[:, 0:ng, :])
                                        nc.sync.dma_start(
                                            out=al2pk[n0:n0 + ng * PART, :].rearrange(
                                                "(k p) c -> p k c", p=PART),
                                            in_=al8[:, 0:ng, :])
                                    if mb2 < PART:
                                        nr = n0 + k * PART
                                        nc.sync.dma_start(
                                            out=T2b[nr:nr + mb2, 0:HC + 8],
                                            in_=row8[0:mb2, k, :])
                                        nc.sync.dma_start(
                                            out=al2pk[nr:nr + mb2, :],
                                            in_=al8[0:mb2, k, :])
                nc.leave_named_scope("dense2", _sc, False)

                if PHASES < 5:
                    raise _PhaseStopE
                edge_phase(T2b, al2pk, b2cv, x2T, relu=True, scope="edge2")

                # ---------- pooling ----------
                if PHASES < 6:
                    raise _PhaseStopE
                _sc = nc.enter_named_scope("pool", False)[0]
                with tc.tile_pool(name="pl", bufs=1) as pl:
                    mk = pl.tile([PART, NPC], F32, tag="mk")
                    nc.sync.dma_start(out=mk[:], in_=mask_d[0:1, :].to_broadcast([PART, NPC]))
                    cv = pl.tile([PART, NPC], F32, tag="cv")
                    nc.sync.dma_start(out=cv[:], in_=cinv_d[0:1, :].to_broadcast([PART, NPC]))
                    for ct in range(4):
                        xt = pl.tile([PART, NPC], F32, tag="xt")
                        if ct < 2:
                            gld = pl.tile([PART, NPC], F16, tag="gld")
                            for ci in range(NAGC):
                                cc0, cc1 = chunk_cols[ci]
                                nc.sync.dma_start(
                                    out=gld[:, cc0:cc1],
                                    in_=g1h_c[ci][ct * PART:(ct + 1) * PART, :])
                            nc.scalar.activation(out=xt[:], in_=gld[:], func=ACTF.Relu,
                                                 bias=shift_c[:, ct:ct + 1],
                                                 scale=scale_c[:, ct:ct + 1])
                        else:
                            x2l = pl.tile([PART, NPC], F16, tag="x2l")
                            nc.sync.dma_start(out=x2l[:],
                                              in_=x2T[(ct - 2) * PART:(ct - 1) * PART, :])
                            nc.scalar.activation(out=xt[:], in_=x2l[:], func=ACTF.Copy)
                        sm = pl.tile([PART, NPC], F32, tag="sm")
                        nc.vector.tensor_tensor_scan(out=sm[:], data0=mk[:], data1=xt[:],
                                                     initial=0.0, op0=ALU.mult, op1=ALU.max)
                        nc.sync.dma_start(out=omax_d[ct * PART:(ct + 1) * PART, :], in_=sm[:])
                        ss = pl.tile([PART, NPC], F32, tag="ss")
                        nc.vector.tensor_tensor_scan(out=ss[:], data0=mk[:], data1=xt[:],
                                                     initial=0.0, op0=ALU.mult, op1=ALU.add)
                        nc.vector.tensor_tensor(out=ss[:], in0=ss[:], in1=cv[:], op=ALU.mult)
                        nc.sync.dma_start(out=omean_d[ct * PART:(ct + 1) * PART, :], in_=ss[:])
                nc.leave_named_scope("pool", _sc, False)

      except _PhaseStopE:
        pass

    nc.compile()
    return nc


# --------------------------------------------------------------------------
# host-side combine
# --------------------------------------------------------------------------

def postprocess(results, meta):
    lastcol = meta["lastcol"]
    mean = np.zeros((G, 2 * HC), np.float32)
    mx = np.zeros((G, 2 * HC), np.float32)
    for r in range(NCORES):
        om = results[r]["out_mean"]   # [512, NPC], dev channel order
        ox = results[r]["out_max"]
        for g_, col in lastcol[r].items():
            mean[g_] += om[:, col]
            mx[g_] = np.maximum(mx[g_], ox[:, col])
    # un-permute dev channel order back to torch order
    dev2orig = np.concatenate([PERM, HC + PERM])
    mean_o = np.empty_like(mean); mx_o = np.empty_like(mx)
    mean_o[:, dev2orig] = mean
    mx_o[:, dev2orig] = mx
    return np.concatenate([mean_o, mx_o], axis=1).astype(np.float32)


_CACHE = {}


def kernel(**inputs):
    in_maps, meta = preprocess(**inputs)
    key = (meta["NB"], meta["KLO"], meta["KHI"])
    if key not in _CACHE:
        _CACHE[key] = build_program(meta)
    nc = _CACHE[key]
    res = bass_utils.run_bass_kernel_spmd(nc, in_maps, core_ids=list(range(NCORES)))
    return postprocess(res.results, meta)


# revision 41
# speedup vs baseline: 8.8202x; 1.2126x over previous
"""Self-contained Trainium2 Bass kernel for a 2-layer GAT + BatchNorm + graph pooling.

Contract: kernel(**inputs) takes the FULL (unsharded) inputs and returns the
FULL [G, 1024] float32 output.

v3 design: replicated dense layers, fp16 gather tables, tiny overlapped
collectives, channel-interleaved layout for fast DVE broadcasts.

  - Channels are stored (c, h)-interleaved (dev channel c*4+h = torch channel
    h*64+c, permuted host-side in the weights and un-permuted in postprocess)
    so every per-head broadcast multiply has a packed innermost dim of 4 —
    DVE runs these at 16-bit double rate instead of broadcast-stride-0 rate.
  - dense L1 is REPLICATED: every core computes the full table
    T1b[n] = [h(256 fp16) | al_src(4 f32 riding as 8 fp16 slots) | pad]
    (768 B rows) from x; writes are batched 8 blocks per DMA (the HWDGE
    ~600 ns fixed cost per dma_start dominated v2's dense phases).
  - "al_dst" logits live in a plain [N, 4] f32 table whose gather view is
    [N/16, 64] (256 B rows = the dma_gather minimum); the per-edge value is
    extracted with a one-hot-over-16 dot on DVE.  This keeps gather indices
    (node//16) inside int16 and lets the replicated dense write it cheaply.
  - edge phase (per 128-dst-node block): one combined index/metadata DMA,
    dma_gather rows by src (lo/hi split for int16; <=1024 idxs per
    instruction — 2048 hangs the HW, verified), -1-padded index streams skip
    pad transfers (per-core valid counts are reg_load-ed from SBUF),
    softmax-weighted segment sum via 0/1 fp16 selector-matrix matmuls
    accumulating [out | denom] in PSUM.  Logits stay f32 (exp via ACT,
    clamped at +8 so pad garbage cannot overflow fp16).  The per-node
    epilogue folds bias+relu into the post-transpose ACT copy.
  - between layers only x1's pre-BN value g1 (256ch fp16, transposed) is
    AllGathered — 3.2 MB per rank in 4 column-chunks issued as edge L1
    drains, overlapping wire time with edge compute.  BN stats go through a
    [128,4] AllReduce; the BN affine + relu is fused into dense L2's
    activation load.  dense L2 is replicated from the gathered chunks.
  - pooling: per-channel-tile segmented running sum & max along the node
    axis (tensor_tensor_scan); host reads each graph's last column and
    combines the <=2 per-graph partials from adjacent cores.
"""

import numpy as np

import concourse.bass as bass
import concourse.bacc as bacc
import concourse.tile as tile
from concourse import mybir
from concourse import bass_utils
from concourse.masks import make_identity

F32 = mybir.dt.float32
F16 = mybir.dt.float16
I16 = mybir.dt.int16
I32 = mybir.dt.int32
ALU = mybir.AluOpType
ACTF = mybir.ActivationFunctionType

# problem constants (hardcoded per the harness contract)
N, F_IN, C0, C1, H, E, G = 50000, 128, 64, 64, 4, 800000, 256
HC = H * C0            # 256
NEG_SLOPE = 0.2
BN_EPS = 1e-5
NCORES = 8
NPC = N // NCORES      # nodes per core (6250)
SPLIT = 32768          # dma_gather int16 index limit -> split gather table
RWH = 384              # fp16 table row width (768 B): h(256) + al_src(8) + pad
PART = 128
NPAD = 50048           # N rounded to 128 blocks (391 blocks)
NBLK = NPAD // PART    # 391 dense blocks
CHUNK = 8              # 128-idx groups per dma_gather (1024 idx HW limit)
ZCLAMP = 8.0           # logit clamp (real logits ~ +-6); keeps exp fp16-finite
NAGC = 4               # AllGather column-chunks for the inter-layer feature
NO_COLL = False        # bisection aid: skip collectives entirely
GRP = 8                # dense blocks batched per table-row DMA

# dev channel k = c*4+h  <->  torch channel h*64+c
PERM = np.array([(k % H) * C0 + k // H for k in range(HC)], np.int64)

PHASES = 6             # build phases 1..6 (bisection aid)


# --------------------------------------------------------------------------
# host-side preprocessing
# --------------------------------------------------------------------------

def _pack16(stream_i16, ncols):
    """dma_gather index layout: position i -> [i%16, i//16], replicated to
    partition groups 16k+p for the 8 Q7 cores."""
    base = stream_i16.reshape(ncols, 16).T          # [16, ncols]
    return np.tile(base, (8, 1)).astype(np.int16)   # [128, ncols]


def _pad_stream(vals, nslots):
    """Pad an index stream to nslots with -1 (skipped by dma_gather) and
    return (idx_i16, per-1024-chunk valid counts).  A chunk with zero valid
    indices gets one dummy index 0 (count 1): the HW needs at least one
    non-negative index per instruction."""
    n = len(vals)
    out = np.full(nslots, -1, np.int16)
    out[:n] = vals
    counts = []
    for c0 in range(0, nslots, CHUNK * PART):
        span = min(CHUNK * PART, nslots - c0)
        cnt = min(max(n - c0, 0), span)
        if cnt == 0:
            out[c0] = 0
            cnt = 1
        counts.append(cnt)
    return out, counts


def preprocess(x, edge_index, batch,
               W1, att_src1, att_dst1, b1, gamma, beta,
               W2, att_src2, att_dst2, b2):
    x = np.asarray(x, np.float32)
    edge_index = np.asarray(edge_index)
    batch = np.asarray(batch).astype(np.int64)
    W1 = np.asarray(W1, np.float32); W2 = np.asarray(W2, np.float32)

    src = np.concatenate([edge_index[0], np.arange(N, dtype=np.int64)])
    dst = np.concatenate([edge_index[1], np.arange(N, dtype=np.int64)])

    NB = (NPC + PART - 1) // PART                      # dst blocks per core

    # ---- per-core edge streams ----
    blocks = []
    nlo_max = nhi_max = 0
    for r in range(NCORES):
        m = (dst >= r * NPC) & (dst < (r + 1) * NPC)
        s_r = src[m]; d_r = dst[m]
        dloc = d_r - r * NPC
        order = np.argsort(dloc, kind="stable")
        s_r = s_r[order]; d_r = d_r[order]; dloc = dloc[order]
        blk = dloc // PART
        core_blocks = []
        for b in range(NB):
            bm = blk == b
            sb_ = s_r[bm]; db_ = d_r[bm]
            lo_m = sb_ < SPLIT
            # sort each gather stream by src: ascending table addresses give
            # HBM row-buffer locality (edge order within a dst block is free
            # — the selector matrix handles any order)
            slo = sb_[lo_m]; dlo = db_[lo_m]
            shi = sb_[~lo_m]; dhi = db_[~lo_m]
            o = np.argsort(slo, kind="stable"); slo = slo[o]; dlo = dlo[o]
            o = np.argsort(shi, kind="stable"); shi = shi[o]; dhi = dhi[o]
            core_blocks.append((slo, shi - SPLIT, dlo, dhi))
            nlo_max = max(nlo_max, len(slo))
            nhi_max = max(nhi_max, len(shi))
        blocks.append(core_blocks)

    KLO = max(1, (nlo_max + PART - 1) // PART)
    KHI = max(1, (nhi_max + PART - 1) // PART)
    KT = KLO + KHI
    CL = (KLO + CHUNK - 1) // CHUNK
    CH = (KHI + CHUNK - 1) // CHUNK
    IDXW = KT * 9 + 9    # [il|ih] (KT*8) + dl (KT) + dmn (1) + idxn (8), i16

    ib_t = np.zeros((NCORES, NB, PART, IDXW), np.int16)
    dlT_t = np.zeros((NCORES, NB, 1, KT * PART), np.float16)
    cnt_t = np.zeros((NCORES, NB, 8), np.int32)
    for r in range(NCORES):
        for b in range(NB):
            lo_src, hi_src, abs_lo, abs_hi = blocks[r][b]
            ls, c_lo = _pad_stream(lo_src, KLO * PART)
            hs, c_hi = _pad_stream(hi_src, KHI * PART)
            ib_t[r, b, :, 0:KLO * 8] = _pack16(ls, KLO * 8)
            ib_t[r, b, :, KLO * 8:KT * 8] = _pack16(hs, KHI * 8)
            cnt_t[r, b, :CL] = c_lo
            cnt_t[r, b, CL:CL + CH] = c_hi
            dl = np.full(KT * PART, 999.0, np.float32)
            dl[:len(abs_lo)] = (abs_lo - r * NPC) % PART
            dl[KLO * PART:KLO * PART + len(abs_hi)] = (abs_hi - r * NPC) % PART
            ib_t[r, b, :, KT * 8:KT * 9] = \
                dl.reshape(KT, PART).T.astype(np.float16).view(np.int16)
            dlT_t[r, b, 0] = dl.astype(np.float16)
            # per-NODE al_dst gather: one 256-B packed row per own dst node
            mb = min(PART, NPC - b * PART)
            nodes = r * NPC + b * PART + np.arange(mb)
            ixn, c_n = _pad_stream(nodes // 16, PART)
            cnt_t[r, b, CL + CH] = c_n[0]
            dmn = np.zeros(PART, np.float32)
            dmn[:mb] = nodes % 16
            ib_t[r, b, :, KT * 9:KT * 9 + 1] = \
                dmn.astype(np.float16).view(np.int16).reshape(PART, 1)
            ib_t[r, b, :, KT * 9 + 1:KT * 9 + 9] = _pack16(ixn, 8)

    # ---- batch-derived pooling metadata ----
    counts = np.bincount(batch, minlength=G).astype(np.float64)
    maskrow = np.zeros((NCORES, 1, NPC), np.float32)
    cinvrow = np.zeros((NCORES, 1, NPC), np.float32)
    lastcol = [dict() for _ in range(NCORES)]
    for r in range(NCORES):
        bseg = batch[r * NPC:(r + 1) * NPC]
        same = np.ones(NPC, np.float32)
        same[0] = 0.0
        same[1:] = (bseg[1:] == bseg[:-1]).astype(np.float32)
        maskrow[r, 0] = same
        cinvrow[r, 0] = (1.0 / np.maximum(counts[bseg], 1.0)).astype(np.float32)
        gids, last_idx = np.unique(bseg[::-1], return_index=True)
        for g_, li in zip(gids, last_idx):
            lastcol[r][int(g_)] = NPC - 1 - int(li)

    # ---- weights (replicated; channel-permuted to dev order) ----
    def bmat(W, a_s, a_d, fin):
        Wr = W.reshape(fin, H, C0)
        bs = np.einsum("khc,hc->kh", Wr, np.asarray(a_s, np.float32))
        bd = np.einsum("khc,hc->kh", Wr, np.asarray(a_d, np.float32))
        return np.concatenate([bs, bd], axis=1).astype(np.float16)  # [fin, 8]

    xh = np.zeros((F_IN, NPAD), np.float16)
    xh[:, :N] = x.T.astype(np.float16)

    W1p = W1[:, PERM]
    W2p = W2[PERM][:, PERM]
    b1p = np.asarray(b1, np.float32)[PERM]
    b2p = np.asarray(b2, np.float32)[PERM]

    shared = dict(
        xh16T=xh,
        W1h=W1p.astype(np.float16), B1h=bmat(W1, att_src1, att_dst1, F_IN),
        W2h=W2p.astype(np.float16),
        B2h=bmat(W2, att_src2, att_dst2, HC)[PERM, :],
        b1colT=b1p.reshape(2, PART).T.copy(),
        b2colT=b2p.reshape(2, PART).T.copy(),
        gcol=np.asarray(gamma, np.float32)[PERM].reshape(2, PART).T.copy(),
        bcol=np.asarray(beta, np.float32)[PERM].reshape(2, PART).T.copy(),
        iotap=np.arange(PART, dtype=np.float32).reshape(PART, 1),
    )
    in_maps = []
    for r in range(NCORES):
        in_maps.append(dict(
            shared,
            ib=ib_t[r],
            dlT=dlT_t[r],
            cnts=cnt_t[r].reshape(1, NB * 8),
            maskrow=maskrow[r],
            cinvrow=cinvrow[r],
        ))
    meta = dict(NB=NB, KLO=KLO, KHI=KHI, KT=KT, CL=CL, CH=CH,
                lastcol=lastcol, counts=counts)
    return in_maps, meta


# --------------------------------------------------------------------------
# device program
# --------------------------------------------------------------------------

def build_program(meta, sim_local=False):
    NB, KLO, KHI, KT = meta["NB"], meta["KLO"], meta["KHI"], meta["KT"]
    CL, CH = meta["CL"], meta["CH"]
    IDXW = KT * 9 + 9
    nc = bacc.Bacc("TRN2", target_bir_lowering=False, debug=False,
                   num_devices=1 if sim_local else NCORES)

    def ein(name, shape, dt=F32):
        return nc.dram_tensor(name, list(shape), dt, kind="ExternalInput").ap()

    xh_d = ein("xh16T", [F_IN, NPAD], F16)
    W1_d = ein("W1h", [F_IN, HC], F16); B1_d = ein("B1h", [F_IN, 8], F16)
    W2_d = ein("W2h", [HC, HC], F16);   B2_d = ein("B2h", [HC, 8], F16)
    b1c_d = ein("b1colT", [PART, 2]); b2c_d = ein("b2colT", [PART, 2])
    gcol_d = ein("gcol", [PART, 2]); bcol_d = ein("bcol", [PART, 2])
    ib_d = ein("ib", [NB, PART, IDXW], I16)
    dlT_d = ein("dlT", [NB, 1, KT * PART], F16)
    iop_d = ein("iotap", [PART, 1])
    cnt_d = ein("cnts", [1, NB * 8], I32)
    mask_d = ein("maskrow", [1, NPC])
    cinv_d = ein("cinvrow", [1, NPC])

    omax_d = nc.dram_tensor("out_max", [4 * PART, NPC], F32, kind="ExternalOutput").ap()
    omean_d = nc.dram_tensor("out_mean", [4 * PART, NPC], F32, kind="ExternalOutput").ap()

    # internal DRAM
    T1b = nc.dram_tensor("T1b", [NPAD, RWH], F16).ap()
    T2b = nc.dram_tensor("T2b", [NPAD, RWH], F16).ap()
    al1pk = nc.dram_tensor("al1pk", [NPAD, 4], F32).ap()
    al2pk = nc.dram_tensor("al2pk", [NPAD, 4], F32).ap()
    x2T = nc.dram_tensor("x2T", [HC, NPC], F16).ap()
    ar_in = nc.dram_tensor("ar_in", [PART, 4], F32).ap()
    ar_out = nc.dram_tensor("ar_out", [PART, 4], F32, addr_space="Shared").ap()

    # AllGather chunks of the inter-layer feature (transposed, fp16)
    bpc = (NB + NAGC - 1) // NAGC
    blk_of_chunk = [list(range(c * bpc, min(NB, (c + 1) * bpc)))
                    for c in range(NAGC)]
    chunk_cols = []
    g1h_c, Tag_c = [], []
    for c in range(NAGC):
        c0 = blk_of_chunk[c][0] * PART
        c1 = min(NPC, (blk_of_chunk[c][-1] + 1) * PART)
        chunk_cols.append((c0, c1))
        g1h_c.append(nc.dram_tensor(f"g1h_{c}", [HC, c1 - c0], F16).ap())
        Tag_c.append(nc.dram_tensor(f"Tag_{c}", [NCORES * HC, c1 - c0], F16,
                                    addr_space="Shared").ap())

    rgroups = [list(range(NCORES))]

    class _PhaseStopE(Exception):
        pass

    with tile.TileContext(nc) as tc:
      try:
        # ---------- shared constant tiles ----------
        with tc.tile_pool(name="const", bufs=1) as cpool:
            ident = cpool.tile([PART, PART], F32)
            make_identity(nc, ident[:])
            iota_i = cpool.tile([PART, PART], mybir.dt.int32)
            nc.gpsimd.iota(iota_i[:], pattern=[[1, PART]], base=0,
                           channel_multiplier=0)
            iota_h = cpool.tile([PART, PART], F16)
            nc.vector.tensor_copy(out=iota_h[:], in_=iota_i[:])
            iota16 = cpool.tile([PART, 16], F16)
            nc.vector.tensor_copy(out=iota16[:], in_=iota_i[:, 0:16])

            cnt_sb = cpool.tile([1, NB * 8], I32)
            nc.sync.dma_start(out=cnt_sb[:], in_=cnt_d[:, :])
            b1cv = cpool.tile([PART, 2], F32)
            nc.sync.dma_start(out=b1cv[:], in_=b1c_d[:, :])
            b2cv = cpool.tile([PART, 2], F32)
            nc.sync.dma_start(out=b2cv[:], in_=b2c_d[:, :])
            iotap = cpool.tile([PART, 1], F32)
            nc.sync.dma_start(out=iotap[:], in_=iop_d[:, :])

            # ---------- dense L1 (replicated: full table on every core) ----
            _sc = nc.enter_named_scope("dense1", False)[0]
            with tc.tile_pool(name="d1", bufs=3) as dp, \
                 tc.tile_pool(name="d1w", bufs=1) as wp, \
                 tc.tile_pool(name="d1x", bufs=2) as xp, \
                 tc.tile_pool(name="d1ps", bufs=3, space="PSUM") as pp:
                W1_sb = wp.tile([F_IN, HC], F16)
                nc.sync.dma_start(out=W1_sb[:], in_=W1_d[:, :])
                B1_sb = wp.tile([F_IN, 8], F16)
                nc.sync.dma_start(out=B1_sb[:], in_=B1_d[:, :])
                XCH = 6272                      # x column chunk (49 blocks)
                x_sb = None
                row8 = al8 = None
                for b in range(NBLK):
                    if b % 49 == 0:
                        x_sb = xp.tile([F_IN, XCH], F16, tag="xsb")
                        x0 = b * PART
                        nc.sync.dma_start(out=x_sb[:, 0:min(XCH, NPAD - x0)],
                                          in_=xh_d[:, x0:min(x0 + XCH, NPAD)])
                    k = b % GRP
                    if k == 0:
                        row8 = dp.tile([PART, GRP, HC + 8], F16, tag="row8")
                        al8 = dp.tile([PART, GRP, 4], F32, tag="al8")
                    col = (b % 49) * PART
                    j = b % 2
                    if j == 0:
                        ps_h = pp.tile([PART, 2, HC], F32, tag="dpsh")
                        ps_al = pp.tile([PART, 2, 8], F32, tag="dpsal")
                    nc.tensor.matmul(ps_h[:, j, :], lhsT=x_sb[:, col:col + PART],
                                     rhs=W1_sb[:], start=True, stop=True)
                    nc.tensor.matmul(ps_al[:, j, :], lhsT=x_sb[:, col:col + PART],
                                     rhs=B1_sb[:], start=True, stop=True)
                    if j == 1 or b == NBLK - 1:
                        nj = j + 1
                        k0 = k - j
                        nc.scalar.activation(out=row8[:, k0:k0 + nj, 0:HC],
                                             in_=ps_h[:, 0:nj, :], func=ACTF.Copy)
                        nc.vector.tensor_copy(
                            out=row8[:, k0:k0 + nj, HC:HC + 8].bitcast(F32),
                            in_=ps_al[:, 0:nj, 0:4])
                        nc.vector.tensor_copy(out=al8[:, k0:k0 + nj, :],
                                              in_=ps_al[:, 0:nj, 4:8])
                    if k == GRP - 1 or b == NBLK - 1:
                        ng = k + 1
                        n0 = (b - k) * PART
                        nc.sync.dma_start(
                            out=T1b[n0:n0 + ng * PART, 0:HC + 8].rearrange(
                                "(k p) w -> p k w", p=PART),
                            in_=row8[:, 0:ng, :])
                        nc.sync.dma_start(
                            out=al1pk[n0:n0 + ng * PART, :].rearrange(
                                "(k p) c -> p k c", p=PART),
                            in_=al8[:, 0:ng, :])
            nc.leave_named_scope("dense1", _sc, False)

            # ---------- edge phase (shared for both layers) ----------
            def edge_phase(Tbl, alpk, bias_cv, outT, relu, scope, ag=False):
                """outT: None for L1 (writes g1h chunks), else x2T."""
                _es = nc.enter_named_scope(scope, False)[0]
                alview = alpk[:, :].rearrange("(r j) c -> r (j c)", j=16)
                with tc.tile_pool(name="eidx", bufs=2) as ip, \
                     tc.tile_pool(name="eg", bufs=2) as gp, \
                     tc.tile_pool(name="ew", bufs=2) as wp2, \
                     tc.tile_pool(name="eps", bufs=2, space="PSUM") as ep, \
                     tc.tile_pool(name="etps", bufs=2, space="PSUM") as tps:
                    # pre-zero both gather buffers: -1-skipped slots must hold
                    # finite floats (uninitialized SBUF could be NaN -> NaN*0
                    # = NaN in PSUM)
                    for _z in range(2):
                        zt = gp.tile([PART, KT, RWH], F16, tag="gall")
                        nc.vector.memset(zt[:], 0.0)

                    cnt_regs = [nc.gpsimd.alloc_register(f"cnt_{scope}_{i}")
                                for i in range(4)]
                    reg_rr = [0]

                    def gather(gtile, src_ap, ixtile, ktot, elem, cnt_base):
                        for ci, c0 in enumerate(range(0, ktot, CHUNK)):
                            cw = min(CHUNK, ktot - c0)
                            reg = cnt_regs[reg_rr[0] % 4]
                            reg_rr[0] += 1
                            nc.gpsimd.reg_load(
                                reg, cnt_sb[0:1, cnt_base + ci:cnt_base + ci + 1])
                            nc.gpsimd.dma_gather(
                                out_ap=gtile[:, c0:c0 + cw, :],
                                in_ap=src_ap, idxs_ap=ixtile[:, c0 * 8:(c0 + cw) * 8],
                                num_idxs=cw * PART, num_idxs_reg=reg,
                                elem_size=elem)

                    for b in range(NB):
                        mb = min(PART, NPC - b * PART)
                        ib = ip.tile([PART, IDXW], I16, tag="ib")
                        nc.sync.dma_start(out=ib[:], in_=ib_d[b, :, :])
                        il = ib[:, 0:KLO * 8]
                        ih = ib[:, KLO * 8:KT * 8]
                        dl = ib[:, KT * 8:KT * 9].bitcast(F16)
                        dmn = ib[:, KT * 9:KT * 9 + 1].bitcast(F16)
                        ixn = ib[:, KT * 9 + 1:KT * 9 + 9]
                        dlF = ip.tile([PART, KT * PART], F16, tag="dlF")
                        nc.sync.dma_start(
                            out=dlF[:],
                            in_=dlT_d[b, 0:1, :].to_broadcast([PART, KT * PART]))

                        gall = gp.tile([PART, KT, RWH], F16, tag="gall")
                        gather(gall[:, 0:KLO, :], Tbl[0:SPLIT, :], il, KLO, RWH, b * 8)
                        gather(gall[:, KLO:KT, :], Tbl[SPLIT:NPAD, :], ih, KHI, RWH,
                               b * 8 + CL)
                        # one 256-B packed al_dst row per OWN dst node (128
                        # descriptors instead of one per edge)
                        gan = gp.tile([PART, 1, 64], F32, tag="gan")
                        gather(gan, alview, ixn, 1, 64, b * 8 + CL + CH)

                        # selector matrix S01[e, kt, d] = (dl == d), fp16
                        S01 = wp2.tile([PART, KT, PART], F16, tag="S01")
                        nc.vector.tensor_tensor(
                            out=S01[:],
                            in0=dl[:].unsqueeze(-1).to_broadcast([PART, KT, PART]),
                            in1=iota_h[:].unsqueeze(1).to_broadcast([PART, KT, PART]),
                            op=ALU.is_equal)

                        # al_dst per own node: one-hot over the 16-node pack
                        ohn = wp2.tile([PART, 16], F32, tag="ohn")
                        nc.vector.tensor_tensor(
                            out=ohn[:],
                            in0=dmn[:].to_broadcast([PART, 16]),
                            in1=iota16[:], op=ALU.is_equal)
                        adn = wp2.tile([PART, 4, 16], F32, tag="adn")
                        nc.vector.tensor_tensor(
                            out=adn[:],
                            in0=gan[:, 0, :].rearrange("p (j h) -> p h j", j=16),
                            in1=ohn[:].unsqueeze(1).to_broadcast([PART, 4, 16]),
                            op=ALU.mult)
                        adstb32 = wp2.tile([PART, 4], F32, tag="adstb32")
                        nc.vector.tensor_reduce(
                            out=adstb32[:].unsqueeze(-1), in_=adn[:],
                            axis=mybir.AxisListType.X, op=ALU.add)
                        adstb = wp2.tile([PART, 4], F16, tag="adstb")
                        nc.vector.tensor_copy(out=adstb[:], in_=adstb32[:])
                        # transposed selector S01T[d, i] = (dl_i == d); PE
                        # looks up al_dst per edge: adps[e,h] = sum_d S01T*adst
                        S01T = wp2.tile([PART, KT * PART], F16, tag="S01T")
                        nc.vector.scalar_tensor_tensor(
                            out=S01T[:], in0=dlF[:], scalar=iotap[:, 0:1],
                            in1=dlF[:], op0=ALU.is_equal, op1=ALU.bypass)
                        adps = ep.tile([PART, KT * 4], F32, tag="adps")
                        for e in range(KT):
                            nc.tensor.matmul(adps[:, e * 4:(e + 1) * 4],
                                             lhsT=S01T[:, e * PART:(e + 1) * PART],
                                             rhs=adstb[:], start=True, stop=True)
                        Z = wp2.tile([PART, KT, 4], F32, tag="Z")
                        nc.vector.tensor_tensor(
                            out=Z[:],
                            in0=adps[:].rearrange("p (k h) -> p k h", h=4),
                            in1=gall[:, :, HC:HC + 8].bitcast(F32), op=ALU.add)
                        # leaky-relu (one fused op), clamp, exp -> fp16
                        nc.vector.scalar_tensor_tensor(
                            out=Z[:], in0=Z[:], scalar=NEG_SLOPE, in1=Z[:],
                            op0=ALU.mult, op1=ALU.max)
                        nc.vector.tensor_scalar_min(out=Z[:], in0=Z[:], scalar1=ZCLAMP)
                        EXh = wp2.tile([PART, KT, 4], F16, tag="EXh")
                        nc.scalar.activation(out=EXh[:], in_=Z[:], func=ACTF.Exp)

                        # Hp = [ex-weighted h | ex] (fp16, (c,h)-interleaved)
                        Hp = wp2.tile([PART, KT, 260], F16, tag="Hp")
                        nc.vector.tensor_tensor(
                            out=Hp[:, :, 0:HC].rearrange("p k (c h) -> p k c h", h=H),
                            in0=gall[:, :, 0:HC].rearrange("p k (c h) -> p k c h", h=H),
                            in1=EXh[:].unsqueeze(2).to_broadcast([PART, KT, C0, H]),
                            op=ALU.mult)
                        nc.vector.tensor_copy(out=Hp[:, :, HC:HC + 4], in_=EXh[:])

                        acc = ep.tile([PART, 260], F32, tag="acc")
                        for e in range(KT):
                            nc.tensor.matmul(acc[:], lhsT=S01[:, e, :], rhs=Hp[:, e, :],
                                             start=(e == 0), stop=(e == KT - 1))

                        dn = wp2.tile([PART, 4], F32, tag="dn")
                        nc.vector.tensor_scalar_add(out=dn[:], in0=acc[:, HC:HC + 4],
                                                    scalar1=1e-16)
                        rec = wp2.tile([PART, 4], F32, tag="rec")
                        nc.vector.reciprocal(out=rec[:], in_=dn[:])
                        ob = wp2.tile([PART, HC], F32, tag="ob")
                        nc.vector.tensor_tensor(
                            out=ob[:].rearrange("p (c h) -> p c h", h=H),
                            in0=acc[:, 0:HC].rearrange("p (c h) -> p c h", h=H),
                            in1=rec[:].unsqueeze(1).to_broadcast([PART, C0, H]),
                            op=ALU.mult)
                        for ct in range(2):
                            tp = tps.tile([PART, PART], F32, tag="ttp")
                            nc.tensor.transpose(out=tp[:], in_=ob[:, ct * PART:(ct + 1) * PART],
                                                identity=ident[:])
                            tsh = wp2.tile([PART, PART], F16, tag="tsh")
                            nc.scalar.activation(out=tsh[:], in_=tp[:],
                                                 func=ACTF.Relu if relu else ACTF.Identity,
                                                 bias=bias_cv[:, ct:ct + 1])
                            if outT is None:
                                ci = min(b // bpc, NAGC - 1)
                                cc0 = chunk_cols[ci][0]
                                nc.sync.dma_start(
                                    out=g1h_c[ci][ct * PART:(ct + 1) * PART,
                                                  b * PART - cc0:b * PART - cc0 + mb],
                                    in_=tsh[:, 0:mb])
                            else:
                                nc.sync.dma_start(
                                    out=outT[ct * PART:(ct + 1) * PART,
                                             b * PART:b * PART + mb],
                                    in_=tsh[:, 0:mb])
                        if ag and not NO_COLL:
                            ci = min(b // bpc, NAGC - 1)
                            if b == blk_of_chunk[ci][-1]:
                                if sim_local:
                                    for r_ in range(NCORES):
                                        nc.sync.dma_start(
                                            out=Tag_c[ci][r_ * HC:(r_ + 1) * HC, :],
                                            in_=g1h_c[ci][:, :])
                                else:
                                    nc.gpsimd.collective_compute(
                                        "AllGather", ALU.bypass,
                                        replica_groups=rgroups,
                                        ins=[g1h_c[ci][:, :]],
                                        outs=[Tag_c[ci][:, :]])
                nc.leave_named_scope(scope, _es, False)

            if PHASES >= 2:
                edge_phase(T1b, al1pk, b1cv, None, relu=False, scope="edge1",
                           ag=True)

            # ---------- BN stats + AllReduce ----------
            if PHASES < 3:
                raise _PhaseStopE
            _sc = nc.enter_named_scope("bnstat", False)[0]
            with tc.tile_pool(name="st", bufs=1) as sp, \
                 tc.tile_pool(name="stw", bufs=1) as sw:
                stats = sw.tile([PART, 4], F32)
                for ct in range(2):
                    gt = sp.tile([PART, NPC], F16, tag="gt")
                    for ci in range(NAGC):
                        cc0, cc1 = chunk_cols[ci]
                        nc.sync.dma_start(
                            out=gt[:, cc0:cc1],
                            in_=g1h_c[ci][ct * PART:(ct + 1) * PART, :])
                    nc.vector.tensor_reduce(out=stats[:, ct:ct + 1], in_=gt[:],
                                            axis=mybir.AxisListType.X, op=ALU.add)
                    sq = sp.tile([PART, NPC], F32, tag="sq")
                    nc.scalar.activation(out=sq[:], in_=gt[:], func=ACTF.Square)
                    nc.vector.tensor_reduce(out=stats[:, 2 + ct:3 + ct], in_=sq[:],
                                            axis=mybir.AxisListType.X, op=ALU.add)
                nc.sync.dma_start(out=ar_in[:, :], in_=stats[:])
            nc.leave_named_scope("bnstat", _sc, False)

            _sc = nc.enter_named_scope("ar", False)[0]
            if sim_local:
                nc.sync.dma_start(out=ar_out[:, :], in_=ar_in[:, :])
            else:
                nc.gpsimd.collective_compute(
                    "AllReduce", ALU.add, replica_groups=rgroups,
                    ins=[ar_in[:, :]], outs=[ar_out[:, :]])
            nc.leave_named_scope("ar", _sc, False)

            with tc.tile_pool(name="bnw", bufs=1) as bw:
                ar_sb = bw.tile([PART, 4], F32)
                nc.sync.dma_start(out=ar_sb[:], in_=ar_out[:, :])
                mean = bw.tile([PART, 2], F32)
                nc.vector.tensor_scalar_mul(out=mean[:], in0=ar_sb[:, 0:2], scalar1=1.0 / N)
                msq = bw.tile([PART, 2], F32)
                nc.vector.tensor_scalar_mul(out=msq[:], in0=ar_sb[:, 2:4], scalar1=1.0 / N)
                var = bw.tile([PART, 2], F32)
                nc.vector.tensor_tensor(out=var[:], in0=mean[:], in1=mean[:], op=ALU.mult)
                nc.vector.tensor_tensor(out=var[:], in0=msq[:], in1=var[:], op=ALU.subtract)
                nc.vector.tensor_scalar_add(out=var[:], in0=var[:], scalar1=BN_EPS)
                sd = bw.tile([PART, 2], F32)
                nc.scalar.activation(out=sd[:], in_=var[:], func=ACTF.Sqrt)
                rinv = bw.tile([PART, 2], F32)
                nc.vector.reciprocal(out=rinv[:], in_=sd[:])
                gc = bw.tile([PART, 2], F32)
                nc.sync.dma_start(out=gc[:], in_=gcol_d[:, :])
                bc = bw.tile([PART, 2], F32)
                nc.sync.dma_start(out=bc[:], in_=bcol_d[:, :])
                scale_c = bw.tile([PART, 2], F32)
                nc.vector.tensor_tensor(out=scale_c[:], in0=gc[:], in1=rinv[:], op=ALU.mult)
                shift_c = bw.tile([PART, 2], F32)
                nc.vector.tensor_tensor(out=shift_c[:], in0=mean[:], in1=scale_c[:], op=ALU.mult)
                nc.vector.tensor_tensor(out=shift_c[:], in0=bc[:], in1=shift_c[:], op=ALU.subtract)

                # ---------- dense L2 (replicated, from AllGathered x1) -----
                if PHASES < 4:
                    raise _PhaseStopE
                _sc = nc.enter_named_scope("dense2", False)[0]
                with tc.tile_pool(name="d2", bufs=3) as dp2, \
                     tc.tile_pool(name="d2w", bufs=1) as wp3, \
                     tc.tile_pool(name="d2x", bufs=2) as xp2, \
                     tc.tile_pool(name="d2ps", bufs=3, space="PSUM") as pp2:
                    W2_sb = [wp3.tile([PART, HC], F16, tag=f"w2_{kt}", name=f"w2_{kt}")
                             for kt in range(2)]
                    B2_sb = [wp3.tile([PART, 8], F16, tag=f"b2_{kt}", name=f"b2_{kt}")
                             for kt in range(2)]
                    for kt in range(2):
                        nc.sync.dma_start(out=W2_sb[kt][:],
                                          in_=W2_d[kt * PART:(kt + 1) * PART, :])
                        nc.sync.dma_start(out=B2_sb[kt][:],
                                          in_=B2_d[kt * PART:(kt + 1) * PART, :])
                    for r_ in range(NCORES):
                        for ci in range(NAGC):
                            cc0, cc1 = chunk_cols[ci]
                            w = cc1 - cc0
                            xs = []
                            for kt in range(2):
                                gl = xp2.tile([PART, bpc * PART], F16, tag=f"gl{kt}",
                                              name=f"gl{kt}")
                                nc.sync.dma_start(
                                    out=gl[:, 0:w],
                                    in_=Tag_c[ci][r_ * HC + kt * PART:r_ * HC + (kt + 1) * PART, :])
                                x1s = xp2.tile([PART, bpc * PART], F16, tag=f"x1s{kt}",
                                               name=f"x1s{kt}")
                                nc.scalar.activation(out=x1s[:, 0:w], in_=gl[:, 0:w],
                                                     func=ACTF.Relu,
                                                     bias=shift_c[:, kt:kt + 1],
                                                     scale=scale_c[:, kt:kt + 1])
                                xs.append(x1s)
                            nblk2 = (w + PART - 1) // PART
                            row8 = al8 = None
                            for bl in range(nblk2):
                                lb = bl * PART
                                mb2 = min(PART, w - lb)
                                k = bl % GRP
                                if k == 0:
                                    row8 = dp2.tile([PART, GRP, HC + 8], F16, tag="d2row8")
                                    al8 = dp2.tile([PART, GRP, 4], F32, tag="d2al8")
                                j = bl % 2
                                if j == 0:
                                    ps_h = pp2.tile([PART, 2, HC], F32, tag="d2psh")
                                    ps_al = pp2.tile([PART, 2, 8], F32, tag="d2psal")
                                for kt in range(2):
                                    nc.tensor.matmul(ps_h[0:mb2, j, :],
                                                     lhsT=xs[kt][:, lb:lb + mb2],
                                                     rhs=W2_sb[kt][:],
                                                     start=(kt == 0), stop=(kt == 1))
                                for kt in range(2):
                                    nc.tensor.matmul(ps_al[0:mb2, j, :],
                                                     lhsT=xs[kt][:, lb:lb + mb2],
                                                     rhs=B2_sb[kt][:],
                                                     start=(kt == 0), stop=(kt == 1))
                                if j == 1 or bl == nblk2 - 1:
                                    nj = j + 1
                                    k0 = k - j
                                    nc.scalar.activation(out=row8[:, k0:k0 + nj, 0:HC],
                                                         in_=ps_h[:, 0:nj, :],
                                                         func=ACTF.Copy)
                                    nc.vector.tensor_copy(
                                        out=row8[:, k0:k0 + nj, HC:HC + 8].bitcast(F32),
                                        in_=ps_al[:, 0:nj, 0:4])
                                    nc.vector.tensor_copy(out=al8[:, k0:k0 + nj, :],
                                                          in_=ps_al[:, 0:nj, 4:8])
                                if k == GRP - 1 or bl == nblk2 - 1:
                                    # batch-flush the full blocks; a ragged
                                    # tail block (mb2 < PART) is written solo
                                    ng = k + (1 if mb2 == PART else 0)
                                    n0 = r_ * NPC + cc0 + (bl - k) * PART
                                    if ng > 0:
                                        nc.sync.dma_start(
                                            out=T2b[n0:n0 + ng * PART, 0:HC + 8].rearrange(
                                                "(k p) w -> p k w", p=PART),
                                            in_=row8[:, 0:ng, :])
                                        nc.sync.dma_start(
                                            out=al2pk[n0:n0 + ng * PART, :].rearrange(
                                                "(k p) c -> p k c", p=PART),
                                            in_=al8[:, 0:ng, :])
                                    if mb2 < PART:
                                        nr = n0 + k * PART
                                        nc.sync.dma_start(
                                            out=T2b[nr:nr + mb2, 0:HC + 8],
                                            in_=row8[0:mb2, k, :])
                                        nc.sync.dma_start(
                                            out=al2pk[nr:nr + mb2, :],
                                            in_=al8[0:mb2, k, :])
                nc.leave_named_scope("dense2", _sc, False)

                if PHASES < 5:
                    raise _PhaseStopE
                edge_phase(T2b, al2pk, b2cv, x2T, relu=True, scope="edge2")

                # ---------- pooling ----------
                if PHASES < 6:
                    raise _PhaseStopE
                _sc = nc.enter_named_scope("pool", False)[0]
                with tc.tile_pool(name="pl", bufs=1) as pl:
                    mk = pl.tile([PART, NPC], F32, tag="mk")
                    nc.sync.dma_start(out=mk[:], in_=mask_d[0:1, :].to_broadcast([PART, NPC]))
                    cv = pl.tile([PART, NPC], F32, tag="cv")
                    nc.sync.dma_start(out=cv[:], in_=cinv_d[0:1, :].to_broadcast([PART, NPC]))
                    for ct in range(4):
                        xt = pl.tile([PART, NPC], F32, tag="xt")
                        if ct < 2:
                            gld = pl.tile([PART, NPC], F16, tag="gld")
                            for ci in range(NAGC):
                                cc0, cc1 = chunk_cols[ci]
                                nc.sync.dma_start(
                                    out=gld[:, cc0:cc1],
                                    in_=g1h_c[ci][ct * PART:(ct + 1) * PART, :])
                            nc.scalar.activation(out=xt[:], in_=gld[:], func=ACTF.Relu,
                                                 bias=shift_c[:, ct:ct + 1],
                                                 scale=scale_c[:, ct:ct + 1])
                        else:
                            x2l = pl.tile([PART, NPC], F16, tag="x2l")
                            nc.sync.dma_start(out=x2l[:],
                                              in_=x2T[(ct - 2) * PART:(ct - 1) * PART, :])
                            nc.scalar.activation(out=xt[:], in_=x2l[:], func=ACTF.Copy)
                        sm = pl.tile([PART, NPC], F32, tag="sm")
                        nc.vector.tensor_tensor_scan(out=sm[:], data0=mk[:], data1=xt[:],
                                                     initial=0.0, op0=ALU.mult, op1=ALU.max)
                        nc.sync.dma_start(out=omax_d[ct * PART:(ct + 1) * PART, :], in_=sm[:])
                        ss = pl.tile([PART, NPC], F32, tag="ss")
                        nc.vector.tensor_tensor_scan(out=ss[:], data0=mk[:], data1=xt[:],
                                                     initial=0.0, op0=ALU.mult, op1=ALU.add)
                        nc.vector.tensor_tensor(out=ss[:], in0=ss[:], in1=cv[:], op=ALU.mult)
                        nc.sync.dma_start(out=omean_d[ct * PART:(ct + 1) * PART, :], in_=ss[:])
                nc.leave_named_scope("pool", _sc, False)

      except _PhaseStopE:
        pass

    nc.compile()
    return nc


# --------------------------------------------------------------------------
# host-side combine
# --------------------------------------------------------------------------

def postprocess(results, meta):
    lastcol = meta["lastcol"]
    mean = np.zeros((G, 2 * HC), np.float32)
    mx = np.zeros((G, 2 * HC), np.float32)
    for r in range(NCORES):
        om = results[r]["out_mean"]   # [512, NPC], dev channel order
        ox = results[r]["out_max"]
        for g_, col in lastcol[r].items():
            mean[g_] += om[:, col]
            mx[g_] = np.maximum(mx[g_], ox[:, col])
    # un-permute dev channel order back to torch order
    dev2orig = np.concatenate([PERM, HC + PERM])
    mean_o = np.empty_like(mean); mx_o = np.empty_like(mx)
    mean_o[:, dev2orig] = mean
    mx_o[:, dev2orig] = mx
    return np.concatenate([mean_o, mx_o], axis=1).astype(np.float32)


_CACHE = {}


def kernel(**inputs):
    in_maps, meta = preprocess(**inputs)
    key = (meta["NB"], meta["KLO"], meta["KHI"])
    if key not in _CACHE:
        _CACHE[key] = build_program(meta)
    nc = _CACHE[key]
    res = bass_utils.run_bass_kernel_spmd(nc, in_maps, core_ids=list(range(NCORES)))
    return postprocess(res.results, meta)


# revision 42
# speedup vs baseline: 8.9912x; 1.0194x over previous
"""Self-contained Trainium2 Bass kernel for a 2-layer GAT + BatchNorm + graph pooling.

Contract: kernel(**inputs) takes the FULL (unsharded) inputs and returns the
FULL [G, 1024] float32 output.

v6 design: replicated dense layers, fp16 gather tables, tiny overlapped
collectives, channel-interleaved layout for fast DVE broadcasts.

  - Channels are stored (c, h)-interleaved (dev channel c*4+h = torch channel
    h*64+c, permuted host-side in the weights and un-permuted in postprocess)
    so every per-head broadcast multiply has a packed innermost dim of 4 —
    DVE runs these at 16-bit double rate instead of broadcast-stride-0 rate.
  - dense L1 is REPLICATED: every core computes the full table
    T1b[n] = [h(256 fp16) | al_src(4 f32 riding as 8 fp16 slots) | pad]
    (768 B rows) from x; writes are batched 8 blocks per DMA (the HWDGE
    ~600 ns fixed cost per dma_start dominated v2's dense phases).
  - "al_dst" logits live in a plain [N, 4] f32 table whose gather view is
    [N/16, 64] (256 B rows = the dma_gather minimum).  Each edge block
    gathers ONE packed row per own dst NODE (128 descriptors) instead of one
    per edge (~2500): the per-node value is extracted with a one-hot-over-16
    dot, then a transposed 0/1 selector (built in one fused DVE op from a
    broadcast dl row) and KT tiny PE matmuls broadcast it back to the edge
    partitions.  Gather descriptors per block drop ~46%, which is the real
    HW bottleneck (random 256-768 B reads cost far more per descriptor than
    the bus-rate model suggests).
  - edge phase (per 128-dst-node block): one combined index/metadata DMA,
    dma_gather rows by src (lo/hi split for int16; <=1024 idxs per
    instruction — 2048 hangs the HW, verified), -1-padded index streams skip
    pad transfers (per-core valid counts are reg_load-ed from SBUF),
    softmax-weighted segment sum via 0/1 fp16 selector-matrix matmuls
    accumulating [out | denom] in PSUM.  Logits stay f32 (exp via ACT,
    clamped at +8 so pad garbage cannot overflow fp16).  The per-node
    epilogue folds bias+relu into the post-transpose ACT copy.
  - between layers only x1's pre-BN value g1 (256ch fp16, transposed) is
    AllGathered — 3.2 MB per rank in 4 column-chunks issued as edge L1
    drains, overlapping wire time with edge compute.  BN stats go through a
    [128,4] AllReduce; the BN affine + relu is fused into dense L2's
    activation load.  dense L2 is replicated from the gathered chunks.
  - pooling: per-channel-tile segmented running sum & max along the node
    axis (tensor_tensor_scan); host reads each graph's last column and
    combines the <=2 per-graph partials from adjacent cores.
"""

import numpy as np

import concourse.bass as bass
import concourse.bacc as bacc
import concourse.tile as tile
from concourse import mybir
from concourse import bass_utils
from concourse.masks import make_identity

F32 = mybir.dt.float32
F16 = mybir.dt.float16
I16 = mybir.dt.int16
I32 = mybir.dt.int32
ALU = mybir.AluOpType
ACTF = mybir.ActivationFunctionType

# problem constants (hardcoded per the harness contract)
N, F_IN, C0, C1, H, E, G = 50000, 128, 64, 64, 4, 800000, 256
HC = H * C0            # 256
NEG_SLOPE = 0.2
BN_EPS = 1e-5
NCORES = 8
NPC = N // NCORES      # nodes per core (6250)
SPLIT = 32768          # dma_gather int16 index limit -> split gather table
RWH = 384              # fp16 table row width (768 B): h(256) + al_src(8) + pad
PART = 128
NPAD = 50048           # N rounded to 128 blocks (391 blocks)
NBLK = NPAD // PART    # 391 dense blocks
CHUNK = 8              # 128-idx groups per dma_gather (1024 idx HW limit)
ZCLAMP = 8.0           # logit clamp (real logits ~ +-6); keeps exp fp16-finite
NAGC = 4               # AllGather column-chunks for the inter-layer feature
NO_COLL = False        # bisection aid: skip collectives entirely
GRP = 8                # dense blocks batched per table-row DMA

# dev channel k = c*4+h  <->  torch channel h*64+c
PERM = np.array([(k % H) * C0 + k // H for k in range(HC)], np.int64)

PHASES = 6             # build phases 1..6 (bisection aid)


# --------------------------------------------------------------------------
# host-side preprocessing
# --------------------------------------------------------------------------

def _pack16(stream_i16, ncols):
    """dma_gather index layout: position i -> [i%16, i//16], replicated to
    partition groups 16k+p for the 8 Q7 cores."""
    base = stream_i16.reshape(ncols, 16).T          # [16, ncols]
    return np.tile(base, (8, 1)).astype(np.int16)   # [128, ncols]


def _pad_stream(vals, nslots):
    """Pad an index stream to nslots with -1 (skipped by dma_gather) and
    return (idx_i16, per-1024-chunk valid counts).  A chunk with zero valid
    indices gets one dummy index 0 (count 1): the HW needs at least one
    non-negative index per instruction."""
    n = len(vals)
    out = np.full(nslots, -1, np.int16)
    out[:n] = vals
    counts = []
    for c0 in range(0, nslots, CHUNK * PART):
        span = min(CHUNK * PART, nslots - c0)
        cnt = min(max(n - c0, 0), span)
        if cnt == 0:
            out[c0] = 0
            cnt = 1
        counts.append(cnt)
    return out, counts


def preprocess(x, edge_index, batch,
               W1, att_src1, att_dst1, b1, gamma, beta,
               W2, att_src2, att_dst2, b2):
    x = np.asarray(x, np.float32)
    edge_index = np.asarray(edge_index)
    batch = np.asarray(batch).astype(np.int64)
    W1 = np.asarray(W1, np.float32); W2 = np.asarray(W2, np.float32)

    src = np.concatenate([edge_index[0], np.arange(N, dtype=np.int64)])
    dst = np.concatenate([edge_index[1], np.arange(N, dtype=np.int64)])

    NB = (NPC + PART - 1) // PART                      # dst blocks per core

    # ---- per-core edge streams ----
    blocks = []
    nlo_max = nhi_max = 0
    for r in range(NCORES):
        m = (dst >= r * NPC) & (dst < (r + 1) * NPC)
        s_r = src[m]; d_r = dst[m]
        dloc = d_r - r * NPC
        order = np.argsort(dloc, kind="stable")
        s_r = s_r[order]; d_r = d_r[order]; dloc = dloc[order]
        blk = dloc // PART
        core_blocks = []
        for b in range(NB):
            bm = blk == b
            sb_ = s_r[bm]; db_ = d_r[bm]
            lo_m = sb_ < SPLIT
            # sort each gather stream by src: ascending table addresses give
            # HBM row-buffer locality (edge order within a dst block is free
            # — the selector matrix handles any order)
            slo = sb_[lo_m]; dlo = db_[lo_m]
            shi = sb_[~lo_m]; dhi = db_[~lo_m]
            o = np.argsort(slo, kind="stable"); slo = slo[o]; dlo = dlo[o]
            o = np.argsort(shi, kind="stable"); shi = shi[o]; dhi = dhi[o]
            core_blocks.append((slo, shi - SPLIT, dlo, dhi))
            nlo_max = max(nlo_max, len(slo))
            nhi_max = max(nhi_max, len(shi))
        blocks.append(core_blocks)

    KLO = max(1, (nlo_max + PART - 1) // PART)
    KHI = max(1, (nhi_max + PART - 1) // PART)
    KT = KLO + KHI
    CL = (KLO + CHUNK - 1) // CHUNK
    CH = (KHI + CHUNK - 1) // CHUNK
    IDXW = KT * 9 + 9    # [il|ih] (KT*8) + dl (KT) + dmn (1) + idxn (8), i16

    ib_t = np.zeros((NCORES, NB, PART, IDXW), np.int16)
    dlT_t = np.zeros((NCORES, NB, 1, KT * PART), np.float16)
    cnt_t = np.zeros((NCORES, NB, 8), np.int32)
    for r in range(NCORES):
        for b in range(NB):
            lo_src, hi_src, abs_lo, abs_hi = blocks[r][b]
            ls, c_lo = _pad_stream(lo_src, KLO * PART)
            hs, c_hi = _pad_stream(hi_src, KHI * PART)
            ib_t[r, b, :, 0:KLO * 8] = _pack16(ls, KLO * 8)
            ib_t[r, b, :, KLO * 8:KT * 8] = _pack16(hs, KHI * 8)
            cnt_t[r, b, :CL] = c_lo
            cnt_t[r, b, CL:CL + CH] = c_hi
            dl = np.full(KT * PART, 999.0, np.float32)
            dl[:len(abs_lo)] = (abs_lo - r * NPC) % PART
            dl[KLO * PART:KLO * PART + len(abs_hi)] = (abs_hi - r * NPC) % PART
            ib_t[r, b, :, KT * 8:KT * 9] = \
                dl.reshape(KT, PART).T.astype(np.float16).view(np.int16)
            dlT_t[r, b, 0] = dl.astype(np.float16)
            # per-NODE al_dst gather: one 256-B packed row per own dst node
            mb = min(PART, NPC - b * PART)
            nodes = r * NPC + b * PART + np.arange(mb)
            ixn, c_n = _pad_stream(nodes // 16, PART)
            cnt_t[r, b, CL + CH] = c_n[0]
            dmn = np.zeros(PART, np.float32)
            dmn[:mb] = nodes % 16
            ib_t[r, b, :, KT * 9:KT * 9 + 1] = \
                dmn.astype(np.float16).view(np.int16).reshape(PART, 1)
            ib_t[r, b, :, KT * 9 + 1:KT * 9 + 9] = _pack16(ixn, 8)

    # ---- batch-derived pooling metadata ----
    counts = np.bincount(batch, minlength=G).astype(np.float64)
    maskrow = np.zeros((NCORES, 1, NPC), np.float32)
    cinvrow = np.zeros((NCORES, 1, NPC), np.float32)
    lastcol = [dict() for _ in range(NCORES)]
    for r in range(NCORES):
        bseg = batch[r * NPC:(r + 1) * NPC]
        same = np.ones(NPC, np.float32)
        same[0] = 0.0
        same[1:] = (bseg[1:] == bseg[:-1]).astype(np.float32)
        maskrow[r, 0] = same
        cinvrow[r, 0] = (1.0 / np.maximum(counts[bseg], 1.0)).astype(np.float32)
        gids, last_idx = np.unique(bseg[::-1], return_index=True)
        for g_, li in zip(gids, last_idx):
            lastcol[r][int(g_)] = NPC - 1 - int(li)

    # ---- weights (replicated; channel-permuted to dev order) ----
    def bmat(W, a_s, a_d, fin):
        Wr = W.reshape(fin, H, C0)
        bs = np.einsum("khc,hc->kh", Wr, np.asarray(a_s, np.float32))
        bd = np.einsum("khc,hc->kh", Wr, np.asarray(a_d, np.float32))
        return np.concatenate([bs, bd], axis=1).astype(np.float16)  # [fin, 8]

    xh = np.zeros((F_IN, NPAD), np.float16)
    xh[:, :N] = x.T.astype(np.float16)

    W1p = W1[:, PERM]
    W2p = W2[PERM][:, PERM]
    b1p = np.asarray(b1, np.float32)[PERM]
    b2p = np.asarray(b2, np.float32)[PERM]

    shared = dict(
        xh16T=xh,
        W1h=W1p.astype(np.float16), B1h=bmat(W1, att_src1, att_dst1, F_IN),
        W2h=W2p.astype(np.float16),
        B2h=bmat(W2, att_src2, att_dst2, HC)[PERM, :],
        b1colT=b1p.reshape(2, PART).T.copy(),
        b2colT=b2p.reshape(2, PART).T.copy(),
        gcol=np.asarray(gamma, np.float32)[PERM].reshape(2, PART).T.copy(),
        bcol=np.asarray(beta, np.float32)[PERM].reshape(2, PART).T.copy(),
        iotap=np.arange(PART, dtype=np.float32).reshape(PART, 1),
    )
    in_maps = []
    for r in range(NCORES):
        in_maps.append(dict(
            shared,
            ib=ib_t[r],
            dlT=dlT_t[r],
            cnts=cnt_t[r].reshape(1, NB * 8),
            maskrow=maskrow[r],
            cinvrow=cinvrow[r],
        ))
    meta = dict(NB=NB, KLO=KLO, KHI=KHI, KT=KT, CL=CL, CH=CH,
                lastcol=lastcol, counts=counts)
    return in_maps, meta


# --------------------------------------------------------------------------
# device program
# --------------------------------------------------------------------------

def build_program(meta, sim_local=False):
    NB, KLO, KHI, KT = meta["NB"], meta["KLO"], meta["KHI"], meta["KT"]
    CL, CH = meta["CL"], meta["CH"]
    IDXW = KT * 9 + 9
    nc = bacc.Bacc("TRN2", target_bir_lowering=False, debug=False,
                   num_devices=1 if sim_local else NCORES)

    def ein(name, shape, dt=F32):
        return nc.dram_tensor(name, list(shape), dt, kind="ExternalInput").ap()

    xh_d = ein("xh16T", [F_IN, NPAD], F16)
    W1_d = ein("W1h", [F_IN, HC], F16); B1_d = ein("B1h", [F_IN, 8], F16)
    W2_d = ein("W2h", [HC, HC], F16);   B2_d = ein("B2h", [HC, 8], F16)
    b1c_d = ein("b1colT", [PART, 2]); b2c_d = ein("b2colT", [PART, 2])
    gcol_d = ein("gcol", [PART, 2]); bcol_d = ein("bcol", [PART, 2])
    ib_d = ein("ib", [NB, PART, IDXW], I16)
    dlT_d = ein("dlT", [NB, 1, KT * PART], F16)
    iop_d = ein("iotap", [PART, 1])
    cnt_d = ein("cnts", [1, NB * 8], I32)
    mask_d = ein("maskrow", [1, NPC])
    cinv_d = ein("cinvrow", [1, NPC])

    omax_d = nc.dram_tensor("out_max", [4 * PART, NPC], F32, kind="ExternalOutput").ap()
    omean_d = nc.dram_tensor("out_mean", [4 * PART, NPC], F32, kind="ExternalOutput").ap()

    # internal DRAM
    T1b = nc.dram_tensor("T1b", [NPAD, RWH], F16).ap()
    T2b = nc.dram_tensor("T2b", [NPAD, RWH], F16).ap()
    al1pk = nc.dram_tensor("al1pk", [NPAD, 4], F32).ap()
    al2pk = nc.dram_tensor("al2pk", [NPAD, 4], F32).ap()
    x2T = nc.dram_tensor("x2T", [HC, NPC], F16).ap()
    ar_in = nc.dram_tensor("ar_in", [PART, 4], F32).ap()
    ar_out = nc.dram_tensor("ar_out", [PART, 4], F32, addr_space="Shared").ap()

    # AllGather chunks of the inter-layer feature (transposed, fp16)
    bpc = (NB + NAGC - 1) // NAGC
    blk_of_chunk = [list(range(c * bpc, min(NB, (c + 1) * bpc)))
                    for c in range(NAGC)]
    chunk_cols = []
    g1h_c, Tag_c = [], []
    for c in range(NAGC):
        c0 = blk_of_chunk[c][0] * PART
        c1 = min(NPC, (blk_of_chunk[c][-1] + 1) * PART)
        chunk_cols.append((c0, c1))
        g1h_c.append(nc.dram_tensor(f"g1h_{c}", [HC, c1 - c0], F16).ap())
        Tag_c.append(nc.dram_tensor(f"Tag_{c}", [NCORES * HC, c1 - c0], F16,
                                    addr_space="Shared").ap())

    rgroups = [list(range(NCORES))]

    class _PhaseStopE(Exception):
        pass

    with tile.TileContext(nc) as tc:
      try:
        # ---------- shared constant tiles ----------
        with tc.tile_pool(name="const", bufs=1) as cpool:
            ident = cpool.tile([PART, PART], F32)
            make_identity(nc, ident[:])
            iota_i = cpool.tile([PART, PART], mybir.dt.int32)
            nc.gpsimd.iota(iota_i[:], pattern=[[1, PART]], base=0,
                           channel_multiplier=0)
            iota_h = cpool.tile([PART, PART], F16)
            nc.vector.tensor_copy(out=iota_h[:], in_=iota_i[:])
            iota16 = cpool.tile([PART, 16], F16)
            nc.vector.tensor_copy(out=iota16[:], in_=iota_i[:, 0:16])

            cnt_sb = cpool.tile([1, NB * 8], I32)
            nc.sync.dma_start(out=cnt_sb[:], in_=cnt_d[:, :])
            b1cv = cpool.tile([PART, 2], F32)
            nc.sync.dma_start(out=b1cv[:], in_=b1c_d[:, :])
            b2cv = cpool.tile([PART, 2], F32)
            nc.sync.dma_start(out=b2cv[:], in_=b2c_d[:, :])
            iotap = cpool.tile([PART, 1], F32)
            nc.sync.dma_start(out=iotap[:], in_=iop_d[:, :])

            # ---------- dense L1 (replicated: full table on every core) ----
            _sc = nc.enter_named_scope("dense1", False)[0]
            with tc.tile_pool(name="d1", bufs=3) as dp, \
                 tc.tile_pool(name="d1w", bufs=1) as wp, \
                 tc.tile_pool(name="d1x", bufs=2) as xp, \
                 tc.tile_pool(name="d1ps", bufs=3, space="PSUM") as pp:
                W1_sb = wp.tile([F_IN, HC], F16)
                nc.sync.dma_start(out=W1_sb[:], in_=W1_d[:, :])
                B1_sb = wp.tile([F_IN, 8], F16)
                nc.sync.dma_start(out=B1_sb[:], in_=B1_d[:, :])
                XCH = 6272                      # x column chunk (49 blocks)
                x_sb = None
                row8 = al8 = None
                for b in range(NBLK):
                    if b % 49 == 0:
                        x_sb = xp.tile([F_IN, XCH], F16, tag="xsb")
                        x0 = b * PART
                        nc.sync.dma_start(out=x_sb[:, 0:min(XCH, NPAD - x0)],
                                          in_=xh_d[:, x0:min(x0 + XCH, NPAD)])
                    k = b % GRP
                    if k == 0:
                        row8 = dp.tile([PART, GRP, HC + 8], F16, tag="row8")
                        al8 = dp.tile([PART, GRP, 4], F32, tag="al8")
                    col = (b % 49) * PART
                    j = b % 2
                    if j == 0:
                        ps_h = pp.tile([PART, 2, HC], F32, tag="dpsh")
                        ps_al = pp.tile([PART, 2, 8], F32, tag="dpsal")
                    nc.tensor.matmul(ps_h[:, j, :], lhsT=x_sb[:, col:col + PART],
                                     rhs=W1_sb[:], start=True, stop=True)
                    nc.tensor.matmul(ps_al[:, j, :], lhsT=x_sb[:, col:col + PART],
                                     rhs=B1_sb[:], start=True, stop=True)
                    if j == 1 or b == NBLK - 1:
                        nj = j + 1
                        k0 = k - j
                        nc.scalar.activation(out=row8[:, k0:k0 + nj, 0:HC],
                                             in_=ps_h[:, 0:nj, :], func=ACTF.Copy)
                        nc.vector.tensor_copy(
                            out=row8[:, k0:k0 + nj, HC:HC + 8].bitcast(F32),
                            in_=ps_al[:, 0:nj, 0:4])
                        nc.vector.tensor_copy(out=al8[:, k0:k0 + nj, :],
                                              in_=ps_al[:, 0:nj, 4:8])
                    if k == GRP - 1 or b == NBLK - 1:
                        ng = k + 1
                        n0 = (b - k) * PART
                        nc.sync.dma_start(
                            out=T1b[n0:n0 + ng * PART, 0:HC + 8].rearrange(
                                "(k p) w -> p k w", p=PART),
                            in_=row8[:, 0:ng, :])
                        nc.sync.dma_start(
                            out=al1pk[n0:n0 + ng * PART, :].rearrange(
                                "(k p) c -> p k c", p=PART),
                            in_=al8[:, 0:ng, :])
            nc.leave_named_scope("dense1", _sc, False)

            # ---------- edge phase (shared for both layers) ----------
            def edge_phase(Tbl, alpk, bias_cv, outT, relu, scope, ag=False):
                """outT: None for L1 (writes g1h chunks), else x2T."""
                _es = nc.enter_named_scope(scope, False)[0]
                alview = alpk[:, :].rearrange("(r j) c -> r (j c)", j=16)
                with tc.tile_pool(name="eidx", bufs=2) as ip, \
                     tc.tile_pool(name="eg", bufs=2) as gp, \
                     tc.tile_pool(name="ew", bufs=2) as wp2, \
                     tc.tile_pool(name="eps", bufs=2, space="PSUM") as ep, \
                     tc.tile_pool(name="etps", bufs=2, space="PSUM") as tps:
                    # pre-zero both gather buffers: -1-skipped slots must hold
                    # finite floats (uninitialized SBUF could be NaN -> NaN*0
                    # = NaN in PSUM)
                    for _z in range(2):
                        zt = gp.tile([PART, KT, RWH], F16, tag="gall")
                        nc.vector.memset(zt[:], 0.0)

                    cnt_regs = [nc.gpsimd.alloc_register(f"cnt_{scope}_{i}")
                                for i in range(4)]
                    reg_rr = [0]

                    def gather(gtile, src_ap, ixtile, ktot, elem, cnt_base):
                        for ci, c0 in enumerate(range(0, ktot, CHUNK)):
                            cw = min(CHUNK, ktot - c0)
                            reg = cnt_regs[reg_rr[0] % 4]
                            reg_rr[0] += 1
                            nc.gpsimd.reg_load(
                                reg, cnt_sb[0:1, cnt_base + ci:cnt_base + ci + 1])
                            nc.gpsimd.dma_gather(
                                out_ap=gtile[:, c0:c0 + cw, :],
                                in_ap=src_ap, idxs_ap=ixtile[:, c0 * 8:(c0 + cw) * 8],
                                num_idxs=cw * PART, num_idxs_reg=reg,
                                elem_size=elem)

                    for b in range(NB):
                        mb = min(PART, NPC - b * PART)
                        ib = ip.tile([PART, IDXW], I16, tag="ib")
                        nc.sync.dma_start(out=ib[:], in_=ib_d[b, :, :])
                        il = ib[:, 0:KLO * 8]
                        ih = ib[:, KLO * 8:KT * 8]
                        dl = ib[:, KT * 8:KT * 9].bitcast(F16)
                        dmn = ib[:, KT * 9:KT * 9 + 1].bitcast(F16)
                        ixn = ib[:, KT * 9 + 1:KT * 9 + 9]
                        dlF = ip.tile([PART, KT * PART], F16, tag="dlF")
                        nc.sync.dma_start(
                            out=dlF[:],
                            in_=dlT_d[b, 0:1, :].to_broadcast([PART, KT * PART]))

                        gall = gp.tile([PART, KT, RWH], F16, tag="gall")
                        gather(gall[:, 0:KLO, :], Tbl[0:SPLIT, :], il, KLO, RWH, b * 8)
                        gather(gall[:, KLO:KT, :], Tbl[SPLIT:NPAD, :], ih, KHI, RWH,
                               b * 8 + CL)
                        # one 256-B packed al_dst row per OWN dst node (128
                        # descriptors instead of one per edge)
                        gan = gp.tile([PART, 1, 64], F32, tag="gan")
                        gather(gan, alview, ixn, 1, 64, b * 8 + CL + CH)

                        # selector matrix S01[e, kt, d] = (dl == d), fp16
                        S01 = wp2.tile([PART, KT, PART], F16, tag="S01")
                        nc.vector.tensor_tensor(
                            out=S01[:],
                            in0=dl[:].unsqueeze(-1).to_broadcast([PART, KT, PART]),
                            in1=iota_h[:].unsqueeze(1).to_broadcast([PART, KT, PART]),
                            op=ALU.is_equal)

                        # al_dst per own node: one-hot over the 16-node pack
                        ohn = wp2.tile([PART, 16], F32, tag="ohn")
                        nc.vector.tensor_tensor(
                            out=ohn[:],
                            in0=dmn[:].to_broadcast([PART, 16]),
                            in1=iota16[:], op=ALU.is_equal)
                        adn = wp2.tile([PART, 4, 16], F32, tag="adn")
                        nc.vector.tensor_tensor(
                            out=adn[:],
                            in0=gan[:, 0, :].rearrange("p (j h) -> p h j", j=16),
                            in1=ohn[:].unsqueeze(1).to_broadcast([PART, 4, 16]),
                            op=ALU.mult)
                        adstb32 = wp2.tile([PART, 4], F32, tag="adstb32")
                        nc.vector.tensor_reduce(
                            out=adstb32[:].unsqueeze(-1), in_=adn[:],
                            axis=mybir.AxisListType.X, op=ALU.add)
                        adstb = wp2.tile([PART, 4], F16, tag="adstb")
                        nc.vector.tensor_copy(out=adstb[:], in_=adstb32[:])
                        # transposed selector S01T[d, i] = (dl_i == d); PE
                        # looks up al_dst per edge: adps[e,h] = sum_d S01T*adst
                        S01T = wp2.tile([PART, KT * PART], F16, tag="S01T")
                        nc.vector.scalar_tensor_tensor(
                            out=S01T[:], in0=dlF[:], scalar=iotap[:, 0:1],
                            in1=dlF[:], op0=ALU.is_equal, op1=ALU.bypass)
                        adps = ep.tile([PART, KT * 4], F32, tag="adps")
                        for e in range(KT):
                            nc.tensor.matmul(adps[:, e * 4:(e + 1) * 4],
                                             lhsT=S01T[:, e * PART:(e + 1) * PART],
                                             rhs=adstb[:], start=True, stop=True)
                        Z = wp2.tile([PART, KT, 4], F32, tag="Z")
                        nc.vector.tensor_tensor(
                            out=Z[:],
                            in0=adps[:].rearrange("p (k h) -> p k h", h=4),
                            in1=gall[:, :, HC:HC + 8].bitcast(F32), op=ALU.add)
                        # leaky-relu (one fused op), clamp, exp -> fp16
                        nc.vector.scalar_tensor_tensor(
                            out=Z[:], in0=Z[:], scalar=NEG_SLOPE, in1=Z[:],
                            op0=ALU.mult, op1=ALU.max)
                        nc.vector.tensor_scalar_min(out=Z[:], in0=Z[:], scalar1=ZCLAMP)
                        EXh = wp2.tile([PART, KT, 4], F16, tag="EXh")
                        nc.scalar.activation(out=EXh[:], in_=Z[:], func=ACTF.Exp)

                        # Hp = [ex-weighted h | ex] (fp16, (c,h)-interleaved)
                        Hp = wp2.tile([PART, KT, 260], F16, tag="Hp")
                        nc.vector.tensor_tensor(
                            out=Hp[:, :, 0:HC].rearrange("p k (c h) -> p k c h", h=H),
                            in0=gall[:, :, 0:HC].rearrange("p k (c h) -> p k c h", h=H),
                            in1=EXh[:].unsqueeze(2).to_broadcast([PART, KT, C0, H]),
                            op=ALU.mult)
                        nc.vector.tensor_copy(out=Hp[:, :, HC:HC + 4], in_=EXh[:])

                        acc = ep.tile([PART, 260], F32, tag="acc")
                        for e in range(KT):
                            nc.tensor.matmul(acc[:], lhsT=S01[:, e, :], rhs=Hp[:, e, :],
                                             start=(e == 0), stop=(e == KT - 1))

                        dn = wp2.tile([PART, 4], F32, tag="dn")
                        nc.vector.tensor_scalar_add(out=dn[:], in0=acc[:, HC:HC + 4],
                                                    scalar1=1e-16)
                        rec = wp2.tile([PART, 4], F32, tag="rec")
                        nc.vector.reciprocal(out=rec[:], in_=dn[:])
                        ob = wp2.tile([PART, HC], F32, tag="ob")
                        nc.vector.tensor_tensor(
                            out=ob[:].rearrange("p (c h) -> p c h", h=H),
                            in0=acc[:, 0:HC].rearrange("p (c h) -> p c h", h=H),
                            in1=rec[:].unsqueeze(1).to_broadcast([PART, C0, H]),
                            op=ALU.mult)
                        for ct in range(2):
                            tp = tps.tile([PART, PART], F32, tag="ttp")
                            nc.tensor.transpose(out=tp[:], in_=ob[:, ct * PART:(ct + 1) * PART],
                                                identity=ident[:])
                            tsh = wp2.tile([PART, PART], F16, tag="tsh")
                            nc.scalar.activation(out=tsh[:], in_=tp[:],
                                                 func=ACTF.Relu if relu else ACTF.Identity,
                                                 bias=bias_cv[:, ct:ct + 1])
                            if outT is None:
                                ci = min(b // bpc, NAGC - 1)
                                cc0 = chunk_cols[ci][0]
                                nc.sync.dma_start(
                                    out=g1h_c[ci][ct * PART:(ct + 1) * PART,
                                                  b * PART - cc0:b * PART - cc0 + mb],
                                    in_=tsh[:, 0:mb])
                            else:
                                nc.sync.dma_start(
                                    out=outT[ct * PART:(ct + 1) * PART,
                                             b * PART:b * PART + mb],
                                    in_=tsh[:, 0:mb])
                        if ag and not NO_COLL:
                            ci = min(b // bpc, NAGC - 1)
                            if b == blk_of_chunk[ci][-1]:
                                if sim_local:
                                    for r_ in range(NCORES):
                                        nc.sync.dma_start(
                                            out=Tag_c[ci][r_ * HC:(r_ + 1) * HC, :],
                                            in_=g1h_c[ci][:, :])
                                else:
                                    nc.gpsimd.collective_compute(
                                        "AllGather", ALU.bypass,
                                        replica_groups=rgroups,
                                        ins=[g1h_c[ci][:, :]],
                                        outs=[Tag_c[ci][:, :]])
                nc.leave_named_scope(scope, _es, False)

            if PHASES >= 2:
                edge_phase(T1b, al1pk, b1cv, None, relu=False, scope="edge1",
                           ag=True)

            # ---------- BN stats + AllReduce ----------
            if PHASES < 3:
                raise _PhaseStopE
            _sc = nc.enter_named_scope("bnstat", False)[0]
            with tc.tile_pool(name="st", bufs=1) as sp, \
                 tc.tile_pool(name="stw", bufs=1) as sw:
                stats = sw.tile([PART, 4], F32)
                for ct in range(2):
                    gt = sp.tile([PART, NPC], F16, tag="gt")
                    for ci in range(NAGC):
                        cc0, cc1 = chunk_cols[ci]
                        nc.sync.dma_start(
                            out=gt[:, cc0:cc1],
                            in_=g1h_c[ci][ct * PART:(ct + 1) * PART, :])
                    nc.vector.tensor_reduce(out=stats[:, ct:ct + 1], in_=gt[:],
                                            axis=mybir.AxisListType.X, op=ALU.add)
                    sq = sp.tile([PART, NPC], F32, tag="sq")
                    nc.scalar.activation(out=sq[:], in_=gt[:], func=ACTF.Square)
                    nc.vector.tensor_reduce(out=stats[:, 2 + ct:3 + ct], in_=sq[:],
                                            axis=mybir.AxisListType.X, op=ALU.add)
                nc.sync.dma_start(out=ar_in[:, :], in_=stats[:])
            nc.leave_named_scope("bnstat", _sc, False)

            _sc = nc.enter_named_scope("ar", False)[0]
            if sim_local:
                nc.sync.dma_start(out=ar_out[:, :], in_=ar_in[:, :])
            else:
                nc.gpsimd.collective_compute(
                    "AllReduce", ALU.add, replica_groups=rgroups,
                    ins=[ar_in[:, :]], outs=[ar_out[:, :]])
            nc.leave_named_scope("ar", _sc, False)

            with tc.tile_pool(name="bnw", bufs=1) as bw:
                ar_sb = bw.tile([PART, 4], F32)
                nc.sync.dma_start(out=ar_sb[:], in_=ar_out[:, :])
                mean = bw.tile([PART, 2], F32)
                nc.vector.tensor_scalar_mul(out=mean[:], in0=ar_sb[:, 0:2], scalar1=1.0 / N)
                msq = bw.tile([PART, 2], F32)
                nc.vector.tensor_scalar_mul(out=msq[:], in0=ar_sb[:, 2:4], scalar1=1.0 / N)
                var = bw.tile([PART, 2], F32)
                nc.vector.tensor_tensor(out=var[:], in0=mean[:], in1=mean[:], op=ALU.mult)
                nc.vector.tensor_tensor(out=var[:], in0=msq[:], in1=var[:], op=ALU.subtract)
                nc.vector.tensor_scalar_add(out=var[:], in0=var[:], scalar1=BN_EPS)
                sd = bw.tile([PART, 2], F32)
                nc.scalar.activation(out=sd[:], in_=var[:], func=ACTF.Sqrt)
                rinv = bw.tile([PART, 2], F32)
                nc.vector.reciprocal(out=rinv[:], in_=sd[:])
                gc = bw.tile([PART, 2], F32)
                nc.sync.dma_start(out=gc[:], in_=gcol_d[:, :])
                bc = bw.tile([PART, 2], F32)
                nc.sync.dma_start(out=bc[:], in_=bcol_d[:, :])
                scale_c = bw.tile([PART, 2], F32)
                nc.vector.tensor_tensor(out=scale_c[:], in0=gc[:], in1=rinv[:], op=ALU.mult)
                shift_c = bw.tile([PART, 2], F32)
                nc.vector.tensor_tensor(out=shift_c[:], in0=mean[:], in1=scale_c[:], op=ALU.mult)
                nc.vector.tensor_tensor(out=shift_c[:], in0=bc[:], in1=shift_c[:], op=ALU.subtract)

                # ---------- dense L2 (replicated, from AllGathered x1) -----
                if PHASES < 4:
                    raise _PhaseStopE
                _sc = nc.enter_named_scope("dense2", False)[0]
                with tc.tile_pool(name="d2", bufs=3) as dp2, \
                     tc.tile_pool(name="d2w", bufs=1) as wp3, \
                     tc.tile_pool(name="d2x", bufs=2) as xp2, \
                     tc.tile_pool(name="d2ps", bufs=3, space="PSUM") as pp2:
                    W2_sb = [wp3.tile([PART, HC], F16, tag=f"w2_{kt}", name=f"w2_{kt}")
                             for kt in range(2)]
                    B2_sb = [wp3.tile([PART, 8], F16, tag=f"b2_{kt}", name=f"b2_{kt}")
                             for kt in range(2)]
                    for kt in range(2):
                        nc.sync.dma_start(out=W2_sb[kt][:],
                                          in_=W2_d[kt * PART:(kt + 1) * PART, :])
                        nc.sync.dma_start(out=B2_sb[kt][:],
                                          in_=B2_d[kt * PART:(kt + 1) * PART, :])
                    for r_ in range(NCORES):
                        for ci in range(NAGC):
                            cc0, cc1 = chunk_cols[ci]
                            w = cc1 - cc0
                            xs = []
                            for kt in range(2):
                                gl = xp2.tile([PART, bpc * PART], F16, tag=f"gl{kt}",
                                              name=f"gl{kt}")
                                nc.sync.dma_start(
                                    out=gl[:, 0:w],
                                    in_=Tag_c[ci][r_ * HC + kt * PART:r_ * HC + (kt + 1) * PART, :])
                                x1s = xp2.tile([PART, bpc * PART], F16, tag=f"x1s{kt}",
                                               name=f"x1s{kt}")
                                nc.scalar.activation(out=x1s[:, 0:w], in_=gl[:, 0:w],
                                                     func=ACTF.Relu,
                                                     bias=shift_c[:, kt:kt + 1],
                                                     scale=scale_c[:, kt:kt + 1])
                                xs.append(x1s)
                            nblk2 = (w + PART - 1) // PART
                            row8 = al8 = None
                            for bl in range(nblk2):
                                lb = bl * PART
                                mb2 = min(PART, w - lb)
                                k = bl % GRP
                                if k == 0:
                                    row8 = dp2.tile([PART, GRP, HC + 8], F16, tag="d2row8")
                                    al8 = dp2.tile([PART, GRP, 4], F32, tag="d2al8")
                                j = bl % 2
                                if j == 0:
                                    ps_h = pp2.tile([PART, 2, HC], F32, tag="d2psh")
                                    ps_al = pp2.tile([PART, 2, 8], F32, tag="d2psal")
                                for kt in range(2):
                                    nc.tensor.matmul(ps_h[0:mb2, j, :],
                                                     lhsT=xs[kt][:, lb:lb + mb2],
                                                     rhs=W2_sb[kt][:],
                                                     start=(kt == 0), stop=(kt == 1))
                                for kt in range(2):
                                    nc.tensor.matmul(ps_al[0:mb2, j, :],
                                                     lhsT=xs[kt][:, lb:lb + mb2],
                                                     rhs=B2_sb[kt][:],
                                                     start=(kt == 0), stop=(kt == 1))
                                if j == 1 or bl == nblk2 - 1:
                                    nj = j + 1
                                    k0 = k - j
                                    nc.scalar.activation(out=row8[:, k0:k0 + nj, 0:HC],
                                                         in_=ps_h[:, 0:nj, :],
                                                         func=ACTF.Copy)
                                    nc.vector.tensor_copy(
                                        out=row8[:, k0:k0 + nj, HC:HC + 8].bitcast(F32),
                                        in_=ps_al[:, 0:nj, 0:4])
                                    nc.vector.tensor_copy(out=al8[:, k0:k0 + nj, :],
                                                          in_=ps_al[:, 0:nj, 4:8])
                                if k == GRP - 1 or bl == nblk2 - 1:
                                    # batch-flush the full blocks; a ragged
                                    # tail block (mb2 < PART) is written solo
                                    ng = k + (1 if mb2 == PART else 0)
                                    n0 = r_ * NPC + cc0 + (bl - k) * PART
                                    if ng > 0:
                                        nc.sync.dma_start(
                                            out=T2b[n0:n0 + ng * PART, 0:HC + 8].rearrange(
                                                "(k p) w -> p k w", p=PART),
                                            in_=row8[:, 0:ng, :])
                                        nc.sync.dma_start(
                                            out=al2pk[n0:n0 + ng * PART, :].rearrange(
                                                "(k p) c -> p k c", p=PART),
                                            in_=al8[:, 0:ng, :])
                                    if mb2 < PART:
                                        nr = n0 + k * PART
                                        nc.sync.dma_start(
                                            out=T2b[nr:nr + mb2, 0:HC + 8],
                                            in_=row8[0:mb2, k, :])
                                        nc.sync.dma_start(
                                            out=al2pk[nr:nr + mb2, :],
                                            in_=al8[0:mb2, k, :])
                nc.leave_named_scope("dense2", _sc, False)

                if PHASES < 5:
                    raise _PhaseStopE
                edge_phase(T2b, al2pk, b2cv, x2T, relu=True, scope="edge2")

                # ---------- pooling ----------
                if PHASES < 6:
                    raise _PhaseStopE
                _sc = nc.enter_named_scope("pool", False)[0]
                with tc.tile_pool(name="pl", bufs=1) as pl:
                    mk = pl.tile([PART, NPC], F32, tag="mk")
                    nc.sync.dma_start(out=mk[:], in_=mask_d[0:1, :].to_broadcast([PART, NPC]))
                    cv = pl.tile([PART, NPC], F32, tag="cv")
                    nc.sync.dma_start(out=cv[:], in_=cinv_d[0:1, :].to_broadcast([PART, NPC]))
                    for ct in range(4):
                        xt = pl.tile([PART, NPC], F32, tag="xt")
                        if ct < 2:
                            gld = pl.tile([PART, NPC], F16, tag="gld")
                            for ci in range(NAGC):
                                cc0, cc1 = chunk_cols[ci]
                                nc.sync.dma_start(
                                    out=gld[:, cc0:cc1],
                                    in_=g1h_c[ci][ct * PART:(ct + 1) * PART, :])
                            nc.scalar.activation(out=xt[:], in_=gld[:], func=ACTF.Relu,
                                                 bias=shift_c[:, ct:ct + 1],
                                                 scale=scale_c[:, ct:ct + 1])
                        else:
                            x2l = pl.tile([PART, NPC], F16, tag="x2l")
                            nc.sync.dma_start(out=x2l[:],
                                              in_=x2T[(ct - 2) * PART:(ct - 1) * PART, :])
                            nc.scalar.activation(out=xt[:], in_=x2l[:], func=ACTF.Copy)
                        sm = pl.tile([PART, NPC], F32, tag="sm")
                        nc.vector.tensor_tensor_scan(out=sm[:], data0=mk[:], data1=xt[:],
                                                     initial=0.0, op0=ALU.mult, op1=ALU.max)
                        nc.sync.dma_start(out=omax_d[ct * PART:(ct + 1) * PART, :], in_=sm[:])
                        ss = pl.tile([PART, NPC], F32, tag="ss")
                        nc.vector.tensor_tensor_scan(out=ss[:], data0=mk[:], data1=xt[:],
                                                     initial=0.0, op0=ALU.mult, op1=ALU.add)
                        nc.vector.tensor_tensor(out=ss[:], in0=ss[:], in1=cv[:], op=ALU.mult)
                        nc.sync.dma_start(out=omean_d[ct * PART:(ct + 1) * PART, :], in_=ss[:])
                nc.leave_named_scope("pool", _sc, False)

      except _PhaseStopE:
        pass

    nc.compile()
    return nc


# --------------------------------------------------------------------------
# host-side combine
# --------------------------------------------------------------------------

def postprocess(results, meta):
    lastcol = meta["lastcol"]
    mean = np.zeros((G, 2 * HC), np.float32)
    mx = np.zeros((G, 2 * HC), np.float32)
    for r in range(NCORES):
        om = results[r]["out_mean"]   # [512, NPC], dev channel order
        ox = results[r]["out_max"]
        for g_, col in lastcol[r].items():
            mean[g_] += om[:, col]
            mx[g_] = np.maximum(mx[g_], ox[:, col])
    # un-permute dev channel order back to torch order
    dev2orig = np.concatenate([PERM, HC + PERM])
    mean_o = np.empty_like(mean); mx_o = np.empty_like(mx)
    mean_o[:, dev2orig] = mean
    mx_o[:, dev2orig] = mx
    return np.concatenate([mean_o, mx_o], axis=1).astype(np.float32)


_CACHE = {}


def kernel(**inputs):
    in_maps, meta = preprocess(**inputs)
    key = (meta["NB"], meta["KLO"], meta["KHI"])
    if key not in _CACHE:
        _CACHE[key] = build_program(meta)
    nc = _CACHE[key]
    res = bass_utils.run_bass_kernel_spmd(nc, in_maps, core_ids=list(range(NCORES)))
    return postprocess(res.results, meta)
